# revision 1
# baseline (speedup 1.0000x reference)
import numpy as np

import concourse.bass as bass
import concourse.mybir as mybir
from concourse.bass_utils import run_bass_kernel_spmd

# nn_NeuralGCDE dims (hardcoded)
B, N, T = 16, 512, 12
IN, HID, HH, EMB, K, OUT = 2, 32, 32, 16, 2, 12
NCORES = 8
BS = B // NCORES          # 2 batch elems per core
R = BS * N                # 1024 rows per core

_cache = {}


# ---------------- host: ODE integration up to z_T (numpy) ----------------
def _zT_host(times, coeff_a, coeff_b, coeff_c2, coeff_d3, Wh, bh, Wz, bz,
             fWin, fbin, fWmid, fbmid, fWout, fbout,
             gWin, gbin, gE, gWpool, gbpool, gWout, gbout):
    maxlen = coeff_b.shape[2] - 1

    def dXdt(t):
        idx = int(np.clip(np.sum(t > times) - 1, 0, maxlen))
        frac = np.float32(t - times[idx])
        return coeff_b[:, :, idx] + (coeff_c2[:, :, idx]
                                     + coeff_d3[:, :, idx] * frac) * frac

    G = np.maximum(gE @ gE.T, 0.0)
    Gm = np.exp(G - G.max(axis=1, keepdims=True))
    A = Gm / Gm.sum(axis=1, keepdims=True)
    supports = [np.eye(N, dtype=np.float32), A]
    for _ in range(2, K):
        supports.append(2.0 * A @ supports[-1] - supports[-2])
    aw = np.einsum('nd,dkio->nkio', gE, gWpool).astype(np.float32)
    ab = gE @ gbpool

    def func_f(h):
        x = np.maximum(h @ fWin + fbin, 0.0)
        x = np.maximum(x @ fWmid + fbmid, 0.0)
        return np.tanh((x @ fWout + fbout).reshape(B, N, HID, IN))

    def func_g(z):
        x = np.maximum(z @ gWin + gbin, 0.0)
        xg = np.stack([x, np.matmul(A, x)], axis=2)
        x = np.einsum('bnki,nkio->bno', xg, aw, optimize=True) + ab
        return np.tanh((x @ gWout + gbout).reshape(B, N, HID, HID))

    def vfield(t, h, z):
        dX = dXdt(t)
        vf = func_f(h)
        vg = func_g(z)
        dh = np.matmul(vf, dX[..., None])[..., 0]
        dz = np.matmul(vg, dh[..., None])[..., 0]
        return dh, dz

    x0 = coeff_a[:, :, 0, :]
    h = x0 @ Wh + bh
    z = x0 @ Wz + bz
    for s in range(T - 1):
        t0, t1 = times[s], times[s + 1]
        dt = t1 - t0
        third = dt / 3.0
        k1h, k1z = vfield(t0, h, z)
        k2h, k2z = vfield(t0 + third, h + third * k1h, z + third * k1z)
        k3h, k3z = vfield(t0 + 2.0 * third,
                          h + dt * (k2h - k1h / 3.0), z + dt * (k2z - k1z / 3.0))
        k4h, k4z = vfield(t1,
                          h + dt * (k1h - k2h + k3h), z + dt * (k1z - k2z + k3z))
        h = h + dt * 0.125 * (k1h + 3.0 * (k2h + k3h) + k4h)
        z = z + dt * 0.125 * (k1z + 3.0 * (k2z + k3z) + k4z)
    return z


# ------------- device: end_conv projection, batch-sharded on 8 cores -------
def _build_conv_kernel():
    """out[o, r] = sum_h convW[o,h] * zT[h, r] + convb[o], r = b*N+n (R rows)."""
    nc = bass.Bass()
    zt = nc.declare_dram_parameter("zt", [HID, R], mybir.dt.float32, isOutput=False)
    cw = nc.declare_dram_parameter("cw", [HID, OUT], mybir.dt.float32, isOutput=False)
    cb = nc.declare_dram_parameter("cb", [OUT, 1], mybir.dt.float32, isOutput=False)
    out = nc.declare_dram_parameter("out", [OUT, R], mybir.dt.float32, isOutput=True)

    NH = R // 512  # fp32 moving-operand free-dim limit is 512

    with (
        nc.sbuf_tensor([HID, R], mybir.dt.float32) as s_zt,
        nc.sbuf_tensor([HID, OUT], mybir.dt.float32) as s_cw,
        nc.sbuf_tensor([OUT, 1], mybir.dt.float32) as s_cb,
        nc.sbuf_tensor([OUT, R], mybir.dt.float32) as s_out,
        nc.psum_tensor([OUT, R], mybir.dt.float32) as p_out,
        nc.semaphore("dma_sem") as dma_sem,
        nc.semaphore("mm_sem") as mm_sem,
        nc.semaphore("v_sem") as v_sem,
        nc.Block() as block,
    ):
        @block.sync
        def _(sync):
            sync.dma_start(out=s_zt[:], in_=zt[:]).then_inc(dma_sem, 16)
            sync.dma_start(out=s_cw[:], in_=cw[:]).then_inc(dma_sem, 16)
            sync.dma_start(out=s_cb[:], in_=cb[:]).then_inc(dma_sem, 16)
            sync.wait_ge(v_sem, NH)
            sync.dma_start(out=out[:], in_=s_out[:]).then_inc(dma_sem, 16)

        @block.tensor
        def _(tensor):
            tensor.wait_ge(dma_sem, 48)
            for j in range(NH):
                nc.tensor.matmul(
                    p_out[:, j * 512:(j + 1) * 512],
                    s_cw[:],
                    s_zt[:, j * 512:(j + 1) * 512],
                    start=True, stop=True,
                ).then_inc(mm_sem, 1)

        @block.vector
        def _(vector):
            for j in range(NH):
                vector.wait_ge(mm_sem, j + 1)
                nc.vector.tensor_scalar_add(
                    s_out[:, j * 512:(j + 1) * 512],
                    p_out[:, j * 512:(j + 1) * 512],
                    s_cb[:],
                ).then_inc(v_sem, 1)

    return nc


def _get_conv_nc():
    if "nc" not in _cache:
        _cache["nc"] = _build_conv_kernel()
    return _cache["nc"]


_ARG_ORDER = ["times", "coeff_a", "coeff_b", "coeff_c2", "coeff_d3", "Wh", "bh",
              "Wz", "bz", "fWin", "fbin", "fWmid", "fbmid", "fWout", "fbout",
              "gWin", "gbin", "gE", "gWpool", "gbpool", "gWout", "gbout"]


def kernel(**inputs):
    a = {k: np.asarray(v, dtype=np.float32) for k, v in inputs.items()}
    zT = _zT_host(*[a[k] for k in _ARG_ORDER])  # (B, N, HID)

    convW = a["convW"]                       # (OUT, HID)
    convb = a["convb"]                       # (OUT,)
    cw = np.ascontiguousarray(convW.T)       # (HID, OUT)
    cb = convb.reshape(OUT, 1).astype(np.float32)

    # batch-shard zT across the 8 cores: core i gets batches [2i, 2i+2)
    in_maps = []
    for i in range(NCORES):
        sh = zT[i * BS:(i + 1) * BS]                      # (BS, N, HID)
        zt = np.ascontiguousarray(
            sh.reshape(R, HID).T).astype(np.float32)      # (HID, R)
        in_maps.append({"zt": zt, "cw": cw, "cb": cb})

    nc = _get_conv_nc()
    res = run_bass_kernel_spmd(nc, in_maps, core_ids=list(range(NCORES)))

    # gather: per core out (OUT, R) -> (BS, N, OUT)
    full = np.empty((B, 1, N, OUT), dtype=np.float32)
    for i in range(NCORES):
        o = res.results[i]["out"]                         # (OUT, R)
        full[i * BS:(i + 1) * BS, 0] = o.T.reshape(BS, N, OUT)
    return full





# revision 2
# speedup vs baseline: 5.9181x; 5.9181x over previous
"""NeuralGCDE on 8 NeuronCores: full RK4 ODE integration on device.

Sharding: data-parallel over batch B=16 -> 2 batch elements per core
(rows r = b*N + n, R = 1024 per core). All graph/MLP params replicated.

Device layout is feature-major (features on SBUF partitions, rows on the
free dim). Every contraction is a PE matmul; partition-dim reductions and
broadcasts use structured 0/1 matrices as stationary operands. The
softmax adjacency (exp(relu(gE gE^T)) with row scaling) is built on
device; only the row-sum reciprocals (512 floats) come from host.

All one-time work (bass build, neuron compile, PJRT load) happens at
import; kernel(**inputs) does host repacks + one SPMD dispatch.
"""
import numpy as np

import concourse.bass as bass
import concourse.mybir as mybir
import concourse.tile as tile
from concourse.bass_utils import run_bass_kernel_spmd

B, N, T = 16, 512, 12
IN, HID, HH, EMB, K, OUT = 2, 32, 32, 16, 2, 12
NCORES = 8
BS = B // NCORES            # 2
R = BS * N                  # 1024
NSTEP = T - 1               # 11
F32 = mybir.dt.float32
AF = mybir.ActivationFunctionType
ALU = mybir.AluOpType

_NO_SPILL = {"InstEventSemaphore", "InstUnconditionalBranch",
             "InstConditionalBranch"}


def _spill_excess_waits(nc):
    """Walrus ISA structs hold one sync-wait slot on most instructions.
    Tile can emit several. Move excess waits onto InstEventSemaphore
    carriers inserted just before, on the same engine (waiting earlier on
    the same engine stream is always sound)."""
    nspill = 0
    for f in nc.m.functions:
        for blk in f.blocks:
            lst = blk.instructions
            i = 0
            while i < len(lst):
                ins = lst[i]
                si = ins.sync_info
                if (type(ins).__name__ in _NO_SPILL or si is None
                        or not si.on_wait or len(si.on_wait) <= 1):
                    i += 1
                    continue
                waits = list(si.on_wait)
                keep, excess = waits[-1:], waits[:-1]
                ins.sync_info = mybir.SyncInfo(on_wait=keep,
                                               on_update=list(si.on_update))
                carriers = []
                while excess:
                    chunk, excess = excess[:2], excess[2:]
                    es = mybir.InstEventSemaphore(
                        name=f"Wspill-{nspill}", ins=[], outs=[])
                    nspill += 1
                    es.engine = ins.engine
                    es.sync_info = mybir.SyncInfo(on_wait=chunk, on_update=[])
                    carriers.append(es)
                for k_, es in enumerate(carriers):
                    lst.insert(i + k_, es)
                i += len(carriers) + 1
    return nspill


def build_nc(nstep=NSTEP):
    nc = bass.Bass()

    def dp(name, shape, out=False):
        return nc.declare_dram_parameter(name, list(shape), F32, isOutput=out)

    d_dx = dp("dx", (2 * 4 * nstep, R))
    d_h0 = dp("h0", (HID, R))
    d_z0 = dp("z0", (HID, R))
    d_gExp = dp("gExp", (EMB, N))          # gE^T
    d_gET2 = dp("gET2", (EMB, R))          # gE^T tiled over b
    d_Gsel = dp("Gsel", (EMB, 4 * 128))    # gE_part selector
    d_fw1 = dp("fw1", (HID, HH))
    d_fb1 = dp("fb1", (HH, 1))
    d_fw2 = dp("fw2", (HH, HH))
    d_fb2 = dp("fb2", (HH, 1))
    d_fw3 = dp("fw3", (HH, HID * IN))      # columns reordered i-major
    d_fb3 = dp("fb3", (HID * IN, 1))
    d_gw1 = dp("gw1", (HID, HH))
    d_gb1 = dp("gb1", (HH, 1))
    d_wpk0 = dp("wpk0", (HH, EMB * HH))    # gWpool k=0, (32, 512)
    d_wpk1 = dp("wpk1", (HH, EMB * HH))
    d_abf = dp("abf", (HH, R))
    d_gwo = dp("gwo", (HH, HID * HID))
    d_gbo = dp("gbo", (128, 8))            # gbout chunked
    d_Bc = dp("Bc", (IN, IN * HID))        # broadcast dX across i-major
    d_S3 = dp("S3", (IN * HID, HID))       # reduce over i
    d_Erep = dp("Erep", (HID, 128))        # replicate dh 4x
    d_Sdz = dp("Sdz", (128, 8 * HID))      # per-chunk reduce for dz
    d_S2 = dp("S2", (128, HH))             # reduce over d
    d_I32 = dp("I32", (HH, HH))
    d_ident = dp("ident", (128, 128))
    d_recip = dp("recip", (128, 4))        # softmax row-sum reciprocals
    d_cw = dp("cw", (HID, OUT))
    d_cb = dp("cb", (OUT, 1))
    d_out = dp("out", (OUT, R), out=True)

    C5 = 512  # fp32 moving-operand free-dim limit

    from contextlib import ExitStack
    with ExitStack() as es:
        tc = es.enter_context(tile.TileContext(nc))
        sgl = es.enter_context(tc.tile_pool(name="sgl", bufs=1))
        wrk = es.enter_context(tc.tile_pool(name="wrk", bufs=1))
        big1 = es.enter_context(tc.tile_pool(name="big1", bufs=1))
        big2 = es.enter_context(tc.tile_pool(name="big2", bufs=2))
        dxp = es.enter_context(tc.tile_pool(name="dxp", bufs=2))
        pA = es.enter_context(tc.tile_pool(name="pA", bufs=2, space="PSUM"))
        pB = es.enter_context(tc.tile_pool(name="pB", bufs=1, space="PSUM"))
        pT = es.enter_context(tc.tile_pool(name="pT", bufs=2, space="PSUM"))

        def load(dram, shape, name):
            t = sgl.tile(list(shape), F32, tag=name, name=name)
            nc.sync.dma_start(out=t[:], in_=dram[:])
            return t

        fw1 = load(d_fw1, (HID, HH), "fw1")
        fb1 = load(d_fb1, (HH, 1), "fb1")
        fw2 = load(d_fw2, (HH, HH), "fw2")
        fb2 = load(d_fb2, (HH, 1), "fb2")
        fw3 = load(d_fw3, (HH, HID * IN), "fw3")
        fb3 = load(d_fb3, (HID * IN, 1), "fb3")
        gw1 = load(d_gw1, (HID, HH), "gw1")
        gb1 = load(d_gb1, (HH, 1), "gb1")
        wpk0 = load(d_wpk0, (HH, EMB * HH), "wpk0")
        wpk1 = load(d_wpk1, (HH, EMB * HH), "wpk1")
        abf = load(d_abf, (HH, R), "abf")
        gwo = load(d_gwo, (HH, HID * HID), "gwo")
        gbo = load(d_gbo, (128, 8), "gbo")
        Bc = load(d_Bc, (IN, IN * HID), "Bc")
        S3 = load(d_S3, (IN * HID, HID), "S3")
        Erep = load(d_Erep, (HID, 128), "Erep")
        Sdz = load(d_Sdz, (128, 8 * HID), "Sdz")
        S2 = load(d_S2, (128, HH), "S2")
        I32 = load(d_I32, (HH, HH), "I32")
        ident = load(d_ident, (128, 128), "ident")
        cw = load(d_cw, (HID, OUT), "cw")
        cb = load(d_cb, (OUT, 1), "cb")
        gExp = load(d_gExp, (EMB, N), "gExp")
        gET2 = load(d_gET2, (EMB, R), "gET2")
        Gsel = load(d_Gsel, (EMB, 4 * 128), "Gsel")
        recip = load(d_recip, (128, 4), "recip")

        def mm2(ps, lhsT, rhs, start=True, stop=True):
            for c in range(2):
                nc.tensor.matmul(ps[:, c * C5:(c + 1) * C5], lhsT,
                                 rhs[:, c * C5:(c + 1) * C5],
                                 start=start, stop=stop)

        def act(out, in_, func, bias=0.0):
            nc.scalar.activation(out, in_, func, bias=bias)

        # ---- adjacency: expG chunks (exp(relu(gE gE^T)), m-major) ----
        expG = []
        for i in range(4):
            gp = pA.tile([128, N], F32, tag="mm", name="mm")
            nc.tensor.matmul(gp[:], gExp[:, i * 128:(i + 1) * 128], gExp[:],
                             start=True, stop=True)
            eg = sgl.tile([128, N], F32, tag=f"expG{i}", name=f"expG{i}")
            act(eg[:], gp[:], AF.Relu)
            act(eg[:], eg[:], AF.Exp)
            expG.append(eg)

        # ---- gE_part chunks: gEp_j[p, r] = gE[n(r), (j*128+p)//32] ----
        gEp = []
        for j in range(4):
            ps = pA.tile([128, R], F32, tag="mm", name="mm")
            mm2(ps, Gsel[:, j * 128:(j + 1) * 128], gET2)
            g = sgl.tile([128, R], F32, tag=f"gEp{j}", name=f"gEp{j}")
            nc.scalar.copy(g[:], ps[:])
            gEp.append(g)

        # ---- state ----
        h = wrk.tile([HID, R], F32, tag="h", name="h", bufs=2)
        z = wrk.tile([HID, R], F32, tag="z", name="z", bufs=2)
        nc.sync.dma_start(out=h[:], in_=d_h0[:])
        nc.sync.dma_start(out=z[:], in_=d_z0[:])

        def vfield(s4, hs, zs, kh, kz):
            dxs = dxp.tile([IN, R], F32, tag="dxs", name="dxs")
            nc.sync.dma_start(out=dxs[:], in_=d_dx[2 * s4:2 * s4 + 2, :])
            # f path: two relu MLP layers + tanh head (i-major columns)
            x1p = pA.tile([HH, R], F32, tag="mm", name="mm")
            mm2(x1p, fw1, hs)
            x1 = wrk.tile([HH, R], F32, tag="fx", name="fx", bufs=2)
            act(x1[:], x1p[:], AF.Relu, bias=fb1[:])
            x2p = pA.tile([HH, R], F32, tag="mm", name="mm")
            mm2(x2p, fw2, x1)
            x2 = wrk.tile([HH, R], F32, tag="fx", name="fx", bufs=2)
            act(x2[:], x2p[:], AF.Relu, bias=fb2[:])
            vfp = pA.tile([HID * IN, R], F32, tag="mm", name="mm")
            mm2(vfp, fw3, x2)
            vf = wrk.tile([HID * IN, R], F32, tag="vf", name="vf")
            act(vf[:], vfp[:], AF.Tanh, bias=fb3[:])
            # dh = sum_i vf_i * dX_i  (dX broadcast via Bc, reduce via S3)
            dXb = pA.tile([IN * HID, R], F32, tag="mm", name="mm")
            mm2(dXb, Bc, dxs)
            nc.vector.tensor_mul(vf[:], vf[:], dXb[:])
            dhp = pB.tile([HID, R], F32, tag="acc", name="acc")
            mm2(dhp, S3, vf)
            nc.scalar.copy(kh[:], dhp[:])
            drp = pA.tile([128, R], F32, tag="mm", name="mm")
            mm2(drp, Erep, kh)
            dhrep = big1.tile([128, R], F32, tag="dhrep", name="dhrep")
            nc.scalar.copy(dhrep[:], drp[:])
            # g path: relu layer (feature-major), node-major transposes
            x1gp = pA.tile([HH, R], F32, tag="mm", name="mm")
            mm2(x1gp, gw1, zs)
            x1g = wrk.tile([HH, R], F32, tag="x1g", name="x1g")
            act(x1g[:], x1gp[:], AF.Relu, bias=gb1[:])
            xT = []
            for k_ in range(4):
                xtp = pT.tile([128, 2 * HH], F32, tag="pt", name="pt")
                for b_ in range(2):
                    nc.tensor.transpose(
                        xtp[:, b_ * HH:(b_ + 1) * HH],
                        x1g[:, b_ * N + k_ * 128: b_ * N + (k_ + 1) * 128],
                        ident[:HH, :HH])
                xt = wrk.tile([128, 2 * HH], F32, tag=f"xT{k_}",
                              name=f"xT{k_}")
                nc.vector.tensor_copy(xt[:], xtp[:])
                xT.append(xt)
            # graph conv: xg1 = A @ x1g per batch, recip folded in
            xg1n = []
            for i in range(4):
                xgp = pT.tile([128, 2 * HH], F32, tag="pt", name="pt")
                for k_ in range(4):
                    nc.tensor.matmul(xgp[:],
                                     expG[k_][:, i * 128:(i + 1) * 128],
                                     xT[k_][:],
                                     start=(k_ == 0), stop=(k_ == 3))
                xn = wrk.tile([128, 2 * HH], F32, tag=f"xg1n{i}",
                              name=f"xg1n{i}")
                nc.vector.tensor_scalar_mul(xn[:], xgp[:], recip[:, i:i + 1])
                xg1n.append(xn)
            xg1f = wrk.tile([HH, R], F32, tag="xg1f", name="xg1f")
            for i in range(4):
                for b_ in range(2):
                    btp = pT.tile([HH, 128], F32, tag="pt", name="pt")
                    nc.tensor.transpose(btp[:],
                                        xg1n[i][:, b_ * HH:(b_ + 1) * HH],
                                        ident[:, :])
                    nc.scalar.copy(
                        xg1f[:, b_ * N + i * 128: b_ * N + (i + 1) * 128],
                        btp[:])
            # per-node pooled weights: y = Wp^T xg scaled by gE_part,
            # reduced over EMB via S2 into x2g (abf preloaded via I32)
            x2gp = pB.tile([HH, R], F32, tag="acc", name="acc")
            for c in range(2):
                nc.tensor.matmul(x2gp[:, c * C5:(c + 1) * C5], I32[:],
                                 abf[:, c * C5:(c + 1) * C5],
                                 start=True, stop=False, skip_group_check=True)
            for j in range(4):
                yp = pA.tile([128, R], F32, tag="mm", name="mm")
                for c in range(2):
                    sl = slice(c * C5, (c + 1) * C5)
                    nc.tensor.matmul(yp[:, sl], wpk0[:, j * 128:(j + 1) * 128],
                                     x1g[:, sl], start=True, stop=False)
                    nc.tensor.matmul(yp[:, sl], wpk1[:, j * 128:(j + 1) * 128],
                                     xg1f[:, sl], start=False, stop=True)
                t_ = big1.tile([128, R], F32, tag="ty", name="ty", bufs=2)
                nc.vector.tensor_mul(t_[:], yp[:], gEp[j][:])
                for c in range(2):
                    sl = slice(c * C5, (c + 1) * C5)
                    nc.tensor.matmul(x2gp[:, sl], S2[:], t_[:, sl],
                                     start=False, stop=(j == 3),
                                     skip_group_check=True)
            x2g = wrk.tile([HH, R], F32, tag="x2g", name="x2g")
            nc.scalar.copy(x2g[:], x2gp[:])
            # vg chunks; dz = sum vg_ho * dh_o accumulated via Sdz
            dzp = pB.tile([HID, R], F32, tag="acc", name="acc")
            for j in range(8):
                vgp = pA.tile([128, R], F32, tag="mm", name="mm")
                mm2(vgp, gwo[:, j * 128:(j + 1) * 128], x2g)
                vg = big2.tile([128, R], F32, tag="vg", name="vg")
                act(vg[:], vgp[:], AF.Tanh, bias=gbo[:, j:j + 1])
                nc.vector.tensor_mul(vg[:], vg[:], dhrep[:])
                for c in range(2):
                    sl = slice(c * C5, (c + 1) * C5)
                    nc.tensor.matmul(dzp[:, sl],
                                     Sdz[:, j * HID:(j + 1) * HID],
                                     vg[:, sl],
                                     start=(j == 0), stop=(j == 7),
                                     skip_group_check=True)
            nc.scalar.copy(kz[:], dzp[:])

        TT = nc.vector.tensor_tensor
        STT = nc.vector.scalar_tensor_tensor

        # RK4 with 3/8 rule, dt = 1 (times are arange; asserted on host)
        for s in range(nstep):
            kh = [wrk.tile([HID, R], F32, tag=f"kh{st}", name=f"kh{st}")
                  for st in range(4)]
            kz = [wrk.tile([HID, R], F32, tag=f"kz{st}", name=f"kz{st}")
                  for st in range(4)]
            vfield(4 * s + 0, h, z, kh[0], kz[0])
            hs = wrk.tile([HID, R], F32, tag="hs", name="hs", bufs=2)
            zs = wrk.tile([HID, R], F32, tag="zs", name="zs", bufs=2)
            STT(hs[:], kh[0][:], 1.0 / 3.0, h[:], op0=ALU.mult, op1=ALU.add)
            STT(zs[:], kz[0][:], 1.0 / 3.0, z[:], op0=ALU.mult, op1=ALU.add)
            vfield(4 * s + 1, hs, zs, kh[1], kz[1])
            hs2 = wrk.tile([HID, R], F32, tag="hs", name="hs", bufs=2)
            zs2 = wrk.tile([HID, R], F32, tag="zs", name="zs", bufs=2)
            STT(hs2[:], kh[0][:], -1.0 / 3.0, kh[1][:],
                op0=ALU.mult, op1=ALU.add)
            TT(hs2[:], hs2[:], h[:], op=ALU.add)
            STT(zs2[:], kz[0][:], -1.0 / 3.0, kz[1][:],
                op0=ALU.mult, op1=ALU.add)
            TT(zs2[:], zs2[:], z[:], op=ALU.add)
            vfield(4 * s + 2, hs2, zs2, kh[2], kz[2])
            hs3 = wrk.tile([HID, R], F32, tag="hs", name="hs", bufs=2)
            zs3 = wrk.tile([HID, R], F32, tag="zs", name="zs", bufs=2)
            STT(hs3[:], kh[1][:], -1.0, kh[0][:], op0=ALU.mult, op1=ALU.add)
            TT(hs3[:], hs3[:], kh[2][:], op=ALU.add)
            TT(hs3[:], hs3[:], h[:], op=ALU.add)
            STT(zs3[:], kz[1][:], -1.0, kz[0][:], op0=ALU.mult, op1=ALU.add)
            TT(zs3[:], zs3[:], kz[2][:], op=ALU.add)
            TT(zs3[:], zs3[:], z[:], op=ALU.add)
            vfield(4 * s + 3, hs3, zs3, kh[3], kz[3])
            hn = wrk.tile([HID, R], F32, tag="h", name="h", bufs=2)
            zn = wrk.tile([HID, R], F32, tag="z", name="z", bufs=2)
            TT(kh[1][:], kh[1][:], kh[2][:], op=ALU.add)
            STT(kh[1][:], kh[1][:], 3.0, kh[0][:], op0=ALU.mult, op1=ALU.add)
            TT(kh[1][:], kh[1][:], kh[3][:], op=ALU.add)
            STT(hn[:], kh[1][:], 0.125, h[:], op0=ALU.mult, op1=ALU.add)
            TT(kz[1][:], kz[1][:], kz[2][:], op=ALU.add)
            STT(kz[1][:], kz[1][:], 3.0, kz[0][:], op0=ALU.mult, op1=ALU.add)
            TT(kz[1][:], kz[1][:], kz[3][:], op=ALU.add)
            STT(zn[:], kz[1][:], 0.125, z[:], op0=ALU.mult, op1=ALU.add)
            h, z = hn, zn

        # ---- end conv ----
        op = pB.tile([OUT, R], F32, tag="acc", name="acc")
        mm2(op, cw, z)
        ob = wrk.tile([OUT, R], F32, tag="x2g", name="x2g")
        nc.vector.tensor_scalar_add(ob[:], op[:], cb[:])
        nc.sync.dma_start(out=d_out[:], in_=ob[:])

    _spill_excess_waits(nc)
    return nc


# ------------------------------------------------------------------
# host-side preprocessing
# ------------------------------------------------------------------
def host_inputs(a, nstep=NSTEP):
    gE = a["gE"]
    times = a["times"]
    assert np.allclose(np.diff(times), 1.0, atol=1e-5), "RK dt=1 baked in"
    maxlen = T - 2
    ts_list = []
    for s in range(nstep):
        t0, t1 = float(times[s]), float(times[s + 1])
        dt = t1 - t0
        ts_list += [t0, t0 + dt / 3.0, t0 + 2.0 * dt / 3.0, t1]
    nst = len(ts_list)
    dX = np.empty((B, N, nst, IN), np.float32)
    for q, t_ in enumerate(ts_list):
        idx = int(np.clip(np.sum(np.float32(t_) > times) - 1, 0, maxlen))
        frac = np.float32(t_ - times[idx])
        dX[:, :, q, :] = (a["coeff_b"][:, :, idx]
                          + (a["coeff_c2"][:, :, idx]
                             + a["coeff_d3"][:, :, idx] * frac) * frac)
    x0 = a["coeff_a"][:, :, 0, :]
    h0 = x0 @ a["Wh"] + a["bh"]
    z0 = x0 @ a["Wz"] + a["bz"]

    Bc = np.zeros((IN, IN * HID), np.float32)
    for i in range(IN):
        Bc[i, i * HID:(i + 1) * HID] = 1.0
    S3 = np.zeros((IN * HID, HID), np.float32)
    for p in range(IN * HID):
        S3[p, p % HID] = 1.0
    Erep = np.zeros((HID, 128), np.float32)
    for p in range(128):
        Erep[p % HID, p] = 1.0
    Sdz = np.zeros((128, 8 * HID), np.float32)
    for j in range(8):
        for p in range(128):
            Sdz[p, j * HID + 4 * j + p // 32] = 1.0
    S2 = np.zeros((128, HH), np.float32)
    for p in range(128):
        S2[p, p % HH] = 1.0
    Gsel = np.zeros((EMB, 4 * 128), np.float32)
    for pg in range(4 * 128):
        Gsel[pg // 32, pg] = 1.0

    fw3 = np.empty((HH, HID * IN), np.float32)
    fb3 = np.empty((HID * IN, 1), np.float32)
    for h_ in range(HID):
        for i in range(IN):
            fw3[:, i * HID + h_] = a["fWout"][:, h_ * IN + i]
            fb3[i * HID + h_, 0] = a["fbout"][h_ * IN + i]

    wpk = np.ascontiguousarray(
        np.transpose(a["gWpool"], (1, 2, 0, 3)).reshape(K, HH, EMB * HH))
    ab = (gE @ a["gbpool"]).astype(np.float32)
    abf = np.concatenate([ab.T] * BS, axis=1)
    gET = np.ascontiguousarray(gE.T)
    gET2 = np.concatenate([gET] * BS, axis=1)
    G = np.maximum(gE @ gE.T, 0.0).astype(np.float32)
    rs = np.exp(G).sum(axis=1)
    recip = np.ascontiguousarray((1.0 / rs).reshape(4, 128).T)

    shared = {
        "recip": recip, "gExp": gET, "gET2": gET2, "Gsel": Gsel,
        "fw1": a["fWin"], "fb1": a["fbin"].reshape(-1, 1),
        "fw2": a["fWmid"], "fb2": a["fbmid"].reshape(-1, 1),
        "fw3": fw3, "fb3": fb3,
        "gw1": a["gWin"], "gb1": a["gbin"].reshape(-1, 1),
        "wpk0": wpk[0], "wpk1": wpk[1], "abf": abf,
        "gwo": a["gWout"],
        "gbo": np.ascontiguousarray(a["gbout"].reshape(8, 128).T),
        "Bc": Bc, "S3": S3, "Erep": Erep, "Sdz": Sdz, "S2": S2,
        "I32": np.eye(HH, dtype=np.float32),
        "ident": np.eye(128, dtype=np.float32),
        "cw": np.ascontiguousarray(a["convW"].T),
        "cb": a["convb"].reshape(-1, 1),
    }
    shared = {k_: np.ascontiguousarray(v).astype(np.float32)
              for k_, v in shared.items()}

    per_core = []
    for c in range(NCORES):
        sl = slice(c * BS, (c + 1) * BS)
        per_core.append({
            "dx": np.ascontiguousarray(
                np.transpose(dX[sl], (2, 3, 0, 1)).reshape(nst * IN, R)),
            "h0": np.ascontiguousarray(
                np.transpose(h0[sl], (2, 0, 1)).reshape(HID, R)).astype(
                    np.float32),
            "z0": np.ascontiguousarray(
                np.transpose(z0[sl], (2, 0, 1)).reshape(HID, R)).astype(
                    np.float32),
        })
    return shared, per_core


_STATE = {}


def _get_nc():
    if "nc" not in _STATE:
        _STATE["nc"] = build_nc()
    return _STATE["nc"]


def _warm():
    """Trigger neuron compile + PJRT executable load with dummy inputs."""
    try:
        nc = _get_nc()
        a = {}
        a["times"] = np.arange(T, dtype=np.float32)
        for nm, sh in [("coeff_a", (B, N, T - 1, IN)),
                       ("coeff_b", (B, N, T - 1, IN)),
                       ("coeff_c2", (B, N, T - 1, IN)),
                       ("coeff_d3", (B, N, T - 1, IN)),
                       ("Wh", (IN, HID)), ("bh", (HID,)),
                       ("Wz", (IN, HID)), ("bz", (HID,)),
                       ("fWin", (HID, HH)), ("fbin", (HH,)),
                       ("fWmid", (HH, HH)), ("fbmid", (HH,)),
                       ("fWout", (HH, HID * IN)), ("fbout", (HID * IN,)),
                       ("gWin", (HID, HH)), ("gbin", (HH,)),
                       ("gE", (N, EMB)), ("gWpool", (EMB, K, HH, HH)),
                       ("gbpool", (EMB, HH)), ("gWout", (HH, HID * HID)),
                       ("gbout", (HID * HID,)), ("convW", (OUT, HID)),
                       ("convb", (OUT,))]:
            a[nm] = np.zeros(sh, np.float32)
        shared, per_core = host_inputs(a)
        in_maps = [{**shared, **pc} for pc in per_core]
        run_bass_kernel_spmd(nc, in_maps, core_ids=list(range(NCORES)))
        _STATE["warm"] = True
    except Exception as e:  # pragma: no cover - keep import usable
        import traceback
        traceback.print_exc()
        _STATE["warm_err"] = e


def kernel(**inputs):
    a = {k_: np.asarray(v, dtype=np.float32) for k_, v in inputs.items()}
    nc = _get_nc()
    shared, per_core = host_inputs(a)
    in_maps = [{**shared, **pc} for pc in per_core]
    res = run_bass_kernel_spmd(nc, in_maps, core_ids=list(range(NCORES)))
    full = np.empty((B, 1, N, OUT), np.float32)
    for c in range(NCORES):
        o = np.asarray(res.results[c]["out"])           # (OUT, R)
        full[c * BS:(c + 1) * BS, 0] = (
            o.reshape(OUT, BS, N).transpose(1, 2, 0))
    return full


_warm()


# revision 4
# speedup vs baseline: 16.9425x; 2.8628x over previous
"""NeuralGCDE on 8 NeuronCores: full RK4 ODE integration on device.

Sharding: data-parallel over batch B=16 -> 2 batch elements per core
(rows r = b*N + n, R = 1024 per core). All graph/MLP params replicated.

Device layout is feature-major (features on SBUF partitions, rows on the
free dim). Every contraction is a PE matmul; partition-dim reductions and
broadcasts use structured 0/1 matrices as stationary operands. The
softmax adjacency (exp(relu(gE gE^T)) with row scaling) is built on
device; only the row-sum reciprocals (512 floats) come from host.

All one-time work (bass build, neuron compile, PJRT load) happens at
import; kernel(**inputs) does host repacks + one SPMD dispatch.
"""
import numpy as np

import concourse.bass as bass
import concourse.mybir as mybir
import concourse.tile as tile
from concourse.bass_utils import run_bass_kernel_spmd

B, N, T = 16, 512, 12
IN, HID, HH, EMB, K, OUT = 2, 32, 32, 16, 2, 12
NCORES = 8
BS = B // NCORES            # 2
R = BS * N                  # 1024
NSTEP = T - 1               # 11
F32 = mybir.dt.float32
AF = mybir.ActivationFunctionType
ALU = mybir.AluOpType

_NO_SPILL = {"InstEventSemaphore", "InstUnconditionalBranch",
             "InstConditionalBranch"}


def _spill_excess_waits(nc):
    """Walrus ISA structs hold one sync-wait slot on most instructions.
    Tile can emit several. Move excess waits onto InstEventSemaphore
    carriers inserted just before, on the same engine (waiting earlier on
    the same engine stream is always sound)."""
    nspill = 0
    for f in nc.m.functions:
        for blk in f.blocks:
            lst = blk.instructions
            i = 0
            while i < len(lst):
                ins = lst[i]
                si = ins.sync_info
                if (type(ins).__name__ in _NO_SPILL or si is None
                        or not si.on_wait or len(si.on_wait) <= 1):
                    i += 1
                    continue
                waits = list(si.on_wait)
                keep, excess = waits[-1:], waits[:-1]
                ins.sync_info = mybir.SyncInfo(on_wait=keep,
                                               on_update=list(si.on_update))
                carriers = []
                while excess:
                    chunk, excess = excess[:2], excess[2:]
                    es = mybir.InstEventSemaphore(
                        name=f"Wspill-{nspill}", ins=[], outs=[])
                    nspill += 1
                    es.engine = ins.engine
                    es.sync_info = mybir.SyncInfo(on_wait=chunk, on_update=[])
                    carriers.append(es)
                for k_, es in enumerate(carriers):
                    lst.insert(i + k_, es)
                i += len(carriers) + 1
    return nspill


def build_nc(nstep=NSTEP):
    nc = bass.Bass()

    def dp(name, shape, out=False):
        return nc.declare_dram_parameter(name, list(shape), F32, isOutput=out)

    d_dx = dp("dx", (2 * 4 * nstep, R))
    d_h0 = dp("h0", (HID, R))
    d_z0 = dp("z0", (HID, R))
    d_gExp = dp("gExp", (EMB, N))          # gE^T
    d_gET2 = dp("gET2", (EMB, R))          # gE^T tiled over b
    d_Gsel = dp("Gsel", (EMB, 4 * 128))    # gE_part selector
    d_fw1 = dp("fw1", (HID, HH))
    d_fb1 = dp("fb1", (HH, 1))
    d_fw2 = dp("fw2", (HH, HH))
    d_fb2 = dp("fb2", (HH, 1))
    d_fw3 = dp("fw3", (HH, HID * IN))      # columns reordered i-major
    d_fb3 = dp("fb3", (HID * IN, 1))
    d_gw1 = dp("gw1", (HID, HH))
    d_gb1 = dp("gb1", (HH, 1))
    d_wpk0 = dp("wpk0", (HH, EMB * HH))    # gWpool k=0, (32, 512)
    d_wpk1 = dp("wpk1", (HH, EMB * HH))
    d_abf = dp("abf", (HH, R))
    d_gwo = dp("gwo", (HH, HID * HID))
    d_gbo = dp("gbo", (128, 8))            # gbout chunked
    d_Bc = dp("Bc", (IN, IN * HID))        # broadcast dX across i-major
    d_S3 = dp("S3", (IN * HID, HID))       # reduce over i
    d_Erep = dp("Erep", (HID, 128))        # replicate dh 4x
    d_Sdz = dp("Sdz", (128, 8 * HID))      # per-chunk reduce for dz
    d_S2 = dp("S2", (128, HH))             # reduce over d
    d_I32 = dp("I32", (HH, HH))
    d_ident = dp("ident", (128, 128))
    d_recip = dp("recip", (128, 4))        # softmax row-sum reciprocals
    d_cw = dp("cw", (HID, OUT))
    d_cb = dp("cb", (OUT, 1))
    d_out = dp("out", (OUT, R), out=True)

    C5 = 512  # fp32 moving-operand free-dim limit

    from contextlib import ExitStack
    with ExitStack() as es:
        tc = es.enter_context(tile.TileContext(nc))
        sgl = es.enter_context(tc.tile_pool(name="sgl", bufs=1))
        wrk = es.enter_context(tc.tile_pool(name="wrk", bufs=1))
        big1 = es.enter_context(tc.tile_pool(name="big1", bufs=1))
        big2 = es.enter_context(tc.tile_pool(name="big2", bufs=2))
        dxp = es.enter_context(tc.tile_pool(name="dxp", bufs=2))
        pA = es.enter_context(tc.tile_pool(name="pA", bufs=2, space="PSUM"))
        pB = es.enter_context(tc.tile_pool(name="pB", bufs=1, space="PSUM"))
        pT = es.enter_context(tc.tile_pool(name="pT", bufs=2, space="PSUM"))

        def load(dram, shape, name):
            t = sgl.tile(list(shape), F32, tag=name, name=name)
            nc.sync.dma_start(out=t[:], in_=dram[:])
            return t

        fw1 = load(d_fw1, (HID, HH), "fw1")
        fb1 = load(d_fb1, (HH, 1), "fb1")
        fw2 = load(d_fw2, (HH, HH), "fw2")
        fb2 = load(d_fb2, (HH, 1), "fb2")
        fw3 = load(d_fw3, (HH, HID * IN), "fw3")
        fb3 = load(d_fb3, (HID * IN, 1), "fb3")
        gw1 = load(d_gw1, (HID, HH), "gw1")
        gb1 = load(d_gb1, (HH, 1), "gb1")
        wpk0 = load(d_wpk0, (HH, EMB * HH), "wpk0")
        wpk1 = load(d_wpk1, (HH, EMB * HH), "wpk1")
        abf = load(d_abf, (HH, R), "abf")
        gwo = load(d_gwo, (HH, HID * HID), "gwo")
        gbo = load(d_gbo, (128, 8), "gbo")
        Bc = load(d_Bc, (IN, IN * HID), "Bc")
        S3 = load(d_S3, (IN * HID, HID), "S3")
        Erep = load(d_Erep, (HID, 128), "Erep")
        Sdz = load(d_Sdz, (128, 8 * HID), "Sdz")
        S2 = load(d_S2, (128, HH), "S2")
        I32 = load(d_I32, (HH, HH), "I32")
        ident = load(d_ident, (128, 128), "ident")
        cw = load(d_cw, (HID, OUT), "cw")
        cb = load(d_cb, (OUT, 1), "cb")
        gExp = load(d_gExp, (EMB, N), "gExp")
        gET2 = load(d_gET2, (EMB, R), "gET2")
        Gsel = load(d_Gsel, (EMB, 4 * 128), "Gsel")
        recip = load(d_recip, (128, 4), "recip")

        def mm2(ps, lhsT, rhs, start=True, stop=True):
            for c in range(2):
                nc.tensor.matmul(ps[:, c * C5:(c + 1) * C5], lhsT,
                                 rhs[:, c * C5:(c + 1) * C5],
                                 start=start, stop=stop)

        def act(out, in_, func, bias=0.0):
            nc.scalar.activation(out, in_, func, bias=bias)

        # ---- adjacency: expG chunks (exp(relu(gE gE^T)), m-major) ----
        expG = []
        for i in range(4):
            gp = pA.tile([128, N], F32, tag="mm", name="mm")
            nc.tensor.matmul(gp[:], gExp[:, i * 128:(i + 1) * 128], gExp[:],
                             start=True, stop=True)
            eg = sgl.tile([128, N], F32, tag=f"expG{i}", name=f"expG{i}")
            act(eg[:], gp[:], AF.Relu)
            act(eg[:], eg[:], AF.Exp)
            expG.append(eg)

        # ---- gE_part chunks: gEp_j[p, r] = gE[n(r), (j*128+p)//32] ----
        gEp = []
        for j in range(4):
            ps = pA.tile([128, R], F32, tag="mm", name="mm")
            mm2(ps, Gsel[:, j * 128:(j + 1) * 128], gET2)
            g = sgl.tile([128, R], F32, tag=f"gEp{j}", name=f"gEp{j}")
            nc.scalar.copy(g[:], ps[:])
            gEp.append(g)

        # ---- state ----
        h = wrk.tile([HID, R], F32, tag="h", name="h", bufs=2)
        z = wrk.tile([HID, R], F32, tag="z", name="z", bufs=2)
        nc.sync.dma_start(out=h[:], in_=d_h0[:])
        nc.sync.dma_start(out=z[:], in_=d_z0[:])

        def vfield(s4, hs, zs, kh, kz):
            dxs = dxp.tile([IN, R], F32, tag="dxs", name="dxs")
            nc.sync.dma_start(out=dxs[:], in_=d_dx[2 * s4:2 * s4 + 2, :])
            # f path: two relu MLP layers + tanh head (i-major columns)
            x1p = pA.tile([HH, R], F32, tag="mm", name="mm")
            mm2(x1p, fw1, hs)
            x1 = wrk.tile([HH, R], F32, tag="fx", name="fx", bufs=2)
            act(x1[:], x1p[:], AF.Relu, bias=fb1[:])
            x2p = pA.tile([HH, R], F32, tag="mm", name="mm")
            mm2(x2p, fw2, x1)
            x2 = wrk.tile([HH, R], F32, tag="fx", name="fx", bufs=2)
            act(x2[:], x2p[:], AF.Relu, bias=fb2[:])
            vfp = pA.tile([HID * IN, R], F32, tag="mm", name="mm")
            mm2(vfp, fw3, x2)
            vf = wrk.tile([HID * IN, R], F32, tag="vf", name="vf")
            act(vf[:], vfp[:], AF.Tanh, bias=fb3[:])
            # dh = sum_i vf_i * dX_i  (dX broadcast via Bc, reduce via S3)
            dXb = pA.tile([IN * HID, R], F32, tag="mm", name="mm")
            mm2(dXb, Bc, dxs)
            nc.vector.tensor_mul(vf[:], vf[:], dXb[:])
            dhp = pB.tile([HID, R], F32, tag="acc", name="acc")
            mm2(dhp, S3, vf)
            nc.scalar.copy(kh[:], dhp[:])
            drp = pA.tile([128, R], F32, tag="mm", name="mm")
            mm2(drp, Erep, kh)
            dhrep = big1.tile([128, R], F32, tag="dhrep", name="dhrep")
            nc.scalar.copy(dhrep[:], drp[:])
            # g path: relu layer (feature-major), node-major transposes
            x1gp = pA.tile([HH, R], F32, tag="mm", name="mm")
            mm2(x1gp, gw1, zs)
            x1g = wrk.tile([HH, R], F32, tag="x1g", name="x1g")
            act(x1g[:], x1gp[:], AF.Relu, bias=gb1[:])
            xT = []
            for k_ in range(4):
                xtp = pT.tile([128, 2 * HH], F32, tag="pt", name="pt")
                for b_ in range(2):
                    nc.tensor.transpose(
                        xtp[:, b_ * HH:(b_ + 1) * HH],
                        x1g[:, b_ * N + k_ * 128: b_ * N + (k_ + 1) * 128],
                        ident[:HH, :HH])
                xt = wrk.tile([128, 2 * HH], F32, tag=f"xT{k_}",
                              name=f"xT{k_}")
                nc.vector.tensor_copy(xt[:], xtp[:])
                xT.append(xt)
            # graph conv: xg1 = A @ x1g per batch, recip folded in
            xg1n = []
            for i in range(4):
                xgp = pT.tile([128, 2 * HH], F32, tag="pt", name="pt")
                for k_ in range(4):
                    nc.tensor.matmul(xgp[:],
                                     expG[k_][:, i * 128:(i + 1) * 128],
                                     xT[k_][:],
                                     start=(k_ == 0), stop=(k_ == 3))
                xn = wrk.tile([128, 2 * HH], F32, tag=f"xg1n{i}",
                              name=f"xg1n{i}")
                nc.vector.tensor_scalar_mul(xn[:], xgp[:], recip[:, i:i + 1])
                xg1n.append(xn)
            xg1f = wrk.tile([HH, R], F32, tag="xg1f", name="xg1f")
            for i in range(4):
                for b_ in range(2):
                    btp = pT.tile([HH, 128], F32, tag="pt", name="pt")
                    nc.tensor.transpose(btp[:],
                                        xg1n[i][:, b_ * HH:(b_ + 1) * HH],
                                        ident[:, :])
                    nc.scalar.copy(
                        xg1f[:, b_ * N + i * 128: b_ * N + (i + 1) * 128],
                        btp[:])
            # per-node pooled weights: y = Wp^T xg scaled by gE_part,
            # reduced over EMB via S2 into x2g (abf preloaded via I32)
            x2gp = pB.tile([HH, R], F32, tag="acc", name="acc")
            for c in range(2):
                nc.tensor.matmul(x2gp[:, c * C5:(c + 1) * C5], I32[:],
                                 abf[:, c * C5:(c + 1) * C5],
                                 start=True, stop=False, skip_group_check=True)
            for j in range(4):
                yp = pA.tile([128, R], F32, tag="mm", name="mm")
                for c in range(2):
                    sl = slice(c * C5, (c + 1) * C5)
                    nc.tensor.matmul(yp[:, sl], wpk0[:, j * 128:(j + 1) * 128],
                                     x1g[:, sl], start=True, stop=False)
                    nc.tensor.matmul(yp[:, sl], wpk1[:, j * 128:(j + 1) * 128],
                                     xg1f[:, sl], start=False, stop=True)
                t_ = big1.tile([128, R], F32, tag="ty", name="ty", bufs=2)
                nc.vector.tensor_mul(t_[:], yp[:], gEp[j][:])
                for c in range(2):
                    sl = slice(c * C5, (c + 1) * C5)
                    nc.tensor.matmul(x2gp[:, sl], S2[:], t_[:, sl],
                                     start=False, stop=(j == 3),
                                     skip_group_check=True)
            x2g = wrk.tile([HH, R], F32, tag="x2g", name="x2g")
            nc.scalar.copy(x2g[:], x2gp[:])
            # vg chunks; dz = sum vg_ho * dh_o accumulated via Sdz
            dzp = pB.tile([HID, R], F32, tag="acc", name="acc")
            for j in range(8):
                vgp = pA.tile([128, R], F32, tag="mm", name="mm")
                mm2(vgp, gwo[:, j * 128:(j + 1) * 128], x2g)
                vg = big2.tile([128, R], F32, tag="vg", name="vg")
                act(vg[:], vgp[:], AF.Tanh, bias=gbo[:, j:j + 1])
                nc.vector.tensor_mul(vg[:], vg[:], dhrep[:])
                for c in range(2):
                    sl = slice(c * C5, (c + 1) * C5)
                    nc.tensor.matmul(dzp[:, sl],
                                     Sdz[:, j * HID:(j + 1) * HID],
                                     vg[:, sl],
                                     start=(j == 0), stop=(j == 7),
                                     skip_group_check=True)
            nc.scalar.copy(kz[:], dzp[:])

        TT = nc.vector.tensor_tensor
        STT = nc.vector.scalar_tensor_tensor

        # RK4 with 3/8 rule, dt = 1 (times are arange; asserted on host)
        for s in range(nstep):
            kh = [wrk.tile([HID, R], F32, tag=f"kh{st}", name=f"kh{st}")
                  for st in range(4)]
            kz = [wrk.tile([HID, R], F32, tag=f"kz{st}", name=f"kz{st}")
                  for st in range(4)]
            vfield(4 * s + 0, h, z, kh[0], kz[0])
            hs = wrk.tile([HID, R], F32, tag="hs", name="hs", bufs=2)
            zs = wrk.tile([HID, R], F32, tag="zs", name="zs", bufs=2)
            STT(hs[:], kh[0][:], 1.0 / 3.0, h[:], op0=ALU.mult, op1=ALU.add)
            STT(zs[:], kz[0][:], 1.0 / 3.0, z[:], op0=ALU.mult, op1=ALU.add)
            vfield(4 * s + 1, hs, zs, kh[1], kz[1])
            hs2 = wrk.tile([HID, R], F32, tag="hs", name="hs", bufs=2)
            zs2 = wrk.tile([HID, R], F32, tag="zs", name="zs", bufs=2)
            STT(hs2[:], kh[0][:], -1.0 / 3.0, kh[1][:],
                op0=ALU.mult, op1=ALU.add)
            TT(hs2[:], hs2[:], h[:], op=ALU.add)
            STT(zs2[:], kz[0][:], -1.0 / 3.0, kz[1][:],
                op0=ALU.mult, op1=ALU.add)
            TT(zs2[:], zs2[:], z[:], op=ALU.add)
            vfield(4 * s + 2, hs2, zs2, kh[2], kz[2])
            hs3 = wrk.tile([HID, R], F32, tag="hs", name="hs", bufs=2)
            zs3 = wrk.tile([HID, R], F32, tag="zs", name="zs", bufs=2)
            STT(hs3[:], kh[1][:], -1.0, kh[0][:], op0=ALU.mult, op1=ALU.add)
            TT(hs3[:], hs3[:], kh[2][:], op=ALU.add)
            TT(hs3[:], hs3[:], h[:], op=ALU.add)
            STT(zs3[:], kz[1][:], -1.0, kz[0][:], op0=ALU.mult, op1=ALU.add)
            TT(zs3[:], zs3[:], kz[2][:], op=ALU.add)
            TT(zs3[:], zs3[:], z[:], op=ALU.add)
            vfield(4 * s + 3, hs3, zs3, kh[3], kz[3])
            hn = wrk.tile([HID, R], F32, tag="h", name="h", bufs=2)
            zn = wrk.tile([HID, R], F32, tag="z", name="z", bufs=2)
            TT(kh[1][:], kh[1][:], kh[2][:], op=ALU.add)
            STT(kh[1][:], kh[1][:], 3.0, kh[0][:], op0=ALU.mult, op1=ALU.add)
            TT(kh[1][:], kh[1][:], kh[3][:], op=ALU.add)
            STT(hn[:], kh[1][:], 0.125, h[:], op0=ALU.mult, op1=ALU.add)
            TT(kz[1][:], kz[1][:], kz[2][:], op=ALU.add)
            STT(kz[1][:], kz[1][:], 3.0, kz[0][:], op0=ALU.mult, op1=ALU.add)
            TT(kz[1][:], kz[1][:], kz[3][:], op=ALU.add)
            STT(zn[:], kz[1][:], 0.125, z[:], op0=ALU.mult, op1=ALU.add)
            h, z = hn, zn

        # ---- end conv ----
        op = pB.tile([OUT, R], F32, tag="acc", name="acc")
        mm2(op, cw, z)
        ob = wrk.tile([OUT, R], F32, tag="x2g", name="x2g")
        nc.vector.tensor_scalar_add(ob[:], op[:], cb[:])
        nc.sync.dma_start(out=d_out[:], in_=ob[:])

    _spill_excess_waits(nc)
    return nc


# ------------------------------------------------------------------
# host-side preprocessing
# ------------------------------------------------------------------
def host_inputs(a, nstep=NSTEP):
    gE = a["gE"]
    times = a["times"]
    assert np.allclose(np.diff(times), 1.0, atol=1e-5), "RK dt=1 baked in"
    maxlen = T - 2
    ts_list = []
    for s in range(nstep):
        t0, t1 = float(times[s]), float(times[s + 1])
        dt = t1 - t0
        ts_list += [t0, t0 + dt / 3.0, t0 + 2.0 * dt / 3.0, t1]
    nst = len(ts_list)
    dX = np.empty((B, N, nst, IN), np.float32)
    for q, t_ in enumerate(ts_list):
        idx = int(np.clip(np.sum(np.float32(t_) > times) - 1, 0, maxlen))
        frac = np.float32(t_ - times[idx])
        dX[:, :, q, :] = (a["coeff_b"][:, :, idx]
                          + (a["coeff_c2"][:, :, idx]
                             + a["coeff_d3"][:, :, idx] * frac) * frac)
    x0 = a["coeff_a"][:, :, 0, :]
    h0 = x0 @ a["Wh"] + a["bh"]
    z0 = x0 @ a["Wz"] + a["bz"]

    Bc = np.zeros((IN, IN * HID), np.float32)
    for i in range(IN):
        Bc[i, i * HID:(i + 1) * HID] = 1.0
    S3 = np.zeros((IN * HID, HID), np.float32)
    for p in range(IN * HID):
        S3[p, p % HID] = 1.0
    Erep = np.zeros((HID, 128), np.float32)
    for p in range(128):
        Erep[p % HID, p] = 1.0
    Sdz = np.zeros((128, 8 * HID), np.float32)
    for j in range(8):
        for p in range(128):
            Sdz[p, j * HID + 4 * j + p // 32] = 1.0
    S2 = np.zeros((128, HH), np.float32)
    for p in range(128):
        S2[p, p % HH] = 1.0
    Gsel = np.zeros((EMB, 4 * 128), np.float32)
    for pg in range(4 * 128):
        Gsel[pg // 32, pg] = 1.0

    fw3 = np.empty((HH, HID * IN), np.float32)
    fb3 = np.empty((HID * IN, 1), np.float32)
    for h_ in range(HID):
        for i in range(IN):
            fw3[:, i * HID + h_] = a["fWout"][:, h_ * IN + i]
            fb3[i * HID + h_, 0] = a["fbout"][h_ * IN + i]

    wpk = np.ascontiguousarray(
        np.transpose(a["gWpool"], (1, 2, 0, 3)).reshape(K, HH, EMB * HH))
    ab = (gE @ a["gbpool"]).astype(np.float32)
    abf = np.concatenate([ab.T] * BS, axis=1)
    gET = np.ascontiguousarray(gE.T)
    gET2 = np.concatenate([gET] * BS, axis=1)
    G = np.maximum(gE @ gE.T, 0.0).astype(np.float32)
    rs = np.exp(G).sum(axis=1)
    recip = np.ascontiguousarray((1.0 / rs).reshape(4, 128).T)

    shared = {
        "recip": recip, "gExp": gET, "gET2": gET2, "Gsel": Gsel,
        "fw1": a["fWin"], "fb1": a["fbin"].reshape(-1, 1),
        "fw2": a["fWmid"], "fb2": a["fbmid"].reshape(-1, 1),
        "fw3": fw3, "fb3": fb3,
        "gw1": a["gWin"], "gb1": a["gbin"].reshape(-1, 1),
        "wpk0": wpk[0], "wpk1": wpk[1], "abf": abf,
        "gwo": a["gWout"],
        "gbo": np.ascontiguousarray(a["gbout"].reshape(8, 128).T),
        "Bc": Bc, "S3": S3, "Erep": Erep, "Sdz": Sdz, "S2": S2,
        "I32": np.eye(HH, dtype=np.float32),
        "ident": np.eye(128, dtype=np.float32),
        "cw": np.ascontiguousarray(a["convW"].T),
        "cb": a["convb"].reshape(-1, 1),
    }
    shared = {k_: np.ascontiguousarray(v).astype(np.float32)
              for k_, v in shared.items()}

    per_core = []
    for c in range(NCORES):
        sl = slice(c * BS, (c + 1) * BS)
        per_core.append({
            "dx": np.ascontiguousarray(
                np.transpose(dX[sl], (2, 3, 0, 1)).reshape(nst * IN, R)),
            "h0": np.ascontiguousarray(
                np.transpose(h0[sl], (2, 0, 1)).reshape(HID, R)).astype(
                    np.float32),
            "z0": np.ascontiguousarray(
                np.transpose(z0[sl], (2, 0, 1)).reshape(HID, R)).astype(
                    np.float32),
        })
    return shared, per_core


_STATE = {}


def _get_nc():
    if "nc" not in _STATE:
        _STATE["nc"] = build_nc()
    return _STATE["nc"]


def _get_runner():
    """Cached jit(shard_map(bass_exec)) callable — built once so per-call
    cost is dispatch only (run_bass_kernel_spmd re-traces every call)."""
    if "runner" in _STATE:
        return _STATE["runner"]
    import jax
    from jax.sharding import Mesh, PartitionSpec
    from jax.experimental.shard_map import shard_map
    from concourse import bass2jax as b2j

    b2j.install_neuronx_cc_hook()
    nc = _get_nc()
    assert nc.dbg_addr is None
    partition_name = (nc.partition_id_tensor.name
                      if nc.partition_id_tensor else None)
    in_names, out_names, out_avals, zero_outs = [], [], [], []
    for alloc in nc.m.functions[0].allocations:
        if not isinstance(alloc, mybir.MemoryLocationSet):
            continue
        name = alloc.memorylocations[0].name
        if alloc.kind == "ExternalInput":
            if name != partition_name:
                in_names.append(name)
        elif alloc.kind == "ExternalOutput":
            shape = tuple(alloc.tensor_shape)
            dtype = mybir.dt.np(alloc.dtype)
            out_names.append(name)
            out_avals.append(jax.core.ShapedArray(shape, dtype))
            zero_outs.append(np.zeros((NCORES * shape[0], *shape[1:]), dtype))
    n_params = len(in_names)
    all_names = list(in_names) + list(out_names)
    if partition_name is not None:
        all_names.append(partition_name)

    def _body(*args):
        operands = list(args)
        if partition_name is not None:
            operands.append(b2j.partition_id_tensor())
        outs = b2j._bass_exec_p.bind(
            *operands,
            out_avals=tuple(out_avals),
            in_names=tuple(all_names),
            out_names=tuple(out_names),
            lowering_input_output_aliases=(),
            sim_require_finite=True,
            sim_require_nnan=True,
            nc=nc,
        )
        return tuple(outs)

    devices = jax.devices()[:NCORES]
    mesh = Mesh(np.asarray(devices), ("core",))
    n_outs = len(out_names)
    sharded = jax.jit(
        shard_map(_body, mesh=mesh,
                  in_specs=(PartitionSpec("core"),) * (n_params + n_outs),
                  out_specs=(PartitionSpec("core"),) * n_outs,
                  check_rep=False),
        donate_argnums=tuple(range(n_params, n_params + n_outs)),
        keep_unused=True,
    )

    def run(in_maps):
        concat_in = [
            np.concatenate([np.asarray(in_maps[c][nm])
                            for c in range(NCORES)], axis=0)
            for nm in in_names
        ]
        out_arrs = sharded(*concat_in, *zero_outs)
        return [
            {nm: np.asarray(out_arrs[i]).reshape(
                NCORES, *out_avals[i].shape)[c]
             for i, nm in enumerate(out_names)}
            for c in range(NCORES)
        ]

    _STATE["runner"] = run
    return run


def _warm():
    """Trigger neuron compile + PJRT executable load with dummy inputs."""
    try:
        nc = _get_nc()
        a = {}
        a["times"] = np.arange(T, dtype=np.float32)
        for nm, sh in [("coeff_a", (B, N, T - 1, IN)),
                       ("coeff_b", (B, N, T - 1, IN)),
                       ("coeff_c2", (B, N, T - 1, IN)),
                       ("coeff_d3", (B, N, T - 1, IN)),
                       ("Wh", (IN, HID)), ("bh", (HID,)),
                       ("Wz", (IN, HID)), ("bz", (HID,)),
                       ("fWin", (HID, HH)), ("fbin", (HH,)),
                       ("fWmid", (HH, HH)), ("fbmid", (HH,)),
                       ("fWout", (HH, HID * IN)), ("fbout", (HID * IN,)),
                       ("gWin", (HID, HH)), ("gbin", (HH,)),
                       ("gE", (N, EMB)), ("gWpool", (EMB, K, HH, HH)),
                       ("gbpool", (EMB, HH)), ("gWout", (HH, HID * HID)),
                       ("gbout", (HID * HID,)), ("convW", (OUT, HID)),
                       ("convb", (OUT,))]:
            a[nm] = np.zeros(sh, np.float32)
        shared, per_core = host_inputs(a)
        in_maps = [{**shared, **pc} for pc in per_core]
        _get_runner()(in_maps)
        _STATE["warm"] = True
    except Exception as e:  # pragma: no cover - keep import usable
        import traceback
        traceback.print_exc()
        _STATE["warm_err"] = e


def kernel(**inputs):
    a = {k_: np.asarray(v, dtype=np.float32) for k_, v in inputs.items()}
    shared, per_core = host_inputs(a)
    in_maps = [{**shared, **pc} for pc in per_core]
    results = _get_runner()(in_maps)
    full = np.empty((B, 1, N, OUT), np.float32)
    for c in range(NCORES):
        o = np.asarray(results[c]["out"])               # (OUT, R)
        full[c * BS:(c + 1) * BS, 0] = (
            o.reshape(OUT, BS, N).transpose(1, 2, 0))
    return full


_warm()


# revision 6
# speedup vs baseline: 28.6840x; 1.6930x over previous
"""NeuralGCDE on 8 NeuronCores: full RK4 ODE integration on device.

Sharding: data-parallel over batch B=16 -> 2 batch elements per core
(rows r = b*N + n, R = 1024 per core). All graph/MLP params replicated.

Device layout is feature-major (features on SBUF partitions, rows on the
free dim). Every contraction is a PE matmul; partition-dim reductions and
broadcasts use structured 0/1 matrices as stationary operands. The
softmax adjacency (exp(relu(gE gE^T)) with row scaling) is built on
device; only the row-sum reciprocals (512 floats) come from host.

All one-time work (bass build, neuron compile, PJRT load) happens at
import; kernel(**inputs) does host repacks + one SPMD dispatch.
"""
import numpy as np

import concourse.bass as bass
import concourse.mybir as mybir
import concourse.tile as tile
from concourse.bass_utils import run_bass_kernel_spmd

B, N, T = 16, 512, 12
IN, HID, HH, EMB, K, OUT = 2, 32, 32, 16, 2, 12
NCORES = 8
BS = B // NCORES            # 2
R = BS * N                  # 1024
NSTEP = T - 1               # 11
F32 = mybir.dt.float32
AF = mybir.ActivationFunctionType
ALU = mybir.AluOpType

_NO_SPILL = {"InstEventSemaphore", "InstUnconditionalBranch",
             "InstConditionalBranch"}


def _spill_excess_waits(nc):
    """Walrus ISA structs hold one sync-wait slot on most instructions.
    Tile can emit several. Move excess waits onto InstEventSemaphore
    carriers inserted just before, on the same engine (waiting earlier on
    the same engine stream is always sound)."""
    nspill = 0
    for f in nc.m.functions:
        for blk in f.blocks:
            lst = blk.instructions
            i = 0
            while i < len(lst):
                ins = lst[i]
                si = ins.sync_info
                if (type(ins).__name__ in _NO_SPILL or si is None
                        or not si.on_wait or len(si.on_wait) <= 1):
                    i += 1
                    continue
                waits = list(si.on_wait)
                keep, excess = waits[-1:], waits[:-1]
                ins.sync_info = mybir.SyncInfo(on_wait=keep,
                                               on_update=list(si.on_update))
                carriers = []
                while excess:
                    chunk, excess = excess[:2], excess[2:]
                    es = mybir.InstEventSemaphore(
                        name=f"Wspill-{nspill}", ins=[], outs=[])
                    nspill += 1
                    es.engine = ins.engine
                    es.sync_info = mybir.SyncInfo(on_wait=chunk, on_update=[])
                    carriers.append(es)
                for k_, es in enumerate(carriers):
                    lst.insert(i + k_, es)
                i += len(carriers) + 1
    return nspill


def build_nc(nstep=NSTEP):
    nc = bass.Bass()

    def dp(name, shape, out=False):
        return nc.declare_dram_parameter(name, list(shape), F32, isOutput=out)

    NU = 3 * nstep + 1                     # unique dX stage rows
    d_dx = dp("dx", (2 * NU, R))
    d_x0 = dp("x0", (IN, R))
    d_Wh = dp("Wh", (IN, HID))
    d_bh = dp("bh", (HID, 1))
    d_Wz = dp("Wz", (IN, HID))
    d_bz = dp("bz", (HID, 1))
    d_gET = dp("gET", (EMB, N))            # gE^T
    d_gbpool = dp("gbpool", (EMB, HH))
    d_fw1 = dp("fw1", (HID, HH))
    d_fb1 = dp("fb1", (HH, 1))
    d_fw2 = dp("fw2", (HH, HH))
    d_fb2 = dp("fb2", (HH, 1))
    d_fw3 = dp("fw3", (HH, HID * IN))      # columns reordered i-major
    d_fb3 = dp("fb3", (HID * IN, 1))
    d_gw1 = dp("gw1", (HID, HH))
    d_gb1 = dp("gb1", (HH, 1))
    d_wpk0 = dp("wpk0", (HH, EMB * HH))    # gWpool k=0, (32, 512)
    d_wpk1 = dp("wpk1", (HH, EMB * HH))
    d_gwo = dp("gwo", (HH, HID * HID))
    d_gbo = dp("gbo", (128, 8))            # gbout chunked
    d_recip = dp("recip", (128, 4))        # softmax row-sum reciprocals
    d_cw = dp("cw", (HID, OUT))
    d_cb = dp("cb", (OUT, 1))
    d_out = dp("out", (OUT, R), out=True)

    C5 = 512  # fp32 moving-operand free-dim limit

    from contextlib import ExitStack
    with ExitStack() as es:
        tc = es.enter_context(tile.TileContext(nc))
        sgl = es.enter_context(tc.tile_pool(name="sgl", bufs=1))
        wrk = es.enter_context(tc.tile_pool(name="wrk", bufs=1))
        big1 = es.enter_context(tc.tile_pool(name="big1", bufs=1))
        big2 = es.enter_context(tc.tile_pool(name="big2", bufs=2))
        dxp = es.enter_context(tc.tile_pool(name="dxp", bufs=2))
        pA = es.enter_context(tc.tile_pool(name="pA", bufs=2, space="PSUM"))
        pB = es.enter_context(tc.tile_pool(name="pB", bufs=1, space="PSUM"))
        pT = es.enter_context(tc.tile_pool(name="pT", bufs=2, space="PSUM"))

        def load(dram, shape, name):
            t = sgl.tile(list(shape), F32, tag=name, name=name)
            nc.sync.dma_start(out=t[:], in_=dram[:])
            return t

        fw1 = load(d_fw1, (HID, HH), "fw1")
        fb1 = load(d_fb1, (HH, 1), "fb1")
        fw2 = load(d_fw2, (HH, HH), "fw2")
        fb2 = load(d_fb2, (HH, 1), "fb2")
        fw3 = load(d_fw3, (HH, HID * IN), "fw3")
        fb3 = load(d_fb3, (HID * IN, 1), "fb3")
        gw1 = load(d_gw1, (HID, HH), "gw1")
        gb1 = load(d_gb1, (HH, 1), "gb1")
        wpk0 = load(d_wpk0, (HH, EMB * HH), "wpk0")
        wpk1 = load(d_wpk1, (HH, EMB * HH), "wpk1")
        gwo = load(d_gwo, (HH, HID * HID), "gwo")
        gbo = load(d_gbo, (128, 8), "gbo")
        cw = load(d_cw, (HID, OUT), "cw")
        cb = load(d_cb, (OUT, 1), "cb")
        gET = load(d_gET, (EMB, N), "gET")
        gbpool = load(d_gbpool, (EMB, HH), "gbpool")
        recip = load(d_recip, (128, 4), "recip")
        x0 = load(d_x0, (IN, R), "x0")
        Wh = load(d_Wh, (IN, HID), "Wh")
        bh = load(d_bh, (HID, 1), "bh")
        Wz = load(d_Wz, (IN, HID), "Wz")
        bz = load(d_bz, (HID, 1), "bz")

        # ---- structured 0/1 matrices, built in place ----
        NE = ALU.not_equal

        def zeros_tile(name, shape):
            t = sgl.tile(list(shape), F32, tag=name, name=name)
            nc.gpsimd.memset(t[:], 0.0)
            return t

        def aff(t, ap, pattern, base=0, cm=0):
            nc.gpsimd.affine_select(out=ap, in_=ap, compare_op=NE, fill=1.0,
                                    base=base, pattern=pattern,
                                    channel_multiplier=cm)

        ident = zeros_tile("ident", (128, 128))
        aff(ident, ident[:], [[-1, 128]], cm=1)
        I32 = zeros_tile("I32", (HH, HH))
        aff(I32, I32[:], [[-1, HH]], cm=1)
        Bc = zeros_tile("Bc", (IN, IN * HID))      # 1 iff col//32 == p
        aff(Bc, Bc[:].rearrange("p (j y) -> p j y", y=HID), [[-1, IN], [0, HID]],
            cm=1)
        Erep = zeros_tile("Erep", (HID, 128))      # 1 iff col%32 == p
        aff(Erep, Erep[:].rearrange("p (j y) -> p j y", y=HID),
            [[0, 4], [-1, HID]], cm=1)
        S3 = zeros_tile("S3", (IN * HID, HID))     # 1 iff p%32 == col
        aff(S3, S3[:], [[-1, HID]], cm=1)
        aff(S3, S3[:], [[-1, HID]], base=-HID, cm=1)
        S2 = zeros_tile("S2", (128, HH))           # 1 iff p%32 == col
        for q in range(4):
            aff(S2, S2[:], [[-1, HH]], base=-q * HH, cm=1)
        Gsel = zeros_tile("Gsel", (EMB, 4 * 128))  # 1 iff col//32 == p
        aff(Gsel, Gsel[:].rearrange("p (j y) -> p j y", y=32),
            [[-1, EMB], [0, 32]], cm=1)
        # Sdz[p, j*32+y] = 1 iff y == 4j + p//32, composed as E4.T @ Cdz
        E4 = zeros_tile("E4", (4, 128))            # 1 iff col//32 == p
        aff(E4, E4[:].rearrange("p (j y) -> p j y", y=32), [[-1, 4], [0, 32]],
            cm=1)
        Cdz = zeros_tile("Cdz", (4, 8 * HID))      # 1 iff y == 4j + p
        aff(Cdz, Cdz[:].rearrange("p (j y) -> p j y", y=HID),
            [[4, 8], [-1, HID]], cm=1)
        sdzp = pA.tile([128, 8 * HID], F32, tag="mm", name="mm")
        nc.tensor.matmul(sdzp[:], E4[:], Cdz[:], start=True, stop=True)
        Sdz = sgl.tile([128, 8 * HID], F32, tag="Sdz", name="Sdz")
        nc.scalar.copy(Sdz[:], sdzp[:])

        # ---- abT[o, n] = (gE @ gbpool).T, used for both batch halves ----
        abp = pA.tile([HH, N], F32, tag="mm", name="mm")
        nc.tensor.matmul(abp[:], gbpool[:], gET[:], start=True, stop=True)
        abT = sgl.tile([HH, N], F32, tag="abT", name="abT")
        nc.scalar.copy(abT[:], abp[:])

        def mm2(ps, lhsT, rhs, start=True, stop=True):
            for c in range(2):
                nc.tensor.matmul(ps[:, c * C5:(c + 1) * C5], lhsT,
                                 rhs[:, c * C5:(c + 1) * C5],
                                 start=start, stop=stop)

        def act(out, in_, func, bias=0.0):
            nc.scalar.activation(out, in_, func, bias=bias)

        # ---- adjacency: expG chunks (exp(relu(gE gE^T)), m-major) ----
        expG = []
        for i in range(4):
            gp = pA.tile([128, N], F32, tag="mm", name="mm")
            nc.tensor.matmul(gp[:], gET[:, i * 128:(i + 1) * 128], gET[:],
                             start=True, stop=True)
            eg = sgl.tile([128, N], F32, tag=f"expG{i}", name=f"expG{i}")
            act(eg[:], gp[:], AF.Relu)
            act(eg[:], eg[:], AF.Exp)
            expG.append(eg)

        # ---- gE_part chunks: gEp_j[p, r] = gE[n(r), (j*128+p)//32] ----
        gEp = []
        for j in range(4):
            ps = pA.tile([128, R], F32, tag="mm", name="mm")
            for c in range(2):
                nc.tensor.matmul(ps[:, c * C5:(c + 1) * C5],
                                 Gsel[:, j * 128:(j + 1) * 128], gET[:],
                                 start=True, stop=True)
            g = sgl.tile([128, R], F32, tag=f"gEp{j}", name=f"gEp{j}")
            nc.scalar.copy(g[:], ps[:])
            gEp.append(g)

        # ---- state: h0 = x0 @ Wh + bh, z0 = x0 @ Wz + bz ----
        h = wrk.tile([HID, R], F32, tag="h", name="h", bufs=2)
        z = wrk.tile([HID, R], F32, tag="z", name="z", bufs=2)
        h0p = pA.tile([HID, R], F32, tag="mm", name="mm")
        mm2(h0p, Wh, x0)
        nc.vector.tensor_scalar_add(h[:], h0p[:], bh[:])
        z0p = pA.tile([HID, R], F32, tag="mm", name="mm")
        mm2(z0p, Wz, x0)
        nc.vector.tensor_scalar_add(z[:], z0p[:], bz[:])

        def vfield(s4, hs, zs, kh, kz):
            u = 3 * (s4 // 4) + (s4 % 4)
            dxs = dxp.tile([IN, R], F32, tag="dxs", name="dxs")
            nc.sync.dma_start(out=dxs[:], in_=d_dx[2 * u:2 * u + 2, :])
            # f path: two relu MLP layers + tanh head (i-major columns)
            x1p = pA.tile([HH, R], F32, tag="mm", name="mm")
            mm2(x1p, fw1, hs)
            x1 = wrk.tile([HH, R], F32, tag="fx", name="fx", bufs=2)
            act(x1[:], x1p[:], AF.Relu, bias=fb1[:])
            x2p = pA.tile([HH, R], F32, tag="mm", name="mm")
            mm2(x2p, fw2, x1)
            x2 = wrk.tile([HH, R], F32, tag="fx", name="fx", bufs=2)
            act(x2[:], x2p[:], AF.Relu, bias=fb2[:])
            vfp = pA.tile([HID * IN, R], F32, tag="mm", name="mm")
            mm2(vfp, fw3, x2)
            vf = wrk.tile([HID * IN, R], F32, tag="vf", name="vf")
            act(vf[:], vfp[:], AF.Tanh, bias=fb3[:])
            # dh = sum_i vf_i * dX_i  (dX broadcast via Bc, reduce via S3)
            dXb = pA.tile([IN * HID, R], F32, tag="mm", name="mm")
            mm2(dXb, Bc, dxs)
            nc.vector.tensor_mul(vf[:], vf[:], dXb[:])
            dhp = pB.tile([HID, R], F32, tag="acc", name="acc")
            mm2(dhp, S3, vf)
            nc.scalar.copy(kh[:], dhp[:])
            drp = pA.tile([128, R], F32, tag="mm", name="mm")
            mm2(drp, Erep, kh)
            dhrep = big1.tile([128, R], F32, tag="dhrep", name="dhrep")
            nc.scalar.copy(dhrep[:], drp[:])
            # g path: relu layer (feature-major), node-major transposes
            x1gp = pA.tile([HH, R], F32, tag="mm", name="mm")
            mm2(x1gp, gw1, zs)
            x1g = wrk.tile([HH, R], F32, tag="x1g", name="x1g")
            act(x1g[:], x1gp[:], AF.Relu, bias=gb1[:])
            xT = []
            for k_ in range(4):
                xtp = pT.tile([128, 2 * HH], F32, tag="pt", name="pt")
                for b_ in range(2):
                    nc.tensor.transpose(
                        xtp[:, b_ * HH:(b_ + 1) * HH],
                        x1g[:, b_ * N + k_ * 128: b_ * N + (k_ + 1) * 128],
                        ident[:HH, :HH])
                xt = wrk.tile([128, 2 * HH], F32, tag=f"xT{k_}",
                              name=f"xT{k_}")
                nc.vector.tensor_copy(xt[:], xtp[:])
                xT.append(xt)
            # graph conv: xg1 = A @ x1g per batch, recip folded in
            xg1n = []
            for i in range(4):
                xgp = pT.tile([128, 2 * HH], F32, tag="pt", name="pt")
                for k_ in range(4):
                    nc.tensor.matmul(xgp[:],
                                     expG[k_][:, i * 128:(i + 1) * 128],
                                     xT[k_][:],
                                     start=(k_ == 0), stop=(k_ == 3))
                xn = wrk.tile([128, 2 * HH], F32, tag=f"xg1n{i}",
                              name=f"xg1n{i}")
                nc.vector.tensor_scalar_mul(xn[:], xgp[:], recip[:, i:i + 1])
                xg1n.append(xn)
            xg1f = wrk.tile([HH, R], F32, tag="xg1f", name="xg1f")
            for i in range(4):
                for b_ in range(2):
                    btp = pT.tile([HH, 128], F32, tag="pt", name="pt")
                    nc.tensor.transpose(btp[:],
                                        xg1n[i][:, b_ * HH:(b_ + 1) * HH],
                                        ident[:, :])
                    nc.scalar.copy(
                        xg1f[:, b_ * N + i * 128: b_ * N + (i + 1) * 128],
                        btp[:])
            # per-node pooled weights: y = Wp^T xg scaled by gE_part,
            # reduced over EMB via S2 into x2g (abf preloaded via I32)
            x2gp = pB.tile([HH, R], F32, tag="acc", name="acc")
            for c in range(2):
                nc.tensor.matmul(x2gp[:, c * C5:(c + 1) * C5], I32[:],
                                 abT[:],
                                 start=True, stop=False, skip_group_check=True)
            for j in range(4):
                yp = pA.tile([128, R], F32, tag="mm", name="mm")
                for c in range(2):
                    sl = slice(c * C5, (c + 1) * C5)
                    nc.tensor.matmul(yp[:, sl], wpk0[:, j * 128:(j + 1) * 128],
                                     x1g[:, sl], start=True, stop=False)
                    nc.tensor.matmul(yp[:, sl], wpk1[:, j * 128:(j + 1) * 128],
                                     xg1f[:, sl], start=False, stop=True)
                t_ = big1.tile([128, R], F32, tag="ty", name="ty", bufs=2)
                nc.vector.tensor_mul(t_[:], yp[:], gEp[j][:])
                for c in range(2):
                    sl = slice(c * C5, (c + 1) * C5)
                    nc.tensor.matmul(x2gp[:, sl], S2[:], t_[:, sl],
                                     start=False, stop=(j == 3),
                                     skip_group_check=True)
            x2g = wrk.tile([HH, R], F32, tag="x2g", name="x2g")
            nc.scalar.copy(x2g[:], x2gp[:])
            # vg chunks; dz = sum vg_ho * dh_o accumulated via Sdz
            dzp = pB.tile([HID, R], F32, tag="acc", name="acc")
            for j in range(8):
                vgp = pA.tile([128, R], F32, tag="mm", name="mm")
                mm2(vgp, gwo[:, j * 128:(j + 1) * 128], x2g)
                vg = big2.tile([128, R], F32, tag="vg", name="vg")
                act(vg[:], vgp[:], AF.Tanh, bias=gbo[:, j:j + 1])
                nc.vector.tensor_mul(vg[:], vg[:], dhrep[:])
                for c in range(2):
                    sl = slice(c * C5, (c + 1) * C5)
                    nc.tensor.matmul(dzp[:, sl],
                                     Sdz[:, j * HID:(j + 1) * HID],
                                     vg[:, sl],
                                     start=(j == 0), stop=(j == 7),
                                     skip_group_check=True)
            nc.scalar.copy(kz[:], dzp[:])

        TT = nc.vector.tensor_tensor
        STT = nc.vector.scalar_tensor_tensor

        # RK4 with 3/8 rule, dt = 1 (times are arange; asserted on host)
        for s in range(nstep):
            kh = [wrk.tile([HID, R], F32, tag=f"kh{st}", name=f"kh{st}")
                  for st in range(4)]
            kz = [wrk.tile([HID, R], F32, tag=f"kz{st}", name=f"kz{st}")
                  for st in range(4)]
            vfield(4 * s + 0, h, z, kh[0], kz[0])
            hs = wrk.tile([HID, R], F32, tag="hs", name="hs", bufs=2)
            zs = wrk.tile([HID, R], F32, tag="zs", name="zs", bufs=2)
            STT(hs[:], kh[0][:], 1.0 / 3.0, h[:], op0=ALU.mult, op1=ALU.add)
            STT(zs[:], kz[0][:], 1.0 / 3.0, z[:], op0=ALU.mult, op1=ALU.add)
            vfield(4 * s + 1, hs, zs, kh[1], kz[1])
            hs2 = wrk.tile([HID, R], F32, tag="hs", name="hs", bufs=2)
            zs2 = wrk.tile([HID, R], F32, tag="zs", name="zs", bufs=2)
            STT(hs2[:], kh[0][:], -1.0 / 3.0, kh[1][:],
                op0=ALU.mult, op1=ALU.add)
            TT(hs2[:], hs2[:], h[:], op=ALU.add)
            STT(zs2[:], kz[0][:], -1.0 / 3.0, kz[1][:],
                op0=ALU.mult, op1=ALU.add)
            TT(zs2[:], zs2[:], z[:], op=ALU.add)
            vfield(4 * s + 2, hs2, zs2, kh[2], kz[2])
            hs3 = wrk.tile([HID, R], F32, tag="hs", name="hs", bufs=2)
            zs3 = wrk.tile([HID, R], F32, tag="zs", name="zs", bufs=2)
            STT(hs3[:], kh[1][:], -1.0, kh[0][:], op0=ALU.mult, op1=ALU.add)
            TT(hs3[:], hs3[:], kh[2][:], op=ALU.add)
            TT(hs3[:], hs3[:], h[:], op=ALU.add)
            STT(zs3[:], kz[1][:], -1.0, kz[0][:], op0=ALU.mult, op1=ALU.add)
            TT(zs3[:], zs3[:], kz[2][:], op=ALU.add)
            TT(zs3[:], zs3[:], z[:], op=ALU.add)
            vfield(4 * s + 3, hs3, zs3, kh[3], kz[3])
            hn = wrk.tile([HID, R], F32, tag="h", name="h", bufs=2)
            zn = wrk.tile([HID, R], F32, tag="z", name="z", bufs=2)
            TT(kh[1][:], kh[1][:], kh[2][:], op=ALU.add)
            STT(kh[1][:], kh[1][:], 3.0, kh[0][:], op0=ALU.mult, op1=ALU.add)
            TT(kh[1][:], kh[1][:], kh[3][:], op=ALU.add)
            STT(hn[:], kh[1][:], 0.125, h[:], op0=ALU.mult, op1=ALU.add)
            TT(kz[1][:], kz[1][:], kz[2][:], op=ALU.add)
            STT(kz[1][:], kz[1][:], 3.0, kz[0][:], op0=ALU.mult, op1=ALU.add)
            TT(kz[1][:], kz[1][:], kz[3][:], op=ALU.add)
            STT(zn[:], kz[1][:], 0.125, z[:], op0=ALU.mult, op1=ALU.add)
            h, z = hn, zn

        # ---- end conv ----
        op = pB.tile([OUT, R], F32, tag="acc", name="acc")
        mm2(op, cw, z)
        ob = wrk.tile([OUT, R], F32, tag="x2g", name="x2g")
        nc.vector.tensor_scalar_add(ob[:], op[:], cb[:])
        nc.sync.dma_start(out=d_out[:], in_=ob[:])

    _spill_excess_waits(nc)
    return nc


# ------------------------------------------------------------------
# host-side preprocessing
# ------------------------------------------------------------------
def host_inputs(a, nstep=NSTEP):
    gE = a["gE"]
    times = a["times"]
    assert np.allclose(np.diff(times), 1.0, atol=1e-5), "RK dt=1 baked in"
    maxlen = T - 2
    # unique stage times: u = 3s + st (stage 4 of step s == stage 0 of s+1)
    ts_list = [float(times[0])]
    for s in range(nstep):
        t0, t1 = float(times[s]), float(times[s + 1])
        dt = t1 - t0
        ts_list += [t0 + dt / 3.0, t0 + 2.0 * dt / 3.0, t1]
    nu = len(ts_list)
    dX = np.empty((B, N, nu, IN), np.float32)
    for q, t_ in enumerate(ts_list):
        idx = int(np.clip(np.sum(np.float32(t_) > times) - 1, 0, maxlen))
        frac = np.float32(t_ - times[idx])
        dX[:, :, q, :] = (a["coeff_b"][:, :, idx]
                          + (a["coeff_c2"][:, :, idx]
                             + a["coeff_d3"][:, :, idx] * frac) * frac)
    x0 = a["coeff_a"][:, :, 0, :]                   # (B, N, IN)

    fw3 = np.empty((HH, HID * IN), np.float32)
    fb3 = np.empty((HID * IN, 1), np.float32)
    for h_ in range(HID):
        for i in range(IN):
            fw3[:, i * HID + h_] = a["fWout"][:, h_ * IN + i]
            fb3[i * HID + h_, 0] = a["fbout"][h_ * IN + i]

    wpk = np.ascontiguousarray(
        np.transpose(a["gWpool"], (1, 2, 0, 3)).reshape(K, HH, EMB * HH))
    G = np.maximum(gE @ gE.T, 0.0).astype(np.float32)
    rs = np.exp(G).sum(axis=1)
    recip = np.ascontiguousarray((1.0 / rs).reshape(4, 128).T)

    shared = {
        "recip": recip, "gET": gE.T, "gbpool": a["gbpool"],
        "Wh": a["Wh"], "bh": a["bh"].reshape(-1, 1),
        "Wz": a["Wz"], "bz": a["bz"].reshape(-1, 1),
        "fw1": a["fWin"], "fb1": a["fbin"].reshape(-1, 1),
        "fw2": a["fWmid"], "fb2": a["fbmid"].reshape(-1, 1),
        "fw3": fw3, "fb3": fb3,
        "gw1": a["gWin"], "gb1": a["gbin"].reshape(-1, 1),
        "wpk0": wpk[0], "wpk1": wpk[1],
        "gwo": a["gWout"],
        "gbo": np.ascontiguousarray(a["gbout"].reshape(8, 128).T),
        "cw": np.ascontiguousarray(a["convW"].T),
        "cb": a["convb"].reshape(-1, 1),
    }
    shared = {k_: np.ascontiguousarray(v).astype(np.float32)
              for k_, v in shared.items()}

    per_core = []
    for c in range(NCORES):
        sl = slice(c * BS, (c + 1) * BS)
        per_core.append({
            "dx": np.ascontiguousarray(
                np.transpose(dX[sl], (2, 3, 0, 1)).reshape(nu * IN, R)),
            "x0": np.ascontiguousarray(
                np.transpose(x0[sl], (2, 0, 1)).reshape(IN, R)).astype(
                    np.float32),
        })
    return shared, per_core


_STATE = {}


def _get_nc():
    if "nc" not in _STATE:
        _STATE["nc"] = build_nc()
    return _STATE["nc"]


def _get_runner():
    """Cached jit(shard_map(bass_exec)) callable — built once so per-call
    cost is dispatch only (run_bass_kernel_spmd re-traces every call)."""
    if "runner" in _STATE:
        return _STATE["runner"]
    import jax
    from jax.sharding import Mesh, PartitionSpec
    from jax.experimental.shard_map import shard_map
    from concourse import bass2jax as b2j

    b2j.install_neuronx_cc_hook()
    nc = _get_nc()
    assert nc.dbg_addr is None
    partition_name = (nc.partition_id_tensor.name
                      if nc.partition_id_tensor else None)
    in_names, out_names, out_avals, zero_outs = [], [], [], []
    for alloc in nc.m.functions[0].allocations:
        if not isinstance(alloc, mybir.MemoryLocationSet):
            continue
        name = alloc.memorylocations[0].name
        if alloc.kind == "ExternalInput":
            if name != partition_name:
                in_names.append(name)
        elif alloc.kind == "ExternalOutput":
            shape = tuple(alloc.tensor_shape)
            dtype = mybir.dt.np(alloc.dtype)
            out_names.append(name)
            out_avals.append(jax.core.ShapedArray(shape, dtype))
            zero_outs.append(np.zeros((NCORES * shape[0], *shape[1:]), dtype))
    n_params = len(in_names)
    all_names = list(in_names) + list(out_names)
    if partition_name is not None:
        all_names.append(partition_name)

    def _body(*args):
        operands = list(args)
        if partition_name is not None:
            operands.append(b2j.partition_id_tensor())
        outs = b2j._bass_exec_p.bind(
            *operands,
            out_avals=tuple(out_avals),
            in_names=tuple(all_names),
            out_names=tuple(out_names),
            lowering_input_output_aliases=(),
            sim_require_finite=True,
            sim_require_nnan=True,
            nc=nc,
        )
        return tuple(outs)

    devices = jax.devices()[:NCORES]
    mesh = Mesh(np.asarray(devices), ("core",))
    n_outs = len(out_names)
    sharded = jax.jit(
        shard_map(_body, mesh=mesh,
                  in_specs=(PartitionSpec("core"),) * (n_params + n_outs),
                  out_specs=(PartitionSpec("core"),) * n_outs,
                  check_rep=False),
        donate_argnums=tuple(range(n_params, n_params + n_outs)),
        keep_unused=True,
    )

    def run(in_maps):
        concat_in = [
            np.concatenate([np.asarray(in_maps[c][nm])
                            for c in range(NCORES)], axis=0)
            for nm in in_names
        ]
        out_arrs = sharded(*concat_in, *zero_outs)
        return [
            {nm: np.asarray(out_arrs[i]).reshape(
                NCORES, *out_avals[i].shape)[c]
             for i, nm in enumerate(out_names)}
            for c in range(NCORES)
        ]

    _STATE["runner"] = run
    return run


def _warm():
    """Trigger neuron compile + PJRT executable load with dummy inputs."""
    try:
        nc = _get_nc()
        a = {}
        a["times"] = np.arange(T, dtype=np.float32)
        for nm, sh in [("coeff_a", (B, N, T - 1, IN)),
                       ("coeff_b", (B, N, T - 1, IN)),
                       ("coeff_c2", (B, N, T - 1, IN)),
                       ("coeff_d3", (B, N, T - 1, IN)),
                       ("Wh", (IN, HID)), ("bh", (HID,)),
                       ("Wz", (IN, HID)), ("bz", (HID,)),
                       ("fWin", (HID, HH)), ("fbin", (HH,)),
                       ("fWmid", (HH, HH)), ("fbmid", (HH,)),
                       ("fWout", (HH, HID * IN)), ("fbout", (HID * IN,)),
                       ("gWin", (HID, HH)), ("gbin", (HH,)),
                       ("gE", (N, EMB)), ("gWpool", (EMB, K, HH, HH)),
                       ("gbpool", (EMB, HH)), ("gWout", (HH, HID * HID)),
                       ("gbout", (HID * HID,)), ("convW", (OUT, HID)),
                       ("convb", (OUT,))]:
            a[nm] = np.zeros(sh, np.float32)
        shared, per_core = host_inputs(a)
        in_maps = [{**shared, **pc} for pc in per_core]
        _get_runner()(in_maps)
        _STATE["warm"] = True
    except Exception as e:  # pragma: no cover - keep import usable
        import traceback
        traceback.print_exc()
        _STATE["warm_err"] = e


def kernel(**inputs):
    a = {k_: np.asarray(v, dtype=np.float32) for k_, v in inputs.items()}
    shared, per_core = host_inputs(a)
    in_maps = [{**shared, **pc} for pc in per_core]
    results = _get_runner()(in_maps)
    full = np.empty((B, 1, N, OUT), np.float32)
    for c in range(NCORES):
        o = np.asarray(results[c]["out"])               # (OUT, R)
        full[c * BS:(c + 1) * BS, 0] = (
            o.reshape(OUT, BS, N).transpose(1, 2, 0))
    return full


_warm()


# revision 8
# speedup vs baseline: 31.4524x; 1.0965x over previous
"""NeuralGCDE on 8 NeuronCores: full RK4 ODE integration on device.

Sharding: data-parallel over batch B=16 -> 2 batch elements per core
(rows r = b*N + n, R = 1024 per core). All graph/MLP params replicated.

Device layout is feature-major (features on SBUF partitions, rows on the
free dim). Every contraction is a PE matmul; partition-dim reductions and
broadcasts use structured 0/1 matrices as stationary operands. The
softmax adjacency (exp(relu(gE gE^T)) with row scaling) is built on
device; only the row-sum reciprocals (512 floats) come from host.

All one-time work (bass build, neuron compile, PJRT load) happens at
import; kernel(**inputs) does host repacks + one SPMD dispatch.
"""
import numpy as np

import concourse.bass as bass
import concourse.mybir as mybir
import concourse.tile as tile
from concourse.bass_utils import run_bass_kernel_spmd

B, N, T = 16, 512, 12
IN, HID, HH, EMB, K, OUT = 2, 32, 32, 16, 2, 12
NCORES = 8
BS = B // NCORES            # 2
R = BS * N                  # 1024
NSTEP = T - 1               # 11
F32 = mybir.dt.float32
AF = mybir.ActivationFunctionType
ALU = mybir.AluOpType

# (name, shape) of every shared parameter, packed flat into one upload
_WSPEC = [
    ("recip", (128, 4)), ("gET", (EMB, N)), ("gbpool", (EMB, HH)),
    ("Wh", (IN, HID)), ("bh", (HID, 1)), ("Wz", (IN, HID)), ("bz", (HID, 1)),
    ("fw1", (HID, HH)), ("fb1", (HH, 1)), ("fw2", (HH, HH)), ("fb2", (HH, 1)),
    ("fw3", (HH, HID * IN)), ("fb3", (HID * IN, 1)),
    ("gw1", (HID, HH)), ("gb1", (HH, 1)),
    ("wpk0", (HH, EMB * HH)), ("wpk1", (HH, EMB * HH)),
    ("gwo", (HH, HID * HID)), ("gbo", (128, 8)),
    ("cw", (HID, OUT)), ("cb", (OUT, 1)),
]
_WSIZE = sum(int(np.prod(s)) for _, s in _WSPEC)

_NO_SPILL = {"InstEventSemaphore", "InstUnconditionalBranch",
             "InstConditionalBranch"}


def _spill_excess_waits(nc):
    """Walrus ISA structs hold one sync-wait slot on most instructions.
    Tile can emit several. Move excess waits onto InstEventSemaphore
    carriers inserted just before, on the same engine (waiting earlier on
    the same engine stream is always sound)."""
    nspill = 0
    for f in nc.m.functions:
        for blk in f.blocks:
            lst = blk.instructions
            i = 0
            while i < len(lst):
                ins = lst[i]
                si = ins.sync_info
                if (type(ins).__name__ in _NO_SPILL or si is None
                        or not si.on_wait or len(si.on_wait) <= 1):
                    i += 1
                    continue
                waits = list(si.on_wait)
                keep, excess = waits[-1:], waits[:-1]
                ins.sync_info = mybir.SyncInfo(on_wait=keep,
                                               on_update=list(si.on_update))
                carriers = []
                while excess:
                    chunk, excess = excess[:2], excess[2:]
                    es = mybir.InstEventSemaphore(
                        name=f"Wspill-{nspill}", ins=[], outs=[])
                    nspill += 1
                    es.engine = ins.engine
                    es.sync_info = mybir.SyncInfo(on_wait=chunk, on_update=[])
                    carriers.append(es)
                for k_, es in enumerate(carriers):
                    lst.insert(i + k_, es)
                i += len(carriers) + 1
    return nspill


def build_nc(nstep=NSTEP):
    nc = bass.Bass()

    def dp(name, shape, out=False):
        return nc.declare_dram_parameter(name, list(shape), F32, isOutput=out)

    NU = 3 * nstep + 1                     # unique dX stage rows
    d_pc = dp("pc", ((NU + 1) * IN, R))    # per-core: dx rows + x0 rows
    d_wb = nc.declare_dram_parameter("wb", [_WSIZE], F32, isOutput=False)
    d_out = dp("out", (OUT, R), out=True)

    C5 = 512  # fp32 moving-operand free-dim limit

    from contextlib import ExitStack
    with ExitStack() as es:
        tc = es.enter_context(tile.TileContext(nc))
        sgl = es.enter_context(tc.tile_pool(name="sgl", bufs=1))
        wrk = es.enter_context(tc.tile_pool(name="wrk", bufs=1))
        big1 = es.enter_context(tc.tile_pool(name="big1", bufs=1))
        big2 = es.enter_context(tc.tile_pool(name="big2", bufs=2))
        dxp = es.enter_context(tc.tile_pool(name="dxp", bufs=2))
        pA = es.enter_context(tc.tile_pool(name="pA", bufs=2, space="PSUM"))
        pB = es.enter_context(tc.tile_pool(name="pB", bufs=1, space="PSUM"))
        pT = es.enter_context(tc.tile_pool(name="pT", bufs=2, space="PSUM"))

        woff = [0]

        def load(name, shape):
            p_, f_ = shape
            t = sgl.tile([p_, f_], F32, tag=name, name=name)
            nc.sync.dma_start(
                out=t[:],
                in_=d_wb[woff[0]:woff[0] + p_ * f_].rearrange(
                    "(p f) -> p f", p=p_))
            woff[0] += p_ * f_
            return t

        W = {nm: load(nm, sh) for nm, sh in _WSPEC}
        (recip, gET, gbpool, Wh, bh, Wz, bz, fw1, fb1, fw2, fb2, fw3, fb3,
         gw1, gb1, wpk0, wpk1, gwo, gbo, cw, cb) = (
            W[nm] for nm, _ in _WSPEC)
        x0 = sgl.tile([IN, R], F32, tag="x0", name="x0")
        nc.sync.dma_start(out=x0[:], in_=d_pc[NU * IN:(NU + 1) * IN, :])

        # ---- structured 0/1 matrices, built in place ----
        NE = ALU.not_equal

        def zeros_tile(name, shape):
            t = sgl.tile(list(shape), F32, tag=name, name=name)
            nc.gpsimd.memset(t[:], 0.0)
            return t

        def aff(t, ap, pattern, base=0, cm=0):
            nc.gpsimd.affine_select(out=ap, in_=ap, compare_op=NE, fill=1.0,
                                    base=base, pattern=pattern,
                                    channel_multiplier=cm)

        ident = zeros_tile("ident", (128, 128))
        aff(ident, ident[:], [[-1, 128]], cm=1)
        I32 = zeros_tile("I32", (HH, HH))
        aff(I32, I32[:], [[-1, HH]], cm=1)
        Bc = zeros_tile("Bc", (IN, IN * HID))      # 1 iff col//32 == p
        aff(Bc, Bc[:].rearrange("p (j y) -> p j y", y=HID), [[-1, IN], [0, HID]],
            cm=1)
        Erep = zeros_tile("Erep", (HID, 128))      # 1 iff col%32 == p
        aff(Erep, Erep[:].rearrange("p (j y) -> p j y", y=HID),
            [[0, 4], [-1, HID]], cm=1)
        S3 = zeros_tile("S3", (IN * HID, HID))     # 1 iff p%32 == col
        aff(S3, S3[:], [[-1, HID]], cm=1)
        aff(S3, S3[:], [[-1, HID]], base=-HID, cm=1)
        S2 = zeros_tile("S2", (128, HH))           # 1 iff p%32 == col
        for q in range(4):
            aff(S2, S2[:], [[-1, HH]], base=-q * HH, cm=1)
        Gsel = zeros_tile("Gsel", (EMB, 4 * 128))  # 1 iff col//32 == p
        aff(Gsel, Gsel[:].rearrange("p (j y) -> p j y", y=32),
            [[-1, EMB], [0, 32]], cm=1)
        # Sdz[p, j*32+y] = 1 iff y == 4j + p//32, composed as E4.T @ Cdz
        E4 = zeros_tile("E4", (4, 128))            # 1 iff col//32 == p
        aff(E4, E4[:].rearrange("p (j y) -> p j y", y=32), [[-1, 4], [0, 32]],
            cm=1)
        Cdz = zeros_tile("Cdz", (4, 8 * HID))      # 1 iff y == 4j + p
        aff(Cdz, Cdz[:].rearrange("p (j y) -> p j y", y=HID),
            [[4, 8], [-1, HID]], cm=1)
        sdzp = pA.tile([128, 8 * HID], F32, tag="mm", name="mm")
        nc.tensor.matmul(sdzp[:], E4[:], Cdz[:], start=True, stop=True)
        Sdz = sgl.tile([128, 8 * HID], F32, tag="Sdz", name="Sdz")
        nc.scalar.copy(Sdz[:], sdzp[:])

        # ---- abT[o, n] = (gE @ gbpool).T, used for both batch halves ----
        abp = pA.tile([HH, N], F32, tag="mm", name="mm")
        nc.tensor.matmul(abp[:], gbpool[:], gET[:], start=True, stop=True)
        abT = sgl.tile([HH, N], F32, tag="abT", name="abT")
        nc.scalar.copy(abT[:], abp[:])

        def mm2(ps, lhsT, rhs, start=True, stop=True):
            for c in range(2):
                nc.tensor.matmul(ps[:, c * C5:(c + 1) * C5], lhsT,
                                 rhs[:, c * C5:(c + 1) * C5],
                                 start=start, stop=stop)

        def act(out, in_, func, bias=0.0):
            nc.scalar.activation(out, in_, func, bias=bias)

        # ---- adjacency: expG chunks (exp(relu(gE gE^T)), m-major) ----
        expG = []
        for i in range(4):
            gp = pA.tile([128, N], F32, tag="mm", name="mm")
            nc.tensor.matmul(gp[:], gET[:, i * 128:(i + 1) * 128], gET[:],
                             start=True, stop=True)
            eg = sgl.tile([128, N], F32, tag=f"expG{i}", name=f"expG{i}")
            act(eg[:], gp[:], AF.Relu)
            act(eg[:], eg[:], AF.Exp)
            expG.append(eg)

        # ---- gE_part chunks: gEp_j[p, r] = gE[n(r), (j*128+p)//32] ----
        gEp = []
        for j in range(4):
            ps = pA.tile([128, R], F32, tag="mm", name="mm")
            for c in range(2):
                nc.tensor.matmul(ps[:, c * C5:(c + 1) * C5],
                                 Gsel[:, j * 128:(j + 1) * 128], gET[:],
                                 start=True, stop=True)
            g = sgl.tile([128, R], F32, tag=f"gEp{j}", name=f"gEp{j}")
            nc.scalar.copy(g[:], ps[:])
            gEp.append(g)

        # ---- state: h0 = x0 @ Wh + bh, z0 = x0 @ Wz + bz ----
        h = wrk.tile([HID, R], F32, tag="h", name="h", bufs=2)
        z = wrk.tile([HID, R], F32, tag="z", name="z", bufs=2)
        h0p = pA.tile([HID, R], F32, tag="mm", name="mm")
        mm2(h0p, Wh, x0)
        nc.vector.tensor_scalar_add(h[:], h0p[:], bh[:])
        z0p = pA.tile([HID, R], F32, tag="mm", name="mm")
        mm2(z0p, Wz, x0)
        nc.vector.tensor_scalar_add(z[:], z0p[:], bz[:])

        def vfield(s4, hs, zs, kh, kz):
            u = 3 * (s4 // 4) + (s4 % 4)
            dxs = dxp.tile([IN, R], F32, tag="dxs", name="dxs")
            nc.sync.dma_start(out=dxs[:], in_=d_pc[2 * u:2 * u + 2, :])
            # f path: two relu MLP layers + tanh head (i-major columns)
            x1p = pA.tile([HH, R], F32, tag="mm", name="mm")
            mm2(x1p, fw1, hs)
            x1 = wrk.tile([HH, R], F32, tag="fx", name="fx", bufs=2)
            act(x1[:], x1p[:], AF.Relu, bias=fb1[:])
            x2p = pA.tile([HH, R], F32, tag="mm", name="mm")
            mm2(x2p, fw2, x1)
            x2 = wrk.tile([HH, R], F32, tag="fx", name="fx", bufs=2)
            act(x2[:], x2p[:], AF.Relu, bias=fb2[:])
            vfp = pA.tile([HID * IN, R], F32, tag="mm", name="mm")
            mm2(vfp, fw3, x2)
            vf = wrk.tile([HID * IN, R], F32, tag="vf", name="vf")
            act(vf[:], vfp[:], AF.Tanh, bias=fb3[:])
            # dh = sum_i vf_i * dX_i  (dX broadcast via Bc, reduce via S3)
            dXb = pA.tile([IN * HID, R], F32, tag="mm", name="mm")
            mm2(dXb, Bc, dxs)
            nc.vector.tensor_mul(vf[:], vf[:], dXb[:])
            dhp = pB.tile([HID, R], F32, tag="acc", name="acc")
            mm2(dhp, S3, vf)
            nc.scalar.copy(kh[:], dhp[:])
            drp = pA.tile([128, R], F32, tag="mm", name="mm")
            mm2(drp, Erep, kh)
            dhrep = big1.tile([128, R], F32, tag="dhrep", name="dhrep")
            nc.scalar.copy(dhrep[:], drp[:])
            # g path: relu layer (feature-major), node-major transposes
            x1gp = pA.tile([HH, R], F32, tag="mm", name="mm")
            mm2(x1gp, gw1, zs)
            x1g = wrk.tile([HH, R], F32, tag="x1g", name="x1g")
            act(x1g[:], x1gp[:], AF.Relu, bias=gb1[:])
            xT = []
            for k_ in range(4):
                xtp = pT.tile([128, 2 * HH], F32, tag="pt", name="pt")
                for b_ in range(2):
                    nc.tensor.transpose(
                        xtp[:, b_ * HH:(b_ + 1) * HH],
                        x1g[:, b_ * N + k_ * 128: b_ * N + (k_ + 1) * 128],
                        ident[:HH, :HH])
                xt = wrk.tile([128, 2 * HH], F32, tag=f"xT{k_}",
                              name=f"xT{k_}")
                nc.vector.tensor_copy(xt[:], xtp[:])
                xT.append(xt)
            # graph conv: xg1 = A @ x1g per batch, recip folded in
            xg1n = []
            for i in range(4):
                xgp = pT.tile([128, 2 * HH], F32, tag="pt", name="pt")
                for k_ in range(4):
                    nc.tensor.matmul(xgp[:],
                                     expG[k_][:, i * 128:(i + 1) * 128],
                                     xT[k_][:],
                                     start=(k_ == 0), stop=(k_ == 3))
                xn = wrk.tile([128, 2 * HH], F32, tag=f"xg1n{i}",
                              name=f"xg1n{i}")
                nc.vector.tensor_scalar_mul(xn[:], xgp[:], recip[:, i:i + 1])
                xg1n.append(xn)
            xg1f = wrk.tile([HH, R], F32, tag="xg1f", name="xg1f")
            for i in range(4):
                for b_ in range(2):
                    btp = pT.tile([HH, 128], F32, tag="pt", name="pt")
                    nc.tensor.transpose(btp[:],
                                        xg1n[i][:, b_ * HH:(b_ + 1) * HH],
                                        ident[:, :])
                    nc.scalar.copy(
                        xg1f[:, b_ * N + i * 128: b_ * N + (i + 1) * 128],
                        btp[:])
            # per-node pooled weights: y = Wp^T xg scaled by gE_part,
            # reduced over EMB via S2 into x2g (abf preloaded via I32)
            x2gp = pB.tile([HH, R], F32, tag="acc", name="acc")
            for c in range(2):
                nc.tensor.matmul(x2gp[:, c * C5:(c + 1) * C5], I32[:],
                                 abT[:],
                                 start=True, stop=False, skip_group_check=True)
            for j in range(4):
                yp = pA.tile([128, R], F32, tag="mm", name="mm")
                for c in range(2):
                    sl = slice(c * C5, (c + 1) * C5)
                    nc.tensor.matmul(yp[:, sl], wpk0[:, j * 128:(j + 1) * 128],
                                     x1g[:, sl], start=True, stop=False)
                    nc.tensor.matmul(yp[:, sl], wpk1[:, j * 128:(j + 1) * 128],
                                     xg1f[:, sl], start=False, stop=True)
                t_ = big1.tile([128, R], F32, tag="ty", name="ty", bufs=2)
                nc.vector.tensor_mul(t_[:], yp[:], gEp[j][:])
                for c in range(2):
                    sl = slice(c * C5, (c + 1) * C5)
                    nc.tensor.matmul(x2gp[:, sl], S2[:], t_[:, sl],
                                     start=False, stop=(j == 3),
                                     skip_group_check=True)
            x2g = wrk.tile([HH, R], F32, tag="x2g", name="x2g")
            nc.scalar.copy(x2g[:], x2gp[:])
            # vg chunks; dz = sum vg_ho * dh_o accumulated via Sdz
            dzp = pB.tile([HID, R], F32, tag="acc", name="acc")
            for j in range(8):
                vgp = pA.tile([128, R], F32, tag="mm", name="mm")
                mm2(vgp, gwo[:, j * 128:(j + 1) * 128], x2g)
                vg = big2.tile([128, R], F32, tag="vg", name="vg")
                act(vg[:], vgp[:], AF.Tanh, bias=gbo[:, j:j + 1])
                nc.vector.tensor_mul(vg[:], vg[:], dhrep[:])
                for c in range(2):
                    sl = slice(c * C5, (c + 1) * C5)
                    nc.tensor.matmul(dzp[:, sl],
                                     Sdz[:, j * HID:(j + 1) * HID],
                                     vg[:, sl],
                                     start=(j == 0), stop=(j == 7),
                                     skip_group_check=True)
            nc.scalar.copy(kz[:], dzp[:])

        TT = nc.vector.tensor_tensor
        STT = nc.vector.scalar_tensor_tensor

        # RK4 with 3/8 rule, dt = 1 (times are arange; asserted on host)
        for s in range(nstep):
            kh = [wrk.tile([HID, R], F32, tag=f"kh{st}", name=f"kh{st}")
                  for st in range(4)]
            kz = [wrk.tile([HID, R], F32, tag=f"kz{st}", name=f"kz{st}")
                  for st in range(4)]
            vfield(4 * s + 0, h, z, kh[0], kz[0])
            hs = wrk.tile([HID, R], F32, tag="hs", name="hs", bufs=2)
            zs = wrk.tile([HID, R], F32, tag="zs", name="zs", bufs=2)
            STT(hs[:], kh[0][:], 1.0 / 3.0, h[:], op0=ALU.mult, op1=ALU.add)
            STT(zs[:], kz[0][:], 1.0 / 3.0, z[:], op0=ALU.mult, op1=ALU.add)
            vfield(4 * s + 1, hs, zs, kh[1], kz[1])
            hs2 = wrk.tile([HID, R], F32, tag="hs", name="hs", bufs=2)
            zs2 = wrk.tile([HID, R], F32, tag="zs", name="zs", bufs=2)
            STT(hs2[:], kh[0][:], -1.0 / 3.0, kh[1][:],
                op0=ALU.mult, op1=ALU.add)
            TT(hs2[:], hs2[:], h[:], op=ALU.add)
            STT(zs2[:], kz[0][:], -1.0 / 3.0, kz[1][:],
                op0=ALU.mult, op1=ALU.add)
            TT(zs2[:], zs2[:], z[:], op=ALU.add)
            vfield(4 * s + 2, hs2, zs2, kh[2], kz[2])
            hs3 = wrk.tile([HID, R], F32, tag="hs", name="hs", bufs=2)
            zs3 = wrk.tile([HID, R], F32, tag="zs", name="zs", bufs=2)
            STT(hs3[:], kh[1][:], -1.0, kh[0][:], op0=ALU.mult, op1=ALU.add)
            TT(hs3[:], hs3[:], kh[2][:], op=ALU.add)
            TT(hs3[:], hs3[:], h[:], op=ALU.add)
            STT(zs3[:], kz[1][:], -1.0, kz[0][:], op0=ALU.mult, op1=ALU.add)
            TT(zs3[:], zs3[:], kz[2][:], op=ALU.add)
            TT(zs3[:], zs3[:], z[:], op=ALU.add)
            vfield(4 * s + 3, hs3, zs3, kh[3], kz[3])
            hn = wrk.tile([HID, R], F32, tag="h", name="h", bufs=2)
            zn = wrk.tile([HID, R], F32, tag="z", name="z", bufs=2)
            TT(kh[1][:], kh[1][:], kh[2][:], op=ALU.add)
            STT(kh[1][:], kh[1][:], 3.0, kh[0][:], op0=ALU.mult, op1=ALU.add)
            TT(kh[1][:], kh[1][:], kh[3][:], op=ALU.add)
            STT(hn[:], kh[1][:], 0.125, h[:], op0=ALU.mult, op1=ALU.add)
            TT(kz[1][:], kz[1][:], kz[2][:], op=ALU.add)
            STT(kz[1][:], kz[1][:], 3.0, kz[0][:], op0=ALU.mult, op1=ALU.add)
            TT(kz[1][:], kz[1][:], kz[3][:], op=ALU.add)
            STT(zn[:], kz[1][:], 0.125, z[:], op0=ALU.mult, op1=ALU.add)
            h, z = hn, zn

        # ---- end conv ----
        op = pB.tile([OUT, R], F32, tag="acc", name="acc")
        mm2(op, cw, z)
        ob = wrk.tile([OUT, R], F32, tag="x2g", name="x2g")
        nc.vector.tensor_scalar_add(ob[:], op[:], cb[:])
        nc.sync.dma_start(out=d_out[:], in_=ob[:])

    _spill_excess_waits(nc)
    return nc


# ------------------------------------------------------------------
# host-side preprocessing
# ------------------------------------------------------------------
def host_inputs(a, nstep=NSTEP):
    gE = a["gE"]
    times = a["times"]
    assert np.allclose(np.diff(times), 1.0, atol=1e-5), "RK dt=1 baked in"
    maxlen = T - 2
    # unique stage times: u = 3s + st (stage 4 of step s == stage 0 of s+1)
    ts_list = [float(times[0])]
    for s in range(nstep):
        t0, t1 = float(times[s]), float(times[s + 1])
        dt = t1 - t0
        ts_list += [t0 + dt / 3.0, t0 + 2.0 * dt / 3.0, t1]
    nu = len(ts_list)
    dX = np.empty((B, N, nu, IN), np.float32)
    for q, t_ in enumerate(ts_list):
        idx = int(np.clip(np.sum(np.float32(t_) > times) - 1, 0, maxlen))
        frac = np.float32(t_ - times[idx])
        dX[:, :, q, :] = (a["coeff_b"][:, :, idx]
                          + (a["coeff_c2"][:, :, idx]
                             + a["coeff_d3"][:, :, idx] * frac) * frac)
    x0 = a["coeff_a"][:, :, 0, :]                   # (B, N, IN)

    fw3 = np.empty((HH, HID * IN), np.float32)
    fb3 = np.empty((HID * IN, 1), np.float32)
    for h_ in range(HID):
        for i in range(IN):
            fw3[:, i * HID + h_] = a["fWout"][:, h_ * IN + i]
            fb3[i * HID + h_, 0] = a["fbout"][h_ * IN + i]

    wpk = np.ascontiguousarray(
        np.transpose(a["gWpool"], (1, 2, 0, 3)).reshape(K, HH, EMB * HH))
    G = np.maximum(gE @ gE.T, 0.0).astype(np.float32)
    rs = np.exp(G).sum(axis=1)
    recip = np.ascontiguousarray((1.0 / rs).reshape(4, 128).T)

    vals = {
        "recip": recip, "gET": gE.T, "gbpool": a["gbpool"],
        "Wh": a["Wh"], "bh": a["bh"].reshape(-1, 1),
        "Wz": a["Wz"], "bz": a["bz"].reshape(-1, 1),
        "fw1": a["fWin"], "fb1": a["fbin"].reshape(-1, 1),
        "fw2": a["fWmid"], "fb2": a["fbmid"].reshape(-1, 1),
        "fw3": fw3, "fb3": fb3,
        "gw1": a["gWin"], "gb1": a["gbin"].reshape(-1, 1),
        "wpk0": wpk[0], "wpk1": wpk[1],
        "gwo": a["gWout"],
        "gbo": np.ascontiguousarray(a["gbout"].reshape(8, 128).T),
        "cw": np.ascontiguousarray(a["convW"].T),
        "cb": a["convb"].reshape(-1, 1),
    }
    wb = np.concatenate(
        [np.ascontiguousarray(vals[nm]).astype(np.float32).ravel()
         for nm, _ in _WSPEC])
    assert wb.size == _WSIZE
    shared = {"wb": wb}

    per_core = []
    for c in range(NCORES):
        sl = slice(c * BS, (c + 1) * BS)
        dxc = np.transpose(dX[sl], (2, 3, 0, 1)).reshape(nu * IN, R)
        x0c = np.transpose(x0[sl], (2, 0, 1)).reshape(IN, R)
        per_core.append({
            "pc": np.ascontiguousarray(
                np.concatenate([dxc, x0c], axis=0)).astype(np.float32),
        })
    return shared, per_core


_STATE = {}


def _get_nc():
    if "nc" not in _STATE:
        _STATE["nc"] = build_nc()
    return _STATE["nc"]


def _get_runner():
    """Cached jit(shard_map(bass_exec)) callable — built once so per-call
    cost is dispatch only (run_bass_kernel_spmd re-traces every call)."""
    if "runner" in _STATE:
        return _STATE["runner"]
    import jax
    from jax.sharding import Mesh, PartitionSpec
    from jax.experimental.shard_map import shard_map
    from concourse import bass2jax as b2j

    b2j.install_neuronx_cc_hook()
    nc = _get_nc()
    assert nc.dbg_addr is None
    partition_name = (nc.partition_id_tensor.name
                      if nc.partition_id_tensor else None)
    in_names, out_names, out_avals, zero_outs = [], [], [], []
    for alloc in nc.m.functions[0].allocations:
        if not isinstance(alloc, mybir.MemoryLocationSet):
            continue
        name = alloc.memorylocations[0].name
        if alloc.kind == "ExternalInput":
            if name != partition_name:
                in_names.append(name)
        elif alloc.kind == "ExternalOutput":
            shape = tuple(alloc.tensor_shape)
            dtype = mybir.dt.np(alloc.dtype)
            out_names.append(name)
            out_avals.append(jax.core.ShapedArray(shape, dtype))
            zero_outs.append(np.zeros((NCORES * shape[0], *shape[1:]), dtype))
    n_params = len(in_names)
    all_names = list(in_names) + list(out_names)
    if partition_name is not None:
        all_names.append(partition_name)

    def _body(*args):
        operands = list(args)
        if partition_name is not None:
            operands.append(b2j.partition_id_tensor())
        outs = b2j._bass_exec_p.bind(
            *operands,
            out_avals=tuple(out_avals),
            in_names=tuple(all_names),
            out_names=tuple(out_names),
            lowering_input_output_aliases=(),
            sim_require_finite=True,
            sim_require_nnan=True,
            nc=nc,
        )
        return tuple(outs)

    devices = jax.devices()[:NCORES]
    mesh = Mesh(np.asarray(devices), ("core",))
    n_outs = len(out_names)
    sharded = jax.jit(
        shard_map(_body, mesh=mesh,
                  in_specs=(PartitionSpec("core"),) * (n_params + n_outs),
                  out_specs=(PartitionSpec("core"),) * n_outs,
                  check_rep=False),
        donate_argnums=tuple(range(n_params, n_params + n_outs)),
        keep_unused=True,
    )

    def run(in_maps):
        concat_in = [
            np.concatenate([np.asarray(in_maps[c][nm])
                            for c in range(NCORES)], axis=0)
            for nm in in_names
        ]
        out_arrs = sharded(*concat_in, *zero_outs)
        return [
            {nm: np.asarray(out_arrs[i]).reshape(
                NCORES, *out_avals[i].shape)[c]
             for i, nm in enumerate(out_names)}
            for c in range(NCORES)
        ]

    _STATE["runner"] = run
    return run


def _warm():
    """Trigger neuron compile + PJRT executable load with dummy inputs."""
    try:
        nc = _get_nc()
        a = {}
        a["times"] = np.arange(T, dtype=np.float32)
        for nm, sh in [("coeff_a", (B, N, T - 1, IN)),
                       ("coeff_b", (B, N, T - 1, IN)),
                       ("coeff_c2", (B, N, T - 1, IN)),
                       ("coeff_d3", (B, N, T - 1, IN)),
                       ("Wh", (IN, HID)), ("bh", (HID,)),
                       ("Wz", (IN, HID)), ("bz", (HID,)),
                       ("fWin", (HID, HH)), ("fbin", (HH,)),
                       ("fWmid", (HH, HH)), ("fbmid", (HH,)),
                       ("fWout", (HH, HID * IN)), ("fbout", (HID * IN,)),
                       ("gWin", (HID, HH)), ("gbin", (HH,)),
                       ("gE", (N, EMB)), ("gWpool", (EMB, K, HH, HH)),
                       ("gbpool", (EMB, HH)), ("gWout", (HH, HID * HID)),
                       ("gbout", (HID * HID,)), ("convW", (OUT, HID)),
                       ("convb", (OUT,))]:
            a[nm] = np.zeros(sh, np.float32)
        shared, per_core = host_inputs(a)
        in_maps = [{**shared, **pc} for pc in per_core]
        _get_runner()(in_maps)
        _STATE["warm"] = True
    except Exception as e:  # pragma: no cover - keep import usable
        import traceback
        traceback.print_exc()
        _STATE["warm_err"] = e


def kernel(**inputs):
    a = {k_: np.asarray(v, dtype=np.float32) for k_, v in inputs.items()}
    shared, per_core = host_inputs(a)
    in_maps = [{**shared, **pc} for pc in per_core]
    results = _get_runner()(in_maps)
    full = np.empty((B, 1, N, OUT), np.float32)
    for c in range(NCORES):
        o = np.asarray(results[c]["out"])               # (OUT, R)
        full[c * BS:(c + 1) * BS, 0] = (
            o.reshape(OUT, BS, N).transpose(1, 2, 0))
    return full


_warm()


# revision 9
# speedup vs baseline: 39.0162x; 1.2405x over previous
"""NeuralGCDE on 8 NeuronCores: full RK4 ODE integration on device.

Sharding: data-parallel over batch B=16 -> 2 batch elements per core
(rows r = b*N + n, R = 1024 per core). All graph/MLP params replicated.

Device layout is feature-major (features on SBUF partitions, rows on the
free dim). Every contraction is a PE matmul; partition-dim reductions and
broadcasts use structured 0/1 matrices as stationary operands. The
softmax adjacency (exp(relu(gE gE^T)) with row scaling) is built on
device; only the row-sum reciprocals (512 floats) come from host.

All one-time work (bass build, neuron compile, PJRT load) happens at
import; kernel(**inputs) does host repacks + one SPMD dispatch.
"""
import numpy as np

import concourse.bass as bass
import concourse.mybir as mybir
import concourse.tile as tile
from concourse.bass_utils import run_bass_kernel_spmd

B, N, T = 16, 512, 12
IN, HID, HH, EMB, K, OUT = 2, 32, 32, 16, 2, 12
NCORES = 8
BS = B // NCORES            # 2
R = BS * N                  # 1024
NSTEP = T - 1               # 11
F32 = mybir.dt.float32
AF = mybir.ActivationFunctionType
ALU = mybir.AluOpType

# (name, shape) of every shared parameter, packed flat into one upload
_WSPEC = [
    ("recip", (128, 4)), ("gET", (EMB, N)), ("gbpool", (EMB, HH)),
    ("Wh", (IN, HID)), ("bh", (HID, 1)), ("Wz", (IN, HID)), ("bz", (HID, 1)),
    ("fw1", (HID, HH)), ("fb1", (HH, 1)), ("fw2", (HH, HH)), ("fb2", (HH, 1)),
    ("fw3", (HH, HID * IN)), ("fb3", (HID * IN, 1)),
    ("gw1", (HID, HH)), ("gb1", (HH, 1)),
    ("wpk0", (HH, EMB * HH)), ("wpk1", (HH, EMB * HH)),
    ("gwo", (HH, HID * HID)), ("gbo", (128, 8)),
    ("cw", (HID, OUT)), ("cb", (OUT, 1)),
]
_WSIZE = sum(int(np.prod(s)) for _, s in _WSPEC)
_WPAD = ((_WSIZE + 7) // 8) * 8
_WSH = _WPAD // 8

_NO_SPILL = {"InstEventSemaphore", "InstUnconditionalBranch",
             "InstConditionalBranch"}


def _spill_excess_waits(nc):
    """Walrus ISA structs hold one sync-wait slot on most instructions.
    Tile can emit several. Move excess waits onto InstEventSemaphore
    carriers inserted just before, on the same engine (waiting earlier on
    the same engine stream is always sound)."""
    nspill = 0
    for f in nc.m.functions:
        for blk in f.blocks:
            lst = blk.instructions
            i = 0
            while i < len(lst):
                ins = lst[i]
                si = ins.sync_info
                if (type(ins).__name__ in _NO_SPILL or si is None
                        or not si.on_wait or len(si.on_wait) <= 1):
                    i += 1
                    continue
                waits = list(si.on_wait)
                keep, excess = waits[-1:], waits[:-1]
                ins.sync_info = mybir.SyncInfo(on_wait=keep,
                                               on_update=list(si.on_update))
                carriers = []
                while excess:
                    chunk, excess = excess[:2], excess[2:]
                    es = mybir.InstEventSemaphore(
                        name=f"Wspill-{nspill}", ins=[], outs=[])
                    nspill += 1
                    es.engine = ins.engine
                    es.sync_info = mybir.SyncInfo(on_wait=chunk, on_update=[])
                    carriers.append(es)
                for k_, es in enumerate(carriers):
                    lst.insert(i + k_, es)
                i += len(carriers) + 1
    return nspill


def build_nc(nstep=NSTEP):
    nc = bass.Bass()

    def dp(name, shape, out=False):
        return nc.declare_dram_parameter(name, list(shape), F32, isOutput=out)

    NU = 3 * nstep + 1                     # unique dX stage rows
    d_pc = dp("pc", ((NU + 1) * IN, R))    # per-core: dx rows + x0 rows
    d_wb = nc.declare_dram_parameter("wb", [_WSH], F32, isOutput=False)
    wb_in = nc.dram_tensor("wb_in", [_WSH], F32)
    wb_all = nc.dram_tensor("wb_all", [_WPAD], F32)
    d_out = dp("out", (OUT, R), out=True)

    C5 = 512  # fp32 moving-operand free-dim limit

    from contextlib import ExitStack
    with ExitStack() as es:
        tc = es.enter_context(tile.TileContext(nc))
        sgl = es.enter_context(tc.tile_pool(name="sgl", bufs=1))
        wrk = es.enter_context(tc.tile_pool(name="wrk", bufs=1))
        big1 = es.enter_context(tc.tile_pool(name="big1", bufs=1))
        big2 = es.enter_context(tc.tile_pool(name="big2", bufs=2))
        dxp = es.enter_context(tc.tile_pool(name="dxp", bufs=2))
        pA = es.enter_context(tc.tile_pool(name="pA", bufs=2, space="PSUM"))
        pB = es.enter_context(tc.tile_pool(name="pB", bufs=1, space="PSUM"))
        pT = es.enter_context(tc.tile_pool(name="pT", bufs=2, space="PSUM"))

        # gather the weight blob: each core uploaded 1/8th
        nc.sync.dma_start(out=wb_in[:], in_=d_wb[:])
        nc.gpsimd.collective_compute(
            "AllGather", ALU.bypass,
            replica_groups=[list(range(NCORES))],
            ins=[wb_in[:]], outs=[wb_all[:]])

        woff = [0]

        def load(name, shape):
            p_, f_ = shape
            t = sgl.tile([p_, f_], F32, tag=name, name=name)
            nc.sync.dma_start(
                out=t[:],
                in_=wb_all[woff[0]:woff[0] + p_ * f_].rearrange(
                    "(p f) -> p f", p=p_))
            woff[0] += p_ * f_
            return t

        W = {nm: load(nm, sh) for nm, sh in _WSPEC}
        (recip, gET, gbpool, Wh, bh, Wz, bz, fw1, fb1, fw2, fb2, fw3, fb3,
         gw1, gb1, wpk0, wpk1, gwo, gbo, cw, cb) = (
            W[nm] for nm, _ in _WSPEC)
        x0 = sgl.tile([IN, R], F32, tag="x0", name="x0")
        nc.sync.dma_start(out=x0[:], in_=d_pc[NU * IN:(NU + 1) * IN, :])

        # ---- structured 0/1 matrices, built in place ----
        NE = ALU.not_equal

        def zeros_tile(name, shape):
            t = sgl.tile(list(shape), F32, tag=name, name=name)
            nc.gpsimd.memset(t[:], 0.0)
            return t

        def aff(t, ap, pattern, base=0, cm=0):
            nc.gpsimd.affine_select(out=ap, in_=ap, compare_op=NE, fill=1.0,
                                    base=base, pattern=pattern,
                                    channel_multiplier=cm)

        ident = zeros_tile("ident", (128, 128))
        aff(ident, ident[:], [[-1, 128]], cm=1)
        I32 = zeros_tile("I32", (HH, HH))
        aff(I32, I32[:], [[-1, HH]], cm=1)
        Bc = zeros_tile("Bc", (IN, IN * HID))      # 1 iff col//32 == p
        aff(Bc, Bc[:].rearrange("p (j y) -> p j y", y=HID), [[-1, IN], [0, HID]],
            cm=1)
        Erep = zeros_tile("Erep", (HID, 128))      # 1 iff col%32 == p
        aff(Erep, Erep[:].rearrange("p (j y) -> p j y", y=HID),
            [[0, 4], [-1, HID]], cm=1)
        S3 = zeros_tile("S3", (IN * HID, HID))     # 1 iff p%32 == col
        aff(S3, S3[:], [[-1, HID]], cm=1)
        aff(S3, S3[:], [[-1, HID]], base=-HID, cm=1)
        S2 = zeros_tile("S2", (128, HH))           # 1 iff p%32 == col
        for q in range(4):
            aff(S2, S2[:], [[-1, HH]], base=-q * HH, cm=1)
        Gsel = zeros_tile("Gsel", (EMB, 4 * 128))  # 1 iff col//32 == p
        aff(Gsel, Gsel[:].rearrange("p (j y) -> p j y", y=32),
            [[-1, EMB], [0, 32]], cm=1)
        # Sdz[p, j*32+y] = 1 iff y == 4j + p//32, composed as E4.T @ Cdz
        E4 = zeros_tile("E4", (4, 128))            # 1 iff col//32 == p
        aff(E4, E4[:].rearrange("p (j y) -> p j y", y=32), [[-1, 4], [0, 32]],
            cm=1)
        Cdz = zeros_tile("Cdz", (4, 8 * HID))      # 1 iff y == 4j + p
        aff(Cdz, Cdz[:].rearrange("p (j y) -> p j y", y=HID),
            [[4, 8], [-1, HID]], cm=1)
        sdzp = pA.tile([128, 8 * HID], F32, tag="mm", name="mm")
        nc.tensor.matmul(sdzp[:], E4[:], Cdz[:], start=True, stop=True)
        Sdz = sgl.tile([128, 8 * HID], F32, tag="Sdz", name="Sdz")
        nc.scalar.copy(Sdz[:], sdzp[:])

        # ---- abT[o, n] = (gE @ gbpool).T, used for both batch halves ----
        abp = pA.tile([HH, N], F32, tag="mm", name="mm")
        nc.tensor.matmul(abp[:], gbpool[:], gET[:], start=True, stop=True)
        abT = sgl.tile([HH, N], F32, tag="abT", name="abT")
        nc.scalar.copy(abT[:], abp[:])

        def mm2(ps, lhsT, rhs, start=True, stop=True):
            for c in range(2):
                nc.tensor.matmul(ps[:, c * C5:(c + 1) * C5], lhsT,
                                 rhs[:, c * C5:(c + 1) * C5],
                                 start=start, stop=stop)

        def act(out, in_, func, bias=0.0):
            nc.scalar.activation(out, in_, func, bias=bias)

        # ---- adjacency: expG chunks (exp(relu(gE gE^T)), m-major) ----
        expG = []
        for i in range(4):
            gp = pA.tile([128, N], F32, tag="mm", name="mm")
            nc.tensor.matmul(gp[:], gET[:, i * 128:(i + 1) * 128], gET[:],
                             start=True, stop=True)
            eg = sgl.tile([128, N], F32, tag=f"expG{i}", name=f"expG{i}")
            act(eg[:], gp[:], AF.Relu)
            act(eg[:], eg[:], AF.Exp)
            expG.append(eg)

        # ---- gE_part chunks: gEp_j[p, r] = gE[n(r), (j*128+p)//32] ----
        gEp = []
        for j in range(4):
            ps = pA.tile([128, R], F32, tag="mm", name="mm")
            for c in range(2):
                nc.tensor.matmul(ps[:, c * C5:(c + 1) * C5],
                                 Gsel[:, j * 128:(j + 1) * 128], gET[:],
                                 start=True, stop=True)
            g = sgl.tile([128, R], F32, tag=f"gEp{j}", name=f"gEp{j}")
            nc.scalar.copy(g[:], ps[:])
            gEp.append(g)

        # ---- state: h0 = x0 @ Wh + bh, z0 = x0 @ Wz + bz ----
        h = wrk.tile([HID, R], F32, tag="h", name="h", bufs=2)
        z = wrk.tile([HID, R], F32, tag="z", name="z", bufs=2)
        h0p = pA.tile([HID, R], F32, tag="mm", name="mm")
        mm2(h0p, Wh, x0)
        nc.vector.tensor_scalar_add(h[:], h0p[:], bh[:])
        z0p = pA.tile([HID, R], F32, tag="mm", name="mm")
        mm2(z0p, Wz, x0)
        nc.vector.tensor_scalar_add(z[:], z0p[:], bz[:])

        def vfield(s4, hs, zs, kh, kz):
            u = 3 * (s4 // 4) + (s4 % 4)
            dxs = dxp.tile([IN, R], F32, tag="dxs", name="dxs")
            nc.sync.dma_start(out=dxs[:], in_=d_pc[2 * u:2 * u + 2, :])
            # f path: two relu MLP layers + tanh head (i-major columns)
            x1p = pA.tile([HH, R], F32, tag="mm", name="mm")
            mm2(x1p, fw1, hs)
            x1 = wrk.tile([HH, R], F32, tag="fx", name="fx", bufs=2)
            act(x1[:], x1p[:], AF.Relu, bias=fb1[:])
            x2p = pA.tile([HH, R], F32, tag="mm", name="mm")
            mm2(x2p, fw2, x1)
            x2 = wrk.tile([HH, R], F32, tag="fx", name="fx", bufs=2)
            act(x2[:], x2p[:], AF.Relu, bias=fb2[:])
            vfp = pA.tile([HID * IN, R], F32, tag="mm", name="mm")
            mm2(vfp, fw3, x2)
            vf = wrk.tile([HID * IN, R], F32, tag="vf", name="vf")
            act(vf[:], vfp[:], AF.Tanh, bias=fb3[:])
            # dh = sum_i vf_i * dX_i  (dX broadcast via Bc, reduce via S3)
            dXb = pA.tile([IN * HID, R], F32, tag="mm", name="mm")
            mm2(dXb, Bc, dxs)
            nc.vector.tensor_mul(vf[:], vf[:], dXb[:])
            dhp = pB.tile([HID, R], F32, tag="acc", name="acc")
            mm2(dhp, S3, vf)
            nc.scalar.copy(kh[:], dhp[:])
            drp = pA.tile([128, R], F32, tag="mm", name="mm")
            mm2(drp, Erep, kh)
            dhrep = big1.tile([128, R], F32, tag="dhrep", name="dhrep")
            nc.scalar.copy(dhrep[:], drp[:])
            # g path: relu layer (feature-major), node-major transposes
            x1gp = pA.tile([HH, R], F32, tag="mm", name="mm")
            mm2(x1gp, gw1, zs)
            x1g = wrk.tile([HH, R], F32, tag="x1g", name="x1g")
            act(x1g[:], x1gp[:], AF.Relu, bias=gb1[:])
            xT = []
            for k_ in range(4):
                xtp = pT.tile([128, 2 * HH], F32, tag="pt", name="pt")
                for b_ in range(2):
                    nc.tensor.transpose(
                        xtp[:, b_ * HH:(b_ + 1) * HH],
                        x1g[:, b_ * N + k_ * 128: b_ * N + (k_ + 1) * 128],
                        ident[:HH, :HH])
                xt = wrk.tile([128, 2 * HH], F32, tag=f"xT{k_}",
                              name=f"xT{k_}")
                nc.vector.tensor_copy(xt[:], xtp[:])
                xT.append(xt)
            # graph conv: xg1 = A @ x1g per batch, recip folded in
            xg1n = []
            for i in range(4):
                xgp = pT.tile([128, 2 * HH], F32, tag="pt", name="pt")
                for k_ in range(4):
                    nc.tensor.matmul(xgp[:],
                                     expG[k_][:, i * 128:(i + 1) * 128],
                                     xT[k_][:],
                                     start=(k_ == 0), stop=(k_ == 3))
                xn = wrk.tile([128, 2 * HH], F32, tag=f"xg1n{i}",
                              name=f"xg1n{i}")
                nc.vector.tensor_scalar_mul(xn[:], xgp[:], recip[:, i:i + 1])
                xg1n.append(xn)
            xg1f = wrk.tile([HH, R], F32, tag="xg1f", name="xg1f")
            for i in range(4):
                for b_ in range(2):
                    btp = pT.tile([HH, 128], F32, tag="pt", name="pt")
                    nc.tensor.transpose(btp[:],
                                        xg1n[i][:, b_ * HH:(b_ + 1) * HH],
                                        ident[:, :])
                    nc.scalar.copy(
                        xg1f[:, b_ * N + i * 128: b_ * N + (i + 1) * 128],
                        btp[:])
            # per-node pooled weights: y = Wp^T xg scaled by gE_part,
            # reduced over EMB via S2 into x2g (abf preloaded via I32)
            x2gp = pB.tile([HH, R], F32, tag="acc", name="acc")
            for c in range(2):
                nc.tensor.matmul(x2gp[:, c * C5:(c + 1) * C5], I32[:],
                                 abT[:],
                                 start=True, stop=False, skip_group_check=True)
            for j in range(4):
                yp = pA.tile([128, R], F32, tag="mm", name="mm")
                for c in range(2):
                    sl = slice(c * C5, (c + 1) * C5)
                    nc.tensor.matmul(yp[:, sl], wpk0[:, j * 128:(j + 1) * 128],
                                     x1g[:, sl], start=True, stop=False)
                    nc.tensor.matmul(yp[:, sl], wpk1[:, j * 128:(j + 1) * 128],
                                     xg1f[:, sl], start=False, stop=True)
                t_ = big1.tile([128, R], F32, tag="ty", name="ty", bufs=2)
                nc.vector.tensor_mul(t_[:], yp[:], gEp[j][:])
                for c in range(2):
                    sl = slice(c * C5, (c + 1) * C5)
                    nc.tensor.matmul(x2gp[:, sl], S2[:], t_[:, sl],
                                     start=False, stop=(j == 3),
                                     skip_group_check=True)
            x2g = wrk.tile([HH, R], F32, tag="x2g", name="x2g")
            nc.scalar.copy(x2g[:], x2gp[:])
            # vg chunks; dz = sum vg_ho * dh_o accumulated via Sdz
            dzp = pB.tile([HID, R], F32, tag="acc", name="acc")
            for j in range(8):
                vgp = pA.tile([128, R], F32, tag="mm", name="mm")
                mm2(vgp, gwo[:, j * 128:(j + 1) * 128], x2g)
                vg = big2.tile([128, R], F32, tag="vg", name="vg")
                act(vg[:], vgp[:], AF.Tanh, bias=gbo[:, j:j + 1])
                nc.vector.tensor_mul(vg[:], vg[:], dhrep[:])
                for c in range(2):
                    sl = slice(c * C5, (c + 1) * C5)
                    nc.tensor.matmul(dzp[:, sl],
                                     Sdz[:, j * HID:(j + 1) * HID],
                                     vg[:, sl],
                                     start=(j == 0), stop=(j == 7),
                                     skip_group_check=True)
            nc.scalar.copy(kz[:], dzp[:])

        TT = nc.vector.tensor_tensor
        STT = nc.vector.scalar_tensor_tensor

        # RK4 with 3/8 rule, dt = 1 (times are arange; asserted on host)
        for s in range(nstep):
            kh = [wrk.tile([HID, R], F32, tag=f"kh{st}", name=f"kh{st}")
                  for st in range(4)]
            kz = [wrk.tile([HID, R], F32, tag=f"kz{st}", name=f"kz{st}")
                  for st in range(4)]
            vfield(4 * s + 0, h, z, kh[0], kz[0])
            hs = wrk.tile([HID, R], F32, tag="hs", name="hs", bufs=2)
            zs = wrk.tile([HID, R], F32, tag="zs", name="zs", bufs=2)
            STT(hs[:], kh[0][:], 1.0 / 3.0, h[:], op0=ALU.mult, op1=ALU.add)
            STT(zs[:], kz[0][:], 1.0 / 3.0, z[:], op0=ALU.mult, op1=ALU.add)
            vfield(4 * s + 1, hs, zs, kh[1], kz[1])
            hs2 = wrk.tile([HID, R], F32, tag="hs", name="hs", bufs=2)
            zs2 = wrk.tile([HID, R], F32, tag="zs", name="zs", bufs=2)
            STT(hs2[:], kh[0][:], -1.0 / 3.0, kh[1][:],
                op0=ALU.mult, op1=ALU.add)
            TT(hs2[:], hs2[:], h[:], op=ALU.add)
            STT(zs2[:], kz[0][:], -1.0 / 3.0, kz[1][:],
                op0=ALU.mult, op1=ALU.add)
            TT(zs2[:], zs2[:], z[:], op=ALU.add)
            vfield(4 * s + 2, hs2, zs2, kh[2], kz[2])
            hs3 = wrk.tile([HID, R], F32, tag="hs", name="hs", bufs=2)
            zs3 = wrk.tile([HID, R], F32, tag="zs", name="zs", bufs=2)
            STT(hs3[:], kh[1][:], -1.0, kh[0][:], op0=ALU.mult, op1=ALU.add)
            TT(hs3[:], hs3[:], kh[2][:], op=ALU.add)
            TT(hs3[:], hs3[:], h[:], op=ALU.add)
            STT(zs3[:], kz[1][:], -1.0, kz[0][:], op0=ALU.mult, op1=ALU.add)
            TT(zs3[:], zs3[:], kz[2][:], op=ALU.add)
            TT(zs3[:], zs3[:], z[:], op=ALU.add)
            vfield(4 * s + 3, hs3, zs3, kh[3], kz[3])
            hn = wrk.tile([HID, R], F32, tag="h", name="h", bufs=2)
            zn = wrk.tile([HID, R], F32, tag="z", name="z", bufs=2)
            TT(kh[1][:], kh[1][:], kh[2][:], op=ALU.add)
            STT(kh[1][:], kh[1][:], 3.0, kh[0][:], op0=ALU.mult, op1=ALU.add)
            TT(kh[1][:], kh[1][:], kh[3][:], op=ALU.add)
            STT(hn[:], kh[1][:], 0.125, h[:], op0=ALU.mult, op1=ALU.add)
            TT(kz[1][:], kz[1][:], kz[2][:], op=ALU.add)
            STT(kz[1][:], kz[1][:], 3.0, kz[0][:], op0=ALU.mult, op1=ALU.add)
            TT(kz[1][:], kz[1][:], kz[3][:], op=ALU.add)
            STT(zn[:], kz[1][:], 0.125, z[:], op0=ALU.mult, op1=ALU.add)
            h, z = hn, zn

        # ---- end conv ----
        op = pB.tile([OUT, R], F32, tag="acc", name="acc")
        mm2(op, cw, z)
        ob = wrk.tile([OUT, R], F32, tag="x2g", name="x2g")
        nc.vector.tensor_scalar_add(ob[:], op[:], cb[:])
        nc.sync.dma_start(out=d_out[:], in_=ob[:])

    _spill_excess_waits(nc)
    return nc


# ------------------------------------------------------------------
# host-side preprocessing
# ------------------------------------------------------------------
def host_inputs(a, nstep=NSTEP):
    gE = a["gE"]
    times = a["times"]
    assert np.allclose(np.diff(times), 1.0, atol=1e-5), "RK dt=1 baked in"
    maxlen = T - 2
    # unique stage times: u = 3s + st (stage 4 of step s == stage 0 of s+1)
    ts_list = [float(times[0])]
    for s in range(nstep):
        t0, t1 = float(times[s]), float(times[s + 1])
        dt = t1 - t0
        ts_list += [t0 + dt / 3.0, t0 + 2.0 * dt / 3.0, t1]
    nu = len(ts_list)
    dX = np.empty((B, N, nu, IN), np.float32)
    for q, t_ in enumerate(ts_list):
        idx = int(np.clip(np.sum(np.float32(t_) > times) - 1, 0, maxlen))
        frac = np.float32(t_ - times[idx])
        dX[:, :, q, :] = (a["coeff_b"][:, :, idx]
                          + (a["coeff_c2"][:, :, idx]
                             + a["coeff_d3"][:, :, idx] * frac) * frac)
    x0 = a["coeff_a"][:, :, 0, :]                   # (B, N, IN)

    fw3 = np.empty((HH, HID * IN), np.float32)
    fb3 = np.empty((HID * IN, 1), np.float32)
    for h_ in range(HID):
        for i in range(IN):
            fw3[:, i * HID + h_] = a["fWout"][:, h_ * IN + i]
            fb3[i * HID + h_, 0] = a["fbout"][h_ * IN + i]

    wpk = np.ascontiguousarray(
        np.transpose(a["gWpool"], (1, 2, 0, 3)).reshape(K, HH, EMB * HH))
    G = np.maximum(gE @ gE.T, 0.0).astype(np.float32)
    rs = np.exp(G).sum(axis=1)
    recip = np.ascontiguousarray((1.0 / rs).reshape(4, 128).T)

    vals = {
        "recip": recip, "gET": gE.T, "gbpool": a["gbpool"],
        "Wh": a["Wh"], "bh": a["bh"].reshape(-1, 1),
        "Wz": a["Wz"], "bz": a["bz"].reshape(-1, 1),
        "fw1": a["fWin"], "fb1": a["fbin"].reshape(-1, 1),
        "fw2": a["fWmid"], "fb2": a["fbmid"].reshape(-1, 1),
        "fw3": fw3, "fb3": fb3,
        "gw1": a["gWin"], "gb1": a["gbin"].reshape(-1, 1),
        "wpk0": wpk[0], "wpk1": wpk[1],
        "gwo": a["gWout"],
        "gbo": np.ascontiguousarray(a["gbout"].reshape(8, 128).T),
        "cw": np.ascontiguousarray(a["convW"].T),
        "cb": a["convb"].reshape(-1, 1),
    }
    wb = np.concatenate(
        [np.ascontiguousarray(vals[nm]).astype(np.float32).ravel()
         for nm, _ in _WSPEC]
        + [np.zeros(_WPAD - _WSIZE, np.float32)])
    assert wb.size == _WPAD
    wb_shards = wb.reshape(NCORES, _WSH)
    shared = {}

    per_core = []
    for c in range(NCORES):
        sl = slice(c * BS, (c + 1) * BS)
        dxc = np.transpose(dX[sl], (2, 3, 0, 1)).reshape(nu * IN, R)
        x0c = np.transpose(x0[sl], (2, 0, 1)).reshape(IN, R)
        per_core.append({
            "pc": np.ascontiguousarray(
                np.concatenate([dxc, x0c], axis=0)).astype(np.float32),
            "wb": wb_shards[c],
        })
    return shared, per_core


_STATE = {}


def _get_nc():
    if "nc" not in _STATE:
        _STATE["nc"] = build_nc()
    return _STATE["nc"]


def _get_runner():
    """Cached jit(shard_map(bass_exec)) callable — built once so per-call
    cost is dispatch only (run_bass_kernel_spmd re-traces every call)."""
    if "runner" in _STATE:
        return _STATE["runner"]
    import jax
    from jax.sharding import Mesh, PartitionSpec
    from jax.experimental.shard_map import shard_map
    from concourse import bass2jax as b2j

    b2j.install_neuronx_cc_hook()
    nc = _get_nc()
    assert nc.dbg_addr is None
    partition_name = (nc.partition_id_tensor.name
                      if nc.partition_id_tensor else None)
    in_names, out_names, out_avals, zero_outs = [], [], [], []
    for alloc in nc.m.functions[0].allocations:
        if not isinstance(alloc, mybir.MemoryLocationSet):
            continue
        name = alloc.memorylocations[0].name
        if alloc.kind == "ExternalInput":
            if name != partition_name:
                in_names.append(name)
        elif alloc.kind == "ExternalOutput":
            shape = tuple(alloc.tensor_shape)
            dtype = mybir.dt.np(alloc.dtype)
            out_names.append(name)
            out_avals.append(jax.core.ShapedArray(shape, dtype))
            zero_outs.append(np.zeros((NCORES * shape[0], *shape[1:]), dtype))
    n_params = len(in_names)
    all_names = list(in_names) + list(out_names)
    if partition_name is not None:
        all_names.append(partition_name)

    def _body(*args):
        operands = list(args)
        if partition_name is not None:
            operands.append(b2j.partition_id_tensor())
        outs = b2j._bass_exec_p.bind(
            *operands,
            out_avals=tuple(out_avals),
            in_names=tuple(all_names),
            out_names=tuple(out_names),
            lowering_input_output_aliases=(),
            sim_require_finite=True,
            sim_require_nnan=True,
            nc=nc,
        )
        return tuple(outs)

    devices = jax.devices()[:NCORES]
    mesh = Mesh(np.asarray(devices), ("core",))
    n_outs = len(out_names)
    sharded = jax.jit(
        shard_map(_body, mesh=mesh,
                  in_specs=(PartitionSpec("core"),) * (n_params + n_outs),
                  out_specs=(PartitionSpec("core"),) * n_outs,
                  check_rep=False),
        donate_argnums=tuple(range(n_params, n_params + n_outs)),
        keep_unused=True,
    )

    def run(in_maps):
        concat_in = [
            np.concatenate([np.asarray(in_maps[c][nm])
                            for c in range(NCORES)], axis=0)
            for nm in in_names
        ]
        out_arrs = sharded(*concat_in, *zero_outs)
        return [
            {nm: np.asarray(out_arrs[i]).reshape(
                NCORES, *out_avals[i].shape)[c]
             for i, nm in enumerate(out_names)}
            for c in range(NCORES)
        ]

    _STATE["runner"] = run
    return run


def _warm():
    """Trigger neuron compile + PJRT executable load with dummy inputs."""
    try:
        nc = _get_nc()
        a = {}
        a["times"] = np.arange(T, dtype=np.float32)
        for nm, sh in [("coeff_a", (B, N, T - 1, IN)),
                       ("coeff_b", (B, N, T - 1, IN)),
                       ("coeff_c2", (B, N, T - 1, IN)),
                       ("coeff_d3", (B, N, T - 1, IN)),
                       ("Wh", (IN, HID)), ("bh", (HID,)),
                       ("Wz", (IN, HID)), ("bz", (HID,)),
                       ("fWin", (HID, HH)), ("fbin", (HH,)),
                       ("fWmid", (HH, HH)), ("fbmid", (HH,)),
                       ("fWout", (HH, HID * IN)), ("fbout", (HID * IN,)),
                       ("gWin", (HID, HH)), ("gbin", (HH,)),
                       ("gE", (N, EMB)), ("gWpool", (EMB, K, HH, HH)),
                       ("gbpool", (EMB, HH)), ("gWout", (HH, HID * HID)),
                       ("gbout", (HID * HID,)), ("convW", (OUT, HID)),
                       ("convb", (OUT,))]:
            a[nm] = np.zeros(sh, np.float32)
        shared, per_core = host_inputs(a)
        in_maps = [{**shared, **pc} for pc in per_core]
        _get_runner()(in_maps)
        _STATE["warm"] = True
    except Exception as e:  # pragma: no cover - keep import usable
        import traceback
        traceback.print_exc()
        _STATE["warm_err"] = e


def kernel(**inputs):
    a = {k_: np.asarray(v, dtype=np.float32) for k_, v in inputs.items()}
    shared, per_core = host_inputs(a)
    in_maps = [{**shared, **pc} for pc in per_core]
    results = _get_runner()(in_maps)
    full = np.empty((B, 1, N, OUT), np.float32)
    for c in range(NCORES):
        o = np.asarray(results[c]["out"])               # (OUT, R)
        full[c * BS:(c + 1) * BS, 0] = (
            o.reshape(OUT, BS, N).transpose(1, 2, 0))
    return full


_warm()


# revision 10
# speedup vs baseline: 44.8867x; 1.1505x over previous
"""NeuralGCDE on 8 NeuronCores: full RK4 ODE integration on device.

Sharding: data-parallel over batch B=16 -> 2 batch elements per core
(rows r = b*N + n, R = 1024 per core). All graph/MLP params replicated.

Device layout is feature-major (features on SBUF partitions, rows on the
free dim). Every contraction is a PE matmul; partition-dim reductions and
broadcasts use structured 0/1 matrices as stationary operands. The
softmax adjacency (exp(relu(gE gE^T)) with row scaling) is built on
device; only the row-sum reciprocals (512 floats) come from host.

All one-time work (bass build, neuron compile, PJRT load) happens at
import; kernel(**inputs) does host repacks + one SPMD dispatch.
"""
import ml_dtypes
import numpy as np

import concourse.bass as bass
import concourse.mybir as mybir
import concourse.tile as tile
from concourse.bass_utils import run_bass_kernel_spmd

B, N, T = 16, 512, 12
IN, HID, HH, EMB, K, OUT = 2, 32, 32, 16, 2, 12
NCORES = 8
BS = B // NCORES            # 2
R = BS * N                  # 1024
NSTEP = T - 1               # 11
F32 = mybir.dt.float32
AF = mybir.ActivationFunctionType
ALU = mybir.AluOpType

# (name, shape) of every shared parameter, packed flat into one upload
_WSPEC = [
    ("recip", (128, 4)), ("gET", (EMB, N)), ("gbpool", (EMB, HH)),
    ("Wh", (IN, HID)), ("bh", (HID, 1)), ("Wz", (IN, HID)), ("bz", (HID, 1)),
    ("fw1", (HID, HH)), ("fb1", (HH, 1)), ("fw2", (HH, HH)), ("fb2", (HH, 1)),
    ("fw3", (HH, HID * IN)), ("fb3", (HID * IN, 1)),
    ("gw1", (HID, HH)), ("gb1", (HH, 1)),
    ("wpk0", (HH, EMB * HH)), ("wpk1", (HH, EMB * HH)),
    ("gwo", (HH, HID * HID)), ("gbo", (128, 8)),
    ("cw", (HID, OUT)), ("cb", (OUT, 1)),
]
_WSIZE = sum(int(np.prod(s)) for _, s in _WSPEC)
_WPAD = ((_WSIZE + 7) // 8) * 8
_WSH = _WPAD // 8

_NO_SPILL = {"InstEventSemaphore", "InstUnconditionalBranch",
             "InstConditionalBranch"}


def _spill_excess_waits(nc):
    """Walrus ISA structs hold one sync-wait slot on most instructions.
    Tile can emit several. Move excess waits onto InstEventSemaphore
    carriers inserted just before, on the same engine (waiting earlier on
    the same engine stream is always sound)."""
    nspill = 0
    for f in nc.m.functions:
        for blk in f.blocks:
            lst = blk.instructions
            i = 0
            while i < len(lst):
                ins = lst[i]
                si = ins.sync_info
                if (type(ins).__name__ in _NO_SPILL or si is None
                        or not si.on_wait or len(si.on_wait) <= 1):
                    i += 1
                    continue
                waits = list(si.on_wait)
                keep, excess = waits[-1:], waits[:-1]
                ins.sync_info = mybir.SyncInfo(on_wait=keep,
                                               on_update=list(si.on_update))
                carriers = []
                while excess:
                    chunk, excess = excess[:2], excess[2:]
                    es = mybir.InstEventSemaphore(
                        name=f"Wspill-{nspill}", ins=[], outs=[])
                    nspill += 1
                    es.engine = ins.engine
                    es.sync_info = mybir.SyncInfo(on_wait=chunk, on_update=[])
                    carriers.append(es)
                for k_, es in enumerate(carriers):
                    lst.insert(i + k_, es)
                i += len(carriers) + 1
    return nspill


def build_nc(nstep=NSTEP):
    nc = bass.Bass()

    def dp(name, shape, out=False):
        return nc.declare_dram_parameter(name, list(shape), F32, isOutput=out)

    NU = 3 * nstep + 1                     # unique dX stage rows
    BF16 = mybir.dt.bfloat16
    d_pc = nc.declare_dram_parameter("pc", [NU * IN, R], BF16, isOutput=False)
    d_x0 = dp("x0", (IN, R))
    d_wb = nc.declare_dram_parameter("wb", [_WSH], F32, isOutput=False)
    wb_in = nc.dram_tensor("wb_in", [_WSH], F32)
    wb_all = nc.dram_tensor("wb_all", [_WPAD], F32)
    d_out = dp("out", (OUT, R), out=True)

    C5 = 512  # fp32 moving-operand free-dim limit

    from contextlib import ExitStack
    with ExitStack() as es:
        tc = es.enter_context(tile.TileContext(nc))
        sgl = es.enter_context(tc.tile_pool(name="sgl", bufs=1))
        wrk = es.enter_context(tc.tile_pool(name="wrk", bufs=1))
        big1 = es.enter_context(tc.tile_pool(name="big1", bufs=1))
        big2 = es.enter_context(tc.tile_pool(name="big2", bufs=2))
        dxp = es.enter_context(tc.tile_pool(name="dxp", bufs=2))
        pA = es.enter_context(tc.tile_pool(name="pA", bufs=2, space="PSUM"))
        pB = es.enter_context(tc.tile_pool(name="pB", bufs=1, space="PSUM"))
        pT = es.enter_context(tc.tile_pool(name="pT", bufs=2, space="PSUM"))

        # gather the weight blob: each core uploaded 1/8th
        nc.sync.dma_start(out=wb_in[:], in_=d_wb[:])
        nc.gpsimd.collective_compute(
            "AllGather", ALU.bypass,
            replica_groups=[list(range(NCORES))],
            ins=[wb_in[:]], outs=[wb_all[:]])

        woff = [0]

        def load(name, shape):
            p_, f_ = shape
            t = sgl.tile([p_, f_], F32, tag=name, name=name)
            nc.sync.dma_start(
                out=t[:],
                in_=wb_all[woff[0]:woff[0] + p_ * f_].rearrange(
                    "(p f) -> p f", p=p_))
            woff[0] += p_ * f_
            return t

        W = {nm: load(nm, sh) for nm, sh in _WSPEC}
        (recip, gET, gbpool, Wh, bh, Wz, bz, fw1, fb1, fw2, fb2, fw3, fb3,
         gw1, gb1, wpk0, wpk1, gwo, gbo, cw, cb) = (
            W[nm] for nm, _ in _WSPEC)
        x0 = sgl.tile([IN, R], F32, tag="x0", name="x0")
        nc.sync.dma_start(out=x0[:], in_=d_x0[:])

        # ---- structured 0/1 matrices, built in place ----
        NE = ALU.not_equal

        def zeros_tile(name, shape):
            t = sgl.tile(list(shape), F32, tag=name, name=name)
            nc.gpsimd.memset(t[:], 0.0)
            return t

        def aff(t, ap, pattern, base=0, cm=0):
            nc.gpsimd.affine_select(out=ap, in_=ap, compare_op=NE, fill=1.0,
                                    base=base, pattern=pattern,
                                    channel_multiplier=cm)

        ident = zeros_tile("ident", (128, 128))
        aff(ident, ident[:], [[-1, 128]], cm=1)
        I32 = zeros_tile("I32", (HH, HH))
        aff(I32, I32[:], [[-1, HH]], cm=1)
        Bc = sgl.tile([IN, IN * HID], BF16, tag="Bc", name="Bc")
        nc.gpsimd.memset(Bc[:], 0.0)
        aff(Bc, Bc[:].rearrange("p (j y) -> p j y", y=HID), [[-1, IN], [0, HID]],
            cm=1)
        Erep = zeros_tile("Erep", (HID, 128))      # 1 iff col%32 == p
        aff(Erep, Erep[:].rearrange("p (j y) -> p j y", y=HID),
            [[0, 4], [-1, HID]], cm=1)
        S3 = zeros_tile("S3", (IN * HID, HID))     # 1 iff p%32 == col
        aff(S3, S3[:], [[-1, HID]], cm=1)
        aff(S3, S3[:], [[-1, HID]], base=-HID, cm=1)
        S2 = zeros_tile("S2", (128, HH))           # 1 iff p%32 == col
        for q in range(4):
            aff(S2, S2[:], [[-1, HH]], base=-q * HH, cm=1)
        Gsel = zeros_tile("Gsel", (EMB, 4 * 128))  # 1 iff col//32 == p
        aff(Gsel, Gsel[:].rearrange("p (j y) -> p j y", y=32),
            [[-1, EMB], [0, 32]], cm=1)
        # Sdz[p, j*32+y] = 1 iff y == 4j + p//32, composed as E4.T @ Cdz
        E4 = zeros_tile("E4", (4, 128))            # 1 iff col//32 == p
        aff(E4, E4[:].rearrange("p (j y) -> p j y", y=32), [[-1, 4], [0, 32]],
            cm=1)
        Cdz = zeros_tile("Cdz", (4, 8 * HID))      # 1 iff y == 4j + p
        aff(Cdz, Cdz[:].rearrange("p (j y) -> p j y", y=HID),
            [[4, 8], [-1, HID]], cm=1)
        sdzp = pA.tile([128, 8 * HID], F32, tag="mm", name="mm")
        nc.tensor.matmul(sdzp[:], E4[:], Cdz[:], start=True, stop=True)
        Sdz = sgl.tile([128, 8 * HID], F32, tag="Sdz", name="Sdz")
        nc.scalar.copy(Sdz[:], sdzp[:])

        # ---- abT[o, n] = (gE @ gbpool).T, used for both batch halves ----
        abp = pA.tile([HH, N], F32, tag="mm", name="mm")
        nc.tensor.matmul(abp[:], gbpool[:], gET[:], start=True, stop=True)
        abT = sgl.tile([HH, N], F32, tag="abT", name="abT")
        nc.scalar.copy(abT[:], abp[:])

        def mm2(ps, lhsT, rhs, start=True, stop=True):
            for c in range(2):
                nc.tensor.matmul(ps[:, c * C5:(c + 1) * C5], lhsT,
                                 rhs[:, c * C5:(c + 1) * C5],
                                 start=start, stop=stop)

        def act(out, in_, func, bias=0.0):
            nc.scalar.activation(out, in_, func, bias=bias)

        # ---- adjacency: expG chunks (exp(relu(gE gE^T)), m-major) ----
        expG = []
        for i in range(4):
            gp = pA.tile([128, N], F32, tag="mm", name="mm")
            nc.tensor.matmul(gp[:], gET[:, i * 128:(i + 1) * 128], gET[:],
                             start=True, stop=True)
            eg = sgl.tile([128, N], F32, tag=f"expG{i}", name=f"expG{i}")
            act(eg[:], gp[:], AF.Relu)
            act(eg[:], eg[:], AF.Exp)
            expG.append(eg)

        # ---- gE_part chunks: gEp_j[p, r] = gE[n(r), (j*128+p)//32] ----
        gEp = []
        for j in range(4):
            ps = pA.tile([128, R], F32, tag="mm", name="mm")
            for c in range(2):
                nc.tensor.matmul(ps[:, c * C5:(c + 1) * C5],
                                 Gsel[:, j * 128:(j + 1) * 128], gET[:],
                                 start=True, stop=True)
            g = sgl.tile([128, R], F32, tag=f"gEp{j}", name=f"gEp{j}")
            nc.scalar.copy(g[:], ps[:])
            gEp.append(g)

        # ---- state: h0 = x0 @ Wh + bh, z0 = x0 @ Wz + bz ----
        h = wrk.tile([HID, R], F32, tag="h", name="h", bufs=2)
        z = wrk.tile([HID, R], F32, tag="z", name="z", bufs=2)
        h0p = pA.tile([HID, R], F32, tag="mm", name="mm")
        mm2(h0p, Wh, x0)
        nc.vector.tensor_scalar_add(h[:], h0p[:], bh[:])
        z0p = pA.tile([HID, R], F32, tag="mm", name="mm")
        mm2(z0p, Wz, x0)
        nc.vector.tensor_scalar_add(z[:], z0p[:], bz[:])

        def vfield(s4, hs, zs, kh, kz):
            u = 3 * (s4 // 4) + (s4 % 4)
            dxs = dxp.tile([IN, R], BF16, tag="dxs", name="dxs")
            nc.sync.dma_start(out=dxs[:], in_=d_pc[2 * u:2 * u + 2, :])
            # f path: two relu MLP layers + tanh head (i-major columns)
            x1p = pA.tile([HH, R], F32, tag="mm", name="mm")
            mm2(x1p, fw1, hs)
            x1 = wrk.tile([HH, R], F32, tag="fx", name="fx", bufs=2)
            act(x1[:], x1p[:], AF.Relu, bias=fb1[:])
            x2p = pA.tile([HH, R], F32, tag="mm", name="mm")
            mm2(x2p, fw2, x1)
            x2 = wrk.tile([HH, R], F32, tag="fx", name="fx", bufs=2)
            act(x2[:], x2p[:], AF.Relu, bias=fb2[:])
            vfp = pA.tile([HID * IN, R], F32, tag="mm", name="mm")
            mm2(vfp, fw3, x2)
            vf = wrk.tile([HID * IN, R], F32, tag="vf", name="vf")
            act(vf[:], vfp[:], AF.Tanh, bias=fb3[:])
            # dh = sum_i vf_i * dX_i  (dX broadcast via Bc, reduce via S3)
            dXb = pA.tile([IN * HID, R], F32, tag="mm", name="mm")
            mm2(dXb, Bc, dxs)
            nc.vector.tensor_mul(vf[:], vf[:], dXb[:])
            dhp = pB.tile([HID, R], F32, tag="acc", name="acc")
            mm2(dhp, S3, vf)
            nc.scalar.copy(kh[:], dhp[:])
            drp = pA.tile([128, R], F32, tag="mm", name="mm")
            mm2(drp, Erep, kh)
            dhrep = big1.tile([128, R], F32, tag="dhrep", name="dhrep")
            nc.scalar.copy(dhrep[:], drp[:])
            # g path: relu layer (feature-major), node-major transposes
            x1gp = pA.tile([HH, R], F32, tag="mm", name="mm")
            mm2(x1gp, gw1, zs)
            x1g = wrk.tile([HH, R], F32, tag="x1g", name="x1g")
            act(x1g[:], x1gp[:], AF.Relu, bias=gb1[:])
            xT = []
            for k_ in range(4):
                xtp = pT.tile([128, 2 * HH], F32, tag="pt", name="pt")
                for b_ in range(2):
                    nc.tensor.transpose(
                        xtp[:, b_ * HH:(b_ + 1) * HH],
                        x1g[:, b_ * N + k_ * 128: b_ * N + (k_ + 1) * 128],
                        ident[:HH, :HH])
                xt = wrk.tile([128, 2 * HH], F32, tag=f"xT{k_}",
                              name=f"xT{k_}")
                nc.vector.tensor_copy(xt[:], xtp[:])
                xT.append(xt)
            # graph conv: xg1 = A @ x1g per batch, recip folded in
            xg1n = []
            for i in range(4):
                xgp = pT.tile([128, 2 * HH], F32, tag="pt", name="pt")
                for k_ in range(4):
                    nc.tensor.matmul(xgp[:],
                                     expG[k_][:, i * 128:(i + 1) * 128],
                                     xT[k_][:],
                                     start=(k_ == 0), stop=(k_ == 3))
                xn = wrk.tile([128, 2 * HH], F32, tag=f"xg1n{i}",
                              name=f"xg1n{i}")
                nc.vector.tensor_scalar_mul(xn[:], xgp[:], recip[:, i:i + 1])
                xg1n.append(xn)
            xg1f = wrk.tile([HH, R], F32, tag="xg1f", name="xg1f")
            for i in range(4):
                for b_ in range(2):
                    btp = pT.tile([HH, 128], F32, tag="pt", name="pt")
                    nc.tensor.transpose(btp[:],
                                        xg1n[i][:, b_ * HH:(b_ + 1) * HH],
                                        ident[:, :])
                    nc.scalar.copy(
                        xg1f[:, b_ * N + i * 128: b_ * N + (i + 1) * 128],
                        btp[:])
            # per-node pooled weights: y = Wp^T xg scaled by gE_part,
            # reduced over EMB via S2 into x2g (abf preloaded via I32)
            x2gp = pB.tile([HH, R], F32, tag="acc", name="acc")
            for c in range(2):
                nc.tensor.matmul(x2gp[:, c * C5:(c + 1) * C5], I32[:],
                                 abT[:],
                                 start=True, stop=False, skip_group_check=True)
            for j in range(4):
                yp = pA.tile([128, R], F32, tag="mm", name="mm")
                for c in range(2):
                    sl = slice(c * C5, (c + 1) * C5)
                    nc.tensor.matmul(yp[:, sl], wpk0[:, j * 128:(j + 1) * 128],
                                     x1g[:, sl], start=True, stop=False)
                    nc.tensor.matmul(yp[:, sl], wpk1[:, j * 128:(j + 1) * 128],
                                     xg1f[:, sl], start=False, stop=True)
                t_ = big1.tile([128, R], F32, tag="ty", name="ty", bufs=2)
                nc.vector.tensor_mul(t_[:], yp[:], gEp[j][:])
                for c in range(2):
                    sl = slice(c * C5, (c + 1) * C5)
                    nc.tensor.matmul(x2gp[:, sl], S2[:], t_[:, sl],
                                     start=False, stop=(j == 3),
                                     skip_group_check=True)
            x2g = wrk.tile([HH, R], F32, tag="x2g", name="x2g")
            nc.scalar.copy(x2g[:], x2gp[:])
            # vg chunks; dz = sum vg_ho * dh_o accumulated via Sdz
            dzp = pB.tile([HID, R], F32, tag="acc", name="acc")
            for j in range(8):
                vgp = pA.tile([128, R], F32, tag="mm", name="mm")
                mm2(vgp, gwo[:, j * 128:(j + 1) * 128], x2g)
                vg = big2.tile([128, R], F32, tag="vg", name="vg")
                act(vg[:], vgp[:], AF.Tanh, bias=gbo[:, j:j + 1])
                nc.vector.tensor_mul(vg[:], vg[:], dhrep[:])
                for c in range(2):
                    sl = slice(c * C5, (c + 1) * C5)
                    nc.tensor.matmul(dzp[:, sl],
                                     Sdz[:, j * HID:(j + 1) * HID],
                                     vg[:, sl],
                                     start=(j == 0), stop=(j == 7),
                                     skip_group_check=True)
            nc.scalar.copy(kz[:], dzp[:])

        TT = nc.vector.tensor_tensor
        STT = nc.vector.scalar_tensor_tensor

        # RK4 with 3/8 rule, dt = 1 (times are arange; asserted on host)
        for s in range(nstep):
            kh = [wrk.tile([HID, R], F32, tag=f"kh{st}", name=f"kh{st}")
                  for st in range(4)]
            kz = [wrk.tile([HID, R], F32, tag=f"kz{st}", name=f"kz{st}")
                  for st in range(4)]
            vfield(4 * s + 0, h, z, kh[0], kz[0])
            hs = wrk.tile([HID, R], F32, tag="hs", name="hs", bufs=2)
            zs = wrk.tile([HID, R], F32, tag="zs", name="zs", bufs=2)
            STT(hs[:], kh[0][:], 1.0 / 3.0, h[:], op0=ALU.mult, op1=ALU.add)
            STT(zs[:], kz[0][:], 1.0 / 3.0, z[:], op0=ALU.mult, op1=ALU.add)
            vfield(4 * s + 1, hs, zs, kh[1], kz[1])
            hs2 = wrk.tile([HID, R], F32, tag="hs", name="hs", bufs=2)
            zs2 = wrk.tile([HID, R], F32, tag="zs", name="zs", bufs=2)
            STT(hs2[:], kh[0][:], -1.0 / 3.0, kh[1][:],
                op0=ALU.mult, op1=ALU.add)
            TT(hs2[:], hs2[:], h[:], op=ALU.add)
            STT(zs2[:], kz[0][:], -1.0 / 3.0, kz[1][:],
                op0=ALU.mult, op1=ALU.add)
            TT(zs2[:], zs2[:], z[:], op=ALU.add)
            vfield(4 * s + 2, hs2, zs2, kh[2], kz[2])
            hs3 = wrk.tile([HID, R], F32, tag="hs", name="hs", bufs=2)
            zs3 = wrk.tile([HID, R], F32, tag="zs", name="zs", bufs=2)
            STT(hs3[:], kh[1][:], -1.0, kh[0][:], op0=ALU.mult, op1=ALU.add)
            TT(hs3[:], hs3[:], kh[2][:], op=ALU.add)
            TT(hs3[:], hs3[:], h[:], op=ALU.add)
            STT(zs3[:], kz[1][:], -1.0, kz[0][:], op0=ALU.mult, op1=ALU.add)
            TT(zs3[:], zs3[:], kz[2][:], op=ALU.add)
            TT(zs3[:], zs3[:], z[:], op=ALU.add)
            vfield(4 * s + 3, hs3, zs3, kh[3], kz[3])
            hn = wrk.tile([HID, R], F32, tag="h", name="h", bufs=2)
            zn = wrk.tile([HID, R], F32, tag="z", name="z", bufs=2)
            TT(kh[1][:], kh[1][:], kh[2][:], op=ALU.add)
            STT(kh[1][:], kh[1][:], 3.0, kh[0][:], op0=ALU.mult, op1=ALU.add)
            TT(kh[1][:], kh[1][:], kh[3][:], op=ALU.add)
            STT(hn[:], kh[1][:], 0.125, h[:], op0=ALU.mult, op1=ALU.add)
            TT(kz[1][:], kz[1][:], kz[2][:], op=ALU.add)
            STT(kz[1][:], kz[1][:], 3.0, kz[0][:], op0=ALU.mult, op1=ALU.add)
            TT(kz[1][:], kz[1][:], kz[3][:], op=ALU.add)
            STT(zn[:], kz[1][:], 0.125, z[:], op0=ALU.mult, op1=ALU.add)
            h, z = hn, zn

        # ---- end conv ----
        op = pB.tile([OUT, R], F32, tag="acc", name="acc")
        mm2(op, cw, z)
        ob = wrk.tile([OUT, R], F32, tag="x2g", name="x2g")
        nc.vector.tensor_scalar_add(ob[:], op[:], cb[:])
        nc.sync.dma_start(out=d_out[:], in_=ob[:])

    _spill_excess_waits(nc)
    return nc


# ------------------------------------------------------------------
# host-side preprocessing
# ------------------------------------------------------------------
def host_inputs(a, nstep=NSTEP):
    gE = a["gE"]
    times = a["times"]
    assert np.allclose(np.diff(times), 1.0, atol=1e-5), "RK dt=1 baked in"
    maxlen = T - 2
    # unique stage times: u = 3s + st (stage 4 of step s == stage 0 of s+1)
    ts_list = [float(times[0])]
    for s in range(nstep):
        t0, t1 = float(times[s]), float(times[s + 1])
        dt = t1 - t0
        ts_list += [t0 + dt / 3.0, t0 + 2.0 * dt / 3.0, t1]
    nu = len(ts_list)
    dX = np.empty((B, N, nu, IN), np.float32)
    for q, t_ in enumerate(ts_list):
        idx = int(np.clip(np.sum(np.float32(t_) > times) - 1, 0, maxlen))
        frac = np.float32(t_ - times[idx])
        dX[:, :, q, :] = (a["coeff_b"][:, :, idx]
                          + (a["coeff_c2"][:, :, idx]
                             + a["coeff_d3"][:, :, idx] * frac) * frac)
    x0 = a["coeff_a"][:, :, 0, :]                   # (B, N, IN)

    fw3 = np.empty((HH, HID * IN), np.float32)
    fb3 = np.empty((HID * IN, 1), np.float32)
    for h_ in range(HID):
        for i in range(IN):
            fw3[:, i * HID + h_] = a["fWout"][:, h_ * IN + i]
            fb3[i * HID + h_, 0] = a["fbout"][h_ * IN + i]

    wpk = np.ascontiguousarray(
        np.transpose(a["gWpool"], (1, 2, 0, 3)).reshape(K, HH, EMB * HH))
    G = np.maximum(gE @ gE.T, 0.0).astype(np.float32)
    rs = np.exp(G).sum(axis=1)
    recip = np.ascontiguousarray((1.0 / rs).reshape(4, 128).T)

    vals = {
        "recip": recip, "gET": gE.T, "gbpool": a["gbpool"],
        "Wh": a["Wh"], "bh": a["bh"].reshape(-1, 1),
        "Wz": a["Wz"], "bz": a["bz"].reshape(-1, 1),
        "fw1": a["fWin"], "fb1": a["fbin"].reshape(-1, 1),
        "fw2": a["fWmid"], "fb2": a["fbmid"].reshape(-1, 1),
        "fw3": fw3, "fb3": fb3,
        "gw1": a["gWin"], "gb1": a["gbin"].reshape(-1, 1),
        "wpk0": wpk[0], "wpk1": wpk[1],
        "gwo": a["gWout"],
        "gbo": np.ascontiguousarray(a["gbout"].reshape(8, 128).T),
        "cw": np.ascontiguousarray(a["convW"].T),
        "cb": a["convb"].reshape(-1, 1),
    }
    wb = np.concatenate(
        [np.ascontiguousarray(vals[nm]).astype(np.float32).ravel()
         for nm, _ in _WSPEC]
        + [np.zeros(_WPAD - _WSIZE, np.float32)])
    assert wb.size == _WPAD
    wb_shards = wb.reshape(NCORES, _WSH)
    shared = {}

    per_core = []
    for c in range(NCORES):
        sl = slice(c * BS, (c + 1) * BS)
        dxc = np.transpose(dX[sl], (2, 3, 0, 1)).reshape(nu * IN, R)
        x0c = np.transpose(x0[sl], (2, 0, 1)).reshape(IN, R)
        per_core.append({
            "pc": np.ascontiguousarray(dxc).astype(ml_dtypes.bfloat16),
            "x0": np.ascontiguousarray(x0c).astype(np.float32),
            "wb": wb_shards[c],
        })
    return shared, per_core


_STATE = {}


def _get_nc():
    if "nc" not in _STATE:
        _STATE["nc"] = build_nc()
    return _STATE["nc"]


def _get_runner():
    """Cached jit(shard_map(bass_exec)) callable — built once so per-call
    cost is dispatch only (run_bass_kernel_spmd re-traces every call)."""
    if "runner" in _STATE:
        return _STATE["runner"]
    import jax
    from jax.sharding import Mesh, PartitionSpec
    from jax.experimental.shard_map import shard_map
    from concourse import bass2jax as b2j

    b2j.install_neuronx_cc_hook()
    nc = _get_nc()
    assert nc.dbg_addr is None
    partition_name = (nc.partition_id_tensor.name
                      if nc.partition_id_tensor else None)
    in_names, out_names, out_avals, zero_outs = [], [], [], []
    for alloc in nc.m.functions[0].allocations:
        if not isinstance(alloc, mybir.MemoryLocationSet):
            continue
        name = alloc.memorylocations[0].name
        if alloc.kind == "ExternalInput":
            if name != partition_name:
                in_names.append(name)
        elif alloc.kind == "ExternalOutput":
            shape = tuple(alloc.tensor_shape)
            dtype = mybir.dt.np(alloc.dtype)
            out_names.append(name)
            out_avals.append(jax.core.ShapedArray(shape, dtype))
            zero_outs.append(np.zeros((NCORES * shape[0], *shape[1:]), dtype))
    n_params = len(in_names)
    all_names = list(in_names) + list(out_names)
    if partition_name is not None:
        all_names.append(partition_name)

    def _body(*args):
        operands = list(args)
        if partition_name is not None:
            operands.append(b2j.partition_id_tensor())
        outs = b2j._bass_exec_p.bind(
            *operands,
            out_avals=tuple(out_avals),
            in_names=tuple(all_names),
            out_names=tuple(out_names),
            lowering_input_output_aliases=(),
            sim_require_finite=True,
            sim_require_nnan=True,
            nc=nc,
        )
        return tuple(outs)

    devices = jax.devices()[:NCORES]
    mesh = Mesh(np.asarray(devices), ("core",))
    n_outs = len(out_names)
    sharded = jax.jit(
        shard_map(_body, mesh=mesh,
                  in_specs=(PartitionSpec("core"),) * (n_params + n_outs),
                  out_specs=(PartitionSpec("core"),) * n_outs,
                  check_rep=False),
        donate_argnums=tuple(range(n_params, n_params + n_outs)),
        keep_unused=True,
    )

    def run(in_maps):
        concat_in = [
            np.concatenate([np.asarray(in_maps[c][nm])
                            for c in range(NCORES)], axis=0)
            for nm in in_names
        ]
        out_arrs = sharded(*concat_in, *zero_outs)
        return [
            {nm: np.asarray(out_arrs[i]).reshape(
                NCORES, *out_avals[i].shape)[c]
             for i, nm in enumerate(out_names)}
            for c in range(NCORES)
        ]

    _STATE["runner"] = run
    return run


def _warm():
    """Trigger neuron compile + PJRT executable load with dummy inputs."""
    try:
        nc = _get_nc()
        a = {}
        a["times"] = np.arange(T, dtype=np.float32)
        for nm, sh in [("coeff_a", (B, N, T - 1, IN)),
                       ("coeff_b", (B, N, T - 1, IN)),
                       ("coeff_c2", (B, N, T - 1, IN)),
                       ("coeff_d3", (B, N, T - 1, IN)),
                       ("Wh", (IN, HID)), ("bh", (HID,)),
                       ("Wz", (IN, HID)), ("bz", (HID,)),
                       ("fWin", (HID, HH)), ("fbin", (HH,)),
                       ("fWmid", (HH, HH)), ("fbmid", (HH,)),
                       ("fWout", (HH, HID * IN)), ("fbout", (HID * IN,)),
                       ("gWin", (HID, HH)), ("gbin", (HH,)),
                       ("gE", (N, EMB)), ("gWpool", (EMB, K, HH, HH)),
                       ("gbpool", (EMB, HH)), ("gWout", (HH, HID * HID)),
                       ("gbout", (HID * HID,)), ("convW", (OUT, HID)),
                       ("convb", (OUT,))]:
            a[nm] = np.zeros(sh, np.float32)
        shared, per_core = host_inputs(a)
        in_maps = [{**shared, **pc} for pc in per_core]
        _get_runner()(in_maps)
        _STATE["warm"] = True
    except Exception as e:  # pragma: no cover - keep import usable
        import traceback
        traceback.print_exc()
        _STATE["warm_err"] = e


def kernel(**inputs):
    a = {k_: np.asarray(v, dtype=np.float32) for k_, v in inputs.items()}
    shared, per_core = host_inputs(a)
    in_maps = [{**shared, **pc} for pc in per_core]
    results = _get_runner()(in_maps)
    full = np.empty((B, 1, N, OUT), np.float32)
    for c in range(NCORES):
        o = np.asarray(results[c]["out"])               # (OUT, R)
        full[c * BS:(c + 1) * BS, 0] = (
            o.reshape(OUT, BS, N).transpose(1, 2, 0))
    return full


_warm()


# revision 12
# speedup vs baseline: 54.5970x; 1.2163x over previous
"""NeuralGCDE on 8 NeuronCores: full RK4 ODE integration on device.

Sharding: data-parallel over batch B=16 -> 2 batch elements per core
(rows r = b*N + n, R = 1024 per core). All graph/MLP params replicated.

Device layout is feature-major (features on SBUF partitions, rows on the
free dim). Every contraction is a PE matmul; partition-dim reductions and
broadcasts use structured 0/1 matrices as stationary operands. The
softmax adjacency (exp(relu(gE gE^T)) with row scaling) is built on
device; only the row-sum reciprocals (512 floats) come from host.

All one-time work (bass build, neuron compile, PJRT load) happens at
import; kernel(**inputs) does host repacks + one SPMD dispatch.
"""
import ml_dtypes
import numpy as np

import concourse.bass as bass
import concourse.mybir as mybir
import concourse.tile as tile
from concourse.bass_utils import run_bass_kernel_spmd

B, N, T = 16, 512, 12
IN, HID, HH, EMB, K, OUT = 2, 32, 32, 16, 2, 12
NCORES = 8
BS = B // NCORES            # 2
R = BS * N                  # 1024
NSTEP = T - 1               # 11
F32 = mybir.dt.float32
AF = mybir.ActivationFunctionType
ALU = mybir.AluOpType

# (name, shape) of every shared parameter, packed flat into one upload
_WSPEC = [
    ("recip", (128, 4)), ("gET", (EMB, N)), ("gbpool", (EMB, HH)),
    ("Wh", (IN, HID)), ("bh", (HID, 1)), ("Wz", (IN, HID)), ("bz", (HID, 1)),
    ("fw1", (HID, HH)), ("fb1", (HH, 1)), ("fw2", (HH, HH)), ("fb2", (HH, 1)),
    ("fw3", (HH, HID * IN)), ("fb3", (HID * IN, 1)),
    ("gw1", (HID, HH)), ("gb1", (HH, 1)),
    ("wpk0", (HH, EMB * HH)), ("wpk1", (HH, EMB * HH)),
    ("gwo", (HH, HID * HID)), ("gbo", (128, 8)),
    ("cw", (HID, OUT)), ("cb", (OUT, 1)),
]
_WSIZE = sum(int(np.prod(s)) for _, s in _WSPEC)
_WPAD = ((_WSIZE + 7) // 8) * 8
_WSH = _WPAD // 8

_NO_SPILL = {"InstEventSemaphore", "InstUnconditionalBranch",
             "InstConditionalBranch"}


def _spill_excess_waits(nc):
    """Walrus ISA structs hold one sync-wait slot on most instructions.
    Tile can emit several. Move excess waits onto InstEventSemaphore
    carriers inserted just before, on the same engine (waiting earlier on
    the same engine stream is always sound)."""
    nspill = 0
    for f in nc.m.functions:
        for blk in f.blocks:
            lst = blk.instructions
            i = 0
            while i < len(lst):
                ins = lst[i]
                si = ins.sync_info
                if (type(ins).__name__ in _NO_SPILL or si is None
                        or not si.on_wait or len(si.on_wait) <= 1):
                    i += 1
                    continue
                waits = list(si.on_wait)
                keep, excess = waits[-1:], waits[:-1]
                ins.sync_info = mybir.SyncInfo(on_wait=keep,
                                               on_update=list(si.on_update))
                carriers = []
                while excess:
                    chunk, excess = excess[:2], excess[2:]
                    es = mybir.InstEventSemaphore(
                        name=f"Wspill-{nspill}", ins=[], outs=[])
                    nspill += 1
                    es.engine = ins.engine
                    es.sync_info = mybir.SyncInfo(on_wait=chunk, on_update=[])
                    carriers.append(es)
                for k_, es in enumerate(carriers):
                    lst.insert(i + k_, es)
                i += len(carriers) + 1
    return nspill


def build_nc(nstep=NSTEP):
    nc = bass.Bass()

    def dp(name, shape, out=False):
        return nc.declare_dram_parameter(name, list(shape), F32, isOutput=out)

    NU = 3 * nstep + 1                     # unique dX stage rows
    BF16 = mybir.dt.bfloat16
    d_pc = nc.declare_dram_parameter("pc", [NU * IN, R], BF16, isOutput=False)
    d_x0 = dp("x0", (IN, R))
    d_wb = nc.declare_dram_parameter("wb", [_WSH], F32, isOutput=False)
    wb_in = nc.dram_tensor("wb_in", [_WSH], F32)
    wb_all = nc.dram_tensor("wb_all", [_WPAD], F32)
    d_out = nc.declare_dram_parameter("out", [OUT, R], mybir.dt.bfloat16,
                                      isOutput=True)

    C5 = 512  # fp32 moving-operand free-dim limit

    from contextlib import ExitStack
    with ExitStack() as es:
        tc = es.enter_context(tile.TileContext(nc))
        sgl = es.enter_context(tc.tile_pool(name="sgl", bufs=1))
        wrk = es.enter_context(tc.tile_pool(name="wrk", bufs=1))
        big1 = es.enter_context(tc.tile_pool(name="big1", bufs=1))
        big2 = es.enter_context(tc.tile_pool(name="big2", bufs=2))
        dxp = es.enter_context(tc.tile_pool(name="dxp", bufs=2))
        pA = es.enter_context(tc.tile_pool(name="pA", bufs=2, space="PSUM"))
        pB = es.enter_context(tc.tile_pool(name="pB", bufs=1, space="PSUM"))
        pT = es.enter_context(tc.tile_pool(name="pT", bufs=2, space="PSUM"))

        # gather the weight blob: each core uploaded 1/8th
        nc.sync.dma_start(out=wb_in[:], in_=d_wb[:])
        nc.gpsimd.collective_compute(
            "AllGather", ALU.bypass,
            replica_groups=[list(range(NCORES))],
            ins=[wb_in[:]], outs=[wb_all[:]])

        woff = [0]

        def load(name, shape):
            p_, f_ = shape
            t = sgl.tile([p_, f_], F32, tag=name, name=name)
            nc.sync.dma_start(
                out=t[:],
                in_=wb_all[woff[0]:woff[0] + p_ * f_].rearrange(
                    "(p f) -> p f", p=p_))
            woff[0] += p_ * f_
            return t

        W = {nm: load(nm, sh) for nm, sh in _WSPEC}
        (recip, gET, gbpool, Wh, bh, Wz, bz, fw1, fb1, fw2, fb2, fw3, fb3,
         gw1, gb1, wpk0, wpk1, gwo, gbo, cw, cb) = (
            W[nm] for nm, _ in _WSPEC)
        x0 = sgl.tile([IN, R], F32, tag="x0", name="x0")
        nc.sync.dma_start(out=x0[:], in_=d_x0[:])

        # ---- structured 0/1 matrices, built in place ----
        NE = ALU.not_equal

        def zeros_tile(name, shape):
            t = sgl.tile(list(shape), F32, tag=name, name=name)
            nc.gpsimd.memset(t[:], 0.0)
            return t

        def aff(t, ap, pattern, base=0, cm=0):
            nc.gpsimd.affine_select(out=ap, in_=ap, compare_op=NE, fill=1.0,
                                    base=base, pattern=pattern,
                                    channel_multiplier=cm)

        ident = zeros_tile("ident", (128, 128))
        aff(ident, ident[:], [[-1, 128]], cm=1)
        I32 = zeros_tile("I32", (HH, HH))
        aff(I32, I32[:], [[-1, HH]], cm=1)
        Bc = sgl.tile([IN, IN * HID], BF16, tag="Bc", name="Bc")
        nc.gpsimd.memset(Bc[:], 0.0)
        aff(Bc, Bc[:].rearrange("p (j y) -> p j y", y=HID), [[-1, IN], [0, HID]],
            cm=1)
        Erep = zeros_tile("Erep", (HID, 128))      # 1 iff col%32 == p
        aff(Erep, Erep[:].rearrange("p (j y) -> p j y", y=HID),
            [[0, 4], [-1, HID]], cm=1)
        S3 = zeros_tile("S3", (IN * HID, HID))     # 1 iff p%32 == col
        aff(S3, S3[:], [[-1, HID]], cm=1)
        aff(S3, S3[:], [[-1, HID]], base=-HID, cm=1)
        S2 = zeros_tile("S2", (128, HH))           # 1 iff p%32 == col
        for q in range(4):
            aff(S2, S2[:], [[-1, HH]], base=-q * HH, cm=1)
        Gsel = zeros_tile("Gsel", (EMB, 4 * 128))  # 1 iff col//32 == p
        aff(Gsel, Gsel[:].rearrange("p (j y) -> p j y", y=32),
            [[-1, EMB], [0, 32]], cm=1)
        # Sdz[p, j*32+y] = 1 iff y == 4j + p//32, composed as E4.T @ Cdz
        E4 = zeros_tile("E4", (4, 128))            # 1 iff col//32 == p
        aff(E4, E4[:].rearrange("p (j y) -> p j y", y=32), [[-1, 4], [0, 32]],
            cm=1)
        Cdz = zeros_tile("Cdz", (4, 8 * HID))      # 1 iff y == 4j + p
        aff(Cdz, Cdz[:].rearrange("p (j y) -> p j y", y=HID),
            [[4, 8], [-1, HID]], cm=1)
        sdzp = pA.tile([128, 8 * HID], F32, tag="mm", name="mm")
        nc.tensor.matmul(sdzp[:], E4[:], Cdz[:], start=True, stop=True)
        Sdz = sgl.tile([128, 8 * HID], F32, tag="Sdz", name="Sdz")
        nc.scalar.copy(Sdz[:], sdzp[:])

        # ---- abT[o, n] = (gE @ gbpool).T, used for both batch halves ----
        abp = pA.tile([HH, N], F32, tag="mm", name="mm")
        nc.tensor.matmul(abp[:], gbpool[:], gET[:], start=True, stop=True)
        abT = sgl.tile([HH, N], F32, tag="abT", name="abT")
        nc.scalar.copy(abT[:], abp[:])

        def mm2(ps, lhsT, rhs, start=True, stop=True):
            for c in range(2):
                nc.tensor.matmul(ps[:, c * C5:(c + 1) * C5], lhsT,
                                 rhs[:, c * C5:(c + 1) * C5],
                                 start=start, stop=stop)

        def act(out, in_, func, bias=0.0):
            nc.scalar.activation(out, in_, func, bias=bias)

        # ---- adjacency: expG chunks (exp(relu(gE gE^T)), m-major) ----
        expG = []
        for i in range(4):
            gp = pA.tile([128, N], F32, tag="mm", name="mm")
            nc.tensor.matmul(gp[:], gET[:, i * 128:(i + 1) * 128], gET[:],
                             start=True, stop=True)
            eg = sgl.tile([128, N], F32, tag=f"expG{i}", name=f"expG{i}")
            act(eg[:], gp[:], AF.Relu)
            act(eg[:], eg[:], AF.Exp)
            expG.append(eg)

        # ---- gE_part chunks: gEp_j[p, r] = gE[n(r), (j*128+p)//32] ----
        gEp = []
        for j in range(4):
            ps = pA.tile([128, R], F32, tag="mm", name="mm")
            for c in range(2):
                nc.tensor.matmul(ps[:, c * C5:(c + 1) * C5],
                                 Gsel[:, j * 128:(j + 1) * 128], gET[:],
                                 start=True, stop=True)
            g = sgl.tile([128, R], F32, tag=f"gEp{j}", name=f"gEp{j}")
            nc.scalar.copy(g[:], ps[:])
            gEp.append(g)

        # ---- state: h0 = x0 @ Wh + bh, z0 = x0 @ Wz + bz ----
        h = wrk.tile([HID, R], F32, tag="h", name="h", bufs=2)
        z = wrk.tile([HID, R], F32, tag="z", name="z", bufs=2)
        h0p = pA.tile([HID, R], F32, tag="mm", name="mm")
        mm2(h0p, Wh, x0)
        nc.vector.tensor_scalar_add(h[:], h0p[:], bh[:])
        z0p = pA.tile([HID, R], F32, tag="mm", name="mm")
        mm2(z0p, Wz, x0)
        nc.vector.tensor_scalar_add(z[:], z0p[:], bz[:])

        def vfield(s4, hs, zs, kh, kz):
            u = 3 * (s4 // 4) + (s4 % 4)
            dxs = dxp.tile([IN, R], BF16, tag="dxs", name="dxs")
            nc.sync.dma_start(out=dxs[:], in_=d_pc[2 * u:2 * u + 2, :])
            # f path: two relu MLP layers + tanh head (i-major columns)
            x1p = pA.tile([HH, R], F32, tag="mm", name="mm")
            mm2(x1p, fw1, hs)
            x1 = wrk.tile([HH, R], F32, tag="fx", name="fx", bufs=2)
            act(x1[:], x1p[:], AF.Relu, bias=fb1[:])
            x2p = pA.tile([HH, R], F32, tag="mm", name="mm")
            mm2(x2p, fw2, x1)
            x2 = wrk.tile([HH, R], F32, tag="fx", name="fx", bufs=2)
            act(x2[:], x2p[:], AF.Relu, bias=fb2[:])
            vfp = pA.tile([HID * IN, R], F32, tag="mm", name="mm")
            mm2(vfp, fw3, x2)
            vf = wrk.tile([HID * IN, R], F32, tag="vf", name="vf")
            act(vf[:], vfp[:], AF.Tanh, bias=fb3[:])
            # dh = sum_i vf_i * dX_i  (dX broadcast via Bc, reduce via S3)
            dXb = pA.tile([IN * HID, R], F32, tag="mm", name="mm")
            mm2(dXb, Bc, dxs)
            nc.vector.tensor_mul(vf[:], vf[:], dXb[:])
            dhp = pB.tile([HID, R], F32, tag="acc", name="acc")
            mm2(dhp, S3, vf)
            nc.scalar.copy(kh[:], dhp[:])
            drp = pA.tile([128, R], F32, tag="mm", name="mm")
            mm2(drp, Erep, kh)
            dhrep = big1.tile([128, R], F32, tag="dhrep", name="dhrep")
            nc.scalar.copy(dhrep[:], drp[:])
            # g path: relu layer (feature-major), node-major transposes
            x1gp = pA.tile([HH, R], F32, tag="mm", name="mm")
            mm2(x1gp, gw1, zs)
            x1g = wrk.tile([HH, R], F32, tag="x1g", name="x1g")
            act(x1g[:], x1gp[:], AF.Relu, bias=gb1[:])
            xT = []
            for k_ in range(4):
                xtp = pT.tile([128, 2 * HH], F32, tag="pt", name="pt")
                for b_ in range(2):
                    nc.tensor.transpose(
                        xtp[:, b_ * HH:(b_ + 1) * HH],
                        x1g[:, b_ * N + k_ * 128: b_ * N + (k_ + 1) * 128],
                        ident[:HH, :HH])
                xt = wrk.tile([128, 2 * HH], F32, tag=f"xT{k_}",
                              name=f"xT{k_}")
                nc.vector.tensor_copy(xt[:], xtp[:])
                xT.append(xt)
            # graph conv: xg1 = A @ x1g per batch, recip folded in
            xg1n = []
            for i in range(4):
                xgp = pT.tile([128, 2 * HH], F32, tag="pt", name="pt")
                for k_ in range(4):
                    nc.tensor.matmul(xgp[:],
                                     expG[k_][:, i * 128:(i + 1) * 128],
                                     xT[k_][:],
                                     start=(k_ == 0), stop=(k_ == 3))
                xn = wrk.tile([128, 2 * HH], F32, tag=f"xg1n{i}",
                              name=f"xg1n{i}")
                nc.vector.tensor_scalar_mul(xn[:], xgp[:], recip[:, i:i + 1])
                xg1n.append(xn)
            xg1f = wrk.tile([HH, R], F32, tag="xg1f", name="xg1f")
            for i in range(4):
                for b_ in range(2):
                    btp = pT.tile([HH, 128], F32, tag="pt", name="pt")
                    nc.tensor.transpose(btp[:],
                                        xg1n[i][:, b_ * HH:(b_ + 1) * HH],
                                        ident[:, :])
                    nc.scalar.copy(
                        xg1f[:, b_ * N + i * 128: b_ * N + (i + 1) * 128],
                        btp[:])
            # per-node pooled weights: y = Wp^T xg scaled by gE_part,
            # reduced over EMB via S2 into x2g (abf preloaded via I32)
            x2gp = pB.tile([HH, R], F32, tag="acc", name="acc")
            for c in range(2):
                nc.tensor.matmul(x2gp[:, c * C5:(c + 1) * C5], I32[:],
                                 abT[:],
                                 start=True, stop=False, skip_group_check=True)
            for j in range(4):
                yp = pA.tile([128, R], F32, tag="mm", name="mm")
                for c in range(2):
                    sl = slice(c * C5, (c + 1) * C5)
                    nc.tensor.matmul(yp[:, sl], wpk0[:, j * 128:(j + 1) * 128],
                                     x1g[:, sl], start=True, stop=False)
                    nc.tensor.matmul(yp[:, sl], wpk1[:, j * 128:(j + 1) * 128],
                                     xg1f[:, sl], start=False, stop=True)
                t_ = big1.tile([128, R], F32, tag="ty", name="ty", bufs=2)
                nc.vector.tensor_mul(t_[:], yp[:], gEp[j][:])
                for c in range(2):
                    sl = slice(c * C5, (c + 1) * C5)
                    nc.tensor.matmul(x2gp[:, sl], S2[:], t_[:, sl],
                                     start=False, stop=(j == 3),
                                     skip_group_check=True)
            x2g = wrk.tile([HH, R], F32, tag="x2g", name="x2g")
            nc.scalar.copy(x2g[:], x2gp[:])
            # vg chunks; dz = sum vg_ho * dh_o accumulated via Sdz
            dzp = pB.tile([HID, R], F32, tag="acc", name="acc")
            for j in range(8):
                vgp = pA.tile([128, R], F32, tag="mm", name="mm")
                mm2(vgp, gwo[:, j * 128:(j + 1) * 128], x2g)
                vg = big2.tile([128, R], F32, tag="vg", name="vg")
                act(vg[:], vgp[:], AF.Tanh, bias=gbo[:, j:j + 1])
                nc.vector.tensor_mul(vg[:], vg[:], dhrep[:])
                for c in range(2):
                    sl = slice(c * C5, (c + 1) * C5)
                    nc.tensor.matmul(dzp[:, sl],
                                     Sdz[:, j * HID:(j + 1) * HID],
                                     vg[:, sl],
                                     start=(j == 0), stop=(j == 7),
                                     skip_group_check=True)
            nc.scalar.copy(kz[:], dzp[:])

        TT = nc.vector.tensor_tensor
        STT = nc.vector.scalar_tensor_tensor

        # RK4 with 3/8 rule, dt = 1 (times are arange; asserted on host)
        for s in range(nstep):
            kh = [wrk.tile([HID, R], F32, tag=f"kh{st}", name=f"kh{st}")
                  for st in range(4)]
            kz = [wrk.tile([HID, R], F32, tag=f"kz{st}", name=f"kz{st}")
                  for st in range(4)]
            vfield(4 * s + 0, h, z, kh[0], kz[0])
            hs = wrk.tile([HID, R], F32, tag="hs", name="hs", bufs=2)
            zs = wrk.tile([HID, R], F32, tag="zs", name="zs", bufs=2)
            STT(hs[:], kh[0][:], 1.0 / 3.0, h[:], op0=ALU.mult, op1=ALU.add)
            STT(zs[:], kz[0][:], 1.0 / 3.0, z[:], op0=ALU.mult, op1=ALU.add)
            vfield(4 * s + 1, hs, zs, kh[1], kz[1])
            hs2 = wrk.tile([HID, R], F32, tag="hs", name="hs", bufs=2)
            zs2 = wrk.tile([HID, R], F32, tag="zs", name="zs", bufs=2)
            STT(hs2[:], kh[0][:], -1.0 / 3.0, kh[1][:],
                op0=ALU.mult, op1=ALU.add)
            TT(hs2[:], hs2[:], h[:], op=ALU.add)
            STT(zs2[:], kz[0][:], -1.0 / 3.0, kz[1][:],
                op0=ALU.mult, op1=ALU.add)
            TT(zs2[:], zs2[:], z[:], op=ALU.add)
            vfield(4 * s + 2, hs2, zs2, kh[2], kz[2])
            hs3 = wrk.tile([HID, R], F32, tag="hs", name="hs", bufs=2)
            zs3 = wrk.tile([HID, R], F32, tag="zs", name="zs", bufs=2)
            STT(hs3[:], kh[1][:], -1.0, kh[0][:], op0=ALU.mult, op1=ALU.add)
            TT(hs3[:], hs3[:], kh[2][:], op=ALU.add)
            TT(hs3[:], hs3[:], h[:], op=ALU.add)
            STT(zs3[:], kz[1][:], -1.0, kz[0][:], op0=ALU.mult, op1=ALU.add)
            TT(zs3[:], zs3[:], kz[2][:], op=ALU.add)
            TT(zs3[:], zs3[:], z[:], op=ALU.add)
            vfield(4 * s + 3, hs3, zs3, kh[3], kz[3])
            hn = wrk.tile([HID, R], F32, tag="h", name="h", bufs=2)
            zn = wrk.tile([HID, R], F32, tag="z", name="z", bufs=2)
            TT(kh[1][:], kh[1][:], kh[2][:], op=ALU.add)
            STT(kh[1][:], kh[1][:], 3.0, kh[0][:], op0=ALU.mult, op1=ALU.add)
            TT(kh[1][:], kh[1][:], kh[3][:], op=ALU.add)
            STT(hn[:], kh[1][:], 0.125, h[:], op0=ALU.mult, op1=ALU.add)
            TT(kz[1][:], kz[1][:], kz[2][:], op=ALU.add)
            STT(kz[1][:], kz[1][:], 3.0, kz[0][:], op0=ALU.mult, op1=ALU.add)
            TT(kz[1][:], kz[1][:], kz[3][:], op=ALU.add)
            STT(zn[:], kz[1][:], 0.125, z[:], op0=ALU.mult, op1=ALU.add)
            h, z = hn, zn

        # ---- end conv ----
        op = pB.tile([OUT, R], F32, tag="acc", name="acc")
        mm2(op, cw, z)
        ob = wrk.tile([OUT, R], mybir.dt.bfloat16, tag="ob", name="ob")
        nc.vector.tensor_scalar_add(ob[:], op[:], cb[:])
        nc.sync.dma_start(out=d_out[:], in_=ob[:])

    _spill_excess_waits(nc)
    return nc


# ------------------------------------------------------------------
# host-side preprocessing
# ------------------------------------------------------------------
def host_inputs(a, nstep=NSTEP):
    gE = a["gE"]
    times = a["times"]
    assert np.allclose(np.diff(times), 1.0, atol=1e-5), "RK dt=1 baked in"
    maxlen = T - 2
    # unique stage times: u = 3s + st (stage 4 of step s == stage 0 of s+1)
    ts_list = [float(times[0])]
    for s in range(nstep):
        t0, t1 = float(times[s]), float(times[s + 1])
        dt = t1 - t0
        ts_list += [t0 + dt / 3.0, t0 + 2.0 * dt / 3.0, t1]
    nu = len(ts_list)
    dX = np.empty((B, N, nu, IN), np.float32)
    for q, t_ in enumerate(ts_list):
        idx = int(np.clip(np.sum(np.float32(t_) > times) - 1, 0, maxlen))
        frac = np.float32(t_ - times[idx])
        dX[:, :, q, :] = (a["coeff_b"][:, :, idx]
                          + (a["coeff_c2"][:, :, idx]
                             + a["coeff_d3"][:, :, idx] * frac) * frac)
    x0 = a["coeff_a"][:, :, 0, :]                   # (B, N, IN)

    fw3 = np.empty((HH, HID * IN), np.float32)
    fb3 = np.empty((HID * IN, 1), np.float32)
    for h_ in range(HID):
        for i in range(IN):
            fw3[:, i * HID + h_] = a["fWout"][:, h_ * IN + i]
            fb3[i * HID + h_, 0] = a["fbout"][h_ * IN + i]

    wpk = np.ascontiguousarray(
        np.transpose(a["gWpool"], (1, 2, 0, 3)).reshape(K, HH, EMB * HH))
    G = np.maximum(gE @ gE.T, 0.0).astype(np.float32)
    rs = np.exp(G).sum(axis=1)
    recip = np.ascontiguousarray((1.0 / rs).reshape(4, 128).T)

    vals = {
        "recip": recip, "gET": gE.T, "gbpool": a["gbpool"],
        "Wh": a["Wh"], "bh": a["bh"].reshape(-1, 1),
        "Wz": a["Wz"], "bz": a["bz"].reshape(-1, 1),
        "fw1": a["fWin"], "fb1": a["fbin"].reshape(-1, 1),
        "fw2": a["fWmid"], "fb2": a["fbmid"].reshape(-1, 1),
        "fw3": fw3, "fb3": fb3,
        "gw1": a["gWin"], "gb1": a["gbin"].reshape(-1, 1),
        "wpk0": wpk[0], "wpk1": wpk[1],
        "gwo": a["gWout"],
        "gbo": np.ascontiguousarray(a["gbout"].reshape(8, 128).T),
        "cw": np.ascontiguousarray(a["convW"].T),
        "cb": a["convb"].reshape(-1, 1),
    }
    wb = np.concatenate(
        [np.ascontiguousarray(vals[nm]).astype(np.float32).ravel()
         for nm, _ in _WSPEC]
        + [np.zeros(_WPAD - _WSIZE, np.float32)])
    assert wb.size == _WPAD
    wb_shards = wb.reshape(NCORES, _WSH)
    shared = {}

    per_core = []
    for c in range(NCORES):
        sl = slice(c * BS, (c + 1) * BS)
        dxc = np.transpose(dX[sl], (2, 3, 0, 1)).reshape(nu * IN, R)
        x0c = np.transpose(x0[sl], (2, 0, 1)).reshape(IN, R)
        per_core.append({
            "pc": np.ascontiguousarray(dxc).astype(ml_dtypes.bfloat16),
            "x0": np.ascontiguousarray(x0c).astype(np.float32),
            "wb": wb_shards[c],
        })
    return shared, per_core


_STATE = {}


def _get_nc():
    if "nc" not in _STATE:
        _STATE["nc"] = build_nc()
    return _STATE["nc"]


def _get_runner():
    """Cached jit(shard_map(bass_exec)) callable — built once so per-call
    cost is dispatch only (run_bass_kernel_spmd re-traces every call)."""
    if "runner" in _STATE:
        return _STATE["runner"]
    import jax
    from jax.sharding import Mesh, PartitionSpec
    from jax.experimental.shard_map import shard_map
    from concourse import bass2jax as b2j

    b2j.install_neuronx_cc_hook()
    nc = _get_nc()
    assert nc.dbg_addr is None
    partition_name = (nc.partition_id_tensor.name
                      if nc.partition_id_tensor else None)
    in_names, out_names, out_avals, zero_outs = [], [], [], []
    for alloc in nc.m.functions[0].allocations:
        if not isinstance(alloc, mybir.MemoryLocationSet):
            continue
        name = alloc.memorylocations[0].name
        if alloc.kind == "ExternalInput":
            if name != partition_name:
                in_names.append(name)
        elif alloc.kind == "ExternalOutput":
            shape = tuple(alloc.tensor_shape)
            dtype = mybir.dt.np(alloc.dtype)
            out_names.append(name)
            out_avals.append(jax.core.ShapedArray(shape, dtype))
            zero_outs.append(np.zeros((NCORES * shape[0], *shape[1:]), dtype))
    n_params = len(in_names)
    all_names = list(in_names) + list(out_names)
    if partition_name is not None:
        all_names.append(partition_name)

    def _body(*args):
        operands = list(args)
        if partition_name is not None:
            operands.append(b2j.partition_id_tensor())
        outs = b2j._bass_exec_p.bind(
            *operands,
            out_avals=tuple(out_avals),
            in_names=tuple(all_names),
            out_names=tuple(out_names),
            lowering_input_output_aliases=(),
            sim_require_finite=True,
            sim_require_nnan=True,
            nc=nc,
        )
        return tuple(outs)

    devices = jax.devices()[:NCORES]
    mesh = Mesh(np.asarray(devices), ("core",))
    n_outs = len(out_names)
    sharded = jax.jit(
        shard_map(_body, mesh=mesh,
                  in_specs=(PartitionSpec("core"),) * (n_params + n_outs),
                  out_specs=(PartitionSpec("core"),) * n_outs,
                  check_rep=False),
        donate_argnums=tuple(range(n_params, n_params + n_outs)),
        keep_unused=True,
    )

    from jax.sharding import NamedSharding
    shardspec = NamedSharding(mesh, PartitionSpec("core"))

    def _put_zeros():
        return [jax.device_put(z, shardspec) for z in zero_outs]

    def run(in_maps):
        concat_in = [
            np.concatenate([np.asarray(in_maps[c][nm])
                            for c in range(NCORES)], axis=0)
            for nm in in_names
        ]
        zs = _STATE.pop("zeros_dev", None)
        if zs is None:
            zs = _put_zeros()
        out_arrs = sharded(*concat_in, *zs)
        res = [
            {nm: np.asarray(out_arrs[i]).reshape(
                NCORES, *out_avals[i].shape)[c]
             for i, nm in enumerate(out_names)}
            for c in range(NCORES)
        ]
        # replenish asynchronously; only the enqueue is paid here
        _STATE["zeros_dev"] = [jax.device_put(z, shardspec)
                               for z in zero_outs]
        return res

    _STATE["runner"] = run
    return run


def _warm():
    """Trigger neuron compile + PJRT executable load with dummy inputs."""
    try:
        nc = _get_nc()
        a = {}
        a["times"] = np.arange(T, dtype=np.float32)
        for nm, sh in [("coeff_a", (B, N, T - 1, IN)),
                       ("coeff_b", (B, N, T - 1, IN)),
                       ("coeff_c2", (B, N, T - 1, IN)),
                       ("coeff_d3", (B, N, T - 1, IN)),
                       ("Wh", (IN, HID)), ("bh", (HID,)),
                       ("Wz", (IN, HID)), ("bz", (HID,)),
                       ("fWin", (HID, HH)), ("fbin", (HH,)),
                       ("fWmid", (HH, HH)), ("fbmid", (HH,)),
                       ("fWout", (HH, HID * IN)), ("fbout", (HID * IN,)),
                       ("gWin", (HID, HH)), ("gbin", (HH,)),
                       ("gE", (N, EMB)), ("gWpool", (EMB, K, HH, HH)),
                       ("gbpool", (EMB, HH)), ("gWout", (HH, HID * HID)),
                       ("gbout", (HID * HID,)), ("convW", (OUT, HID)),
                       ("convb", (OUT,))]:
            a[nm] = np.zeros(sh, np.float32)
        shared, per_core = host_inputs(a)
        in_maps = [{**shared, **pc} for pc in per_core]
        _get_runner()(in_maps)
        _STATE["warm"] = True
    except Exception as e:  # pragma: no cover - keep import usable
        import traceback
        traceback.print_exc()
        _STATE["warm_err"] = e


def kernel(**inputs):
    a = {k_: np.asarray(v, dtype=np.float32) for k_, v in inputs.items()}
    shared, per_core = host_inputs(a)
    in_maps = [{**shared, **pc} for pc in per_core]
    results = _get_runner()(in_maps)
    full = np.empty((B, 1, N, OUT), np.float32)
    for c in range(NCORES):
        o = np.asarray(results[c]["out"]).astype(np.float32)  # (OUT, R)
        full[c * BS:(c + 1) * BS, 0] = (
            o.reshape(OUT, BS, N).transpose(1, 2, 0))
    return full


_warm()
